# revision 1
# baseline (speedup 1.0000x reference)
"""Trainium2 Bass kernel for nn_BondConvLayer (gnn_message_passing).

8-core data-parallel: 2500 atoms (30000 bonds) per core.

out = softplus(bn2(softplus(bn1(cat @ W1.T)) @ W2.T)) * bw  where
cat = [center, gathered_nbr_atom, nbr_fea, rolled_nbr_fea, angle] per bond;
b1/b2 cancel inside training-mode BatchNorm and are dropped.

Device layout is transposed (features on partitions, bonds on free dim),
bonds ordered j-major so per-atom terms (center + angle projections) align
with bond tiles without broadcasts. The nbr_atom gather happens AFTER
projecting atom_fea by W1_n: every core builds the full 20000x128
projection table in SBUF as bf16 (row a at partition a%128, free
(a//128)*128), then dma_gather (SBUF source, transpose mode) pulls rows
per bond, emerging feature-major - directly addable to z1. The remaining
W1 blocks accumulate in PSUM (f/r from nbr tiles, center+angle via an
identity-matmul re-injection of the per-atom base). BN batch stats:
bn_stats/bn_aggr per core + tiny AllReduce. Softplus = Exp then Ln(x+1)
on ACT with the BN affine fused in. Matmuls run as float32r.
"""
import sys, os

sys.path.insert(0, "/opt/trn_rl_repo")

import numpy as np

import concourse.bass as bass
import concourse.bacc as bacc
import concourse.tile as tile
from concourse import mybir
from concourse.bass_utils import run_bass_kernel_spmd

F32 = mybir.dt.float32
F32R = mybir.dt.float32r
BF16 = mybir.dt.bfloat16
I16 = mybir.dt.int16
AF = mybir.ActivationFunctionType
ALU = mybir.AluOpType

NCORES = 8
N, M = 20000, 12
ATOM_F, NBR_F, ANG_F, A = 64, 64, 16, 66
H, O = 128, 64
BN_EPS = 1e-5
NLOC = N // NCORES          # 2500
NCHUNK = 5                  # chunks of 512 atoms (last ragged 452)
CW = 512
NPAD = NCHUNK * CW          # 2560
TAIL = NLOC - (NCHUNK - 1) * CW   # 452
NTILE = NCHUNK * M          # 60 bond tiles per core
KA = 9                      # angle K-chunks of 128 (1056 padded to 1152)
GCALLS = 3                  # gathers per chunk (4 slots x 512 idx each)
NRANK = (N + 127) // 128    # 157 table ranks
NFULL = NRANK * 128         # 20096 padded atoms
NSLAB = NTILE // 4          # 15 slabs of 4 bond-tiles

_CACHE = {}
TRACE = bool(int(os.environ.get("BASS_KERNEL_TRACE", "0")))
LAST_EXEC_NS = None
LAST_RESULTS = None


def _pin_act_tables():
    """Restrict the activation-table sets bacc may choose so Exp/Ln/Copy/
    Square all land in natural_log_exp_and_others (one load, no per-op
    table swaps). Set names/order (= act_func_set_id) are preserved."""
    if getattr(bacc, "_act_tables_pinned", False):
        return
    orig = bacc.get_activation_tables

    def pinned(arch):
        tabs = orig(arch)
        keep_all = "natural_log_exp_and_others"
        sqrt_home = "sqrt_and_others"
        strip = {AF.Exp, AF.Ln, AF.Copy, AF.Identity, AF.Square, AF.Sqrt}
        out = {}
        for name, funcs in tabs.items():
            if name == keep_all:
                out[name] = funcs
            elif name == sqrt_home:
                out[name] = {f for f in funcs
                             if f not in (strip - {AF.Sqrt})}
            else:
                out[name] = {f for f in funcs if f not in strip}
        return out

    bacc.get_activation_tables = pinned
    bacc._act_tables_pinned = True


def _build():
    if "nc" in _CACHE:
        return _CACHE["nc"]
    _pin_act_tables()
    import concourse.tile_utils as tile_utils
    tile_utils.max_sbuf_usage = 206 * 1024

    nc = bacc.Bacc("TRN2", target_bir_lowering=False, debug=False,
                   num_devices=NCORES)

    atom_full = nc.dram_tensor("atom_full", [64, NFULL], F32R, kind="ExternalInput").ap()
    atom_loc = nc.dram_tensor("atom_loc", [64, NPAD], F32R, kind="ExternalInput").ap()
    angle_t = nc.dram_tensor("angle_t", [KA * 128, NPAD], F32R, kind="ExternalInput").ap()
    nbr_t = nc.dram_tensor("nbr_t", [M, 64, NPAD], F32R, kind="ExternalInput").ap()
    idx_w = nc.dram_tensor("idx_w", [128, NCHUNK * GCALLS * 128], I16, kind="ExternalInput").ap()
    bw = nc.dram_tensor("bw", [M, NPAD], F32, kind="ExternalInput").ap()
    w1t_c = nc.dram_tensor("w1t_c", [64, 128], F32R, kind="ExternalInput").ap()
    w1t_nw = nc.dram_tensor("w1t_nw", [64, 256], F32R, kind="ExternalInput").ap()
    w1t_f = nc.dram_tensor("w1t_f", [64, 128], F32R, kind="ExternalInput").ap()
    w1t_r = nc.dram_tensor("w1t_r", [64, 128], F32R, kind="ExternalInput").ap()
    w1t_a = nc.dram_tensor("w1t_a", [KA * 128, 128], F32R, kind="ExternalInput").ap()
    w2t = nc.dram_tensor("w2t", [128, 64], F32R, kind="ExternalInput").ap()
    ident = nc.dram_tensor("ident", [128, 128], F32R, kind="ExternalInput").ap()
    g1 = nc.dram_tensor("g1", [128, 1], F32, kind="ExternalInput").ap()
    be1 = nc.dram_tensor("be1", [128, 1], F32, kind="ExternalInput").ap()
    g2 = nc.dram_tensor("g2", [64, 1], F32, kind="ExternalInput").ap()
    be2 = nc.dram_tensor("be2", [64, 1], F32, kind="ExternalInput").ap()
    out_p = nc.dram_tensor("out_p", [128, NTILE * 256], F32, kind="ExternalOutput").ap()

    with tile.TileContext(nc) as tc:
        with (tc.tile_pool(name="consts", bufs=1) as consts,
              tc.tile_pool(name="angle", bufs=3) as angle_pl,
              tc.tile_pool(name="nbr", bufs=4) as nbr_pl,
              tc.tile_pool(name="atom", bufs=2) as atom_pl,
              tc.tile_pool(name="gath", bufs=2) as gath_pl,
              tc.tile_pool(name="absb", bufs=2) as absb_pl,
              tc.tile_pool(name="slab", bufs=2) as slab_pl,
              tc.tile_pool(name="h1", bufs=2) as h1_pl,
              tc.tile_pool(name="tmpb", bufs=2) as tmpb_pl,
              tc.tile_pool(name="outc", bufs=2) as outc_pl,
              tc.tile_pool(name="bwbc", bufs=2) as bwbc_pl,
              tc.tile_pool(name="psA", bufs=3, space="PSUM") as psA,
              tc.tile_pool(name="psB", bufs=2, space="PSUM") as psB,
              tc.tile_pool(name="dram", bufs=1, space="DRAM") as dram):

            # ---------------- constants -------------------------------
            ident_sb = consts.tile([128, 128], F32R)
            nc.sync.dma_start(out=ident_sb, in_=ident)
            w1c_sb = consts.tile([64, 128], F32R)
            nc.sync.dma_start(out=w1c_sb, in_=w1t_c)
            w1nw_sb = consts.tile([64, 256], F32R)
            nc.sync.dma_start(out=w1nw_sb, in_=w1t_nw)
            w1f_sb = consts.tile([64, 128], F32R)
            nc.sync.dma_start(out=w1f_sb, in_=w1t_f)
            w1r_sb = consts.tile([64, 128], F32R)
            nc.sync.dma_start(out=w1r_sb, in_=w1t_r)
            w1a_sb = consts.tile([128, KA, 128], F32R)
            nc.sync.dma_start(
                out=w1a_sb,
                in_=bass.AP(tensor=w1t_a.tensor, offset=0,
                            ap=[[128, 128], [128 * 128, KA], [1, 128]]))
            w2t_sb = consts.tile([128, 64], F32R)
            nc.sync.dma_start(out=w2t_sb, in_=w2t)
            idx_sb = consts.tile([128, NCHUNK * GCALLS * 128], I16)
            nc.sync.dma_start(out=idx_sb, in_=idx_w)
            g1_sb = consts.tile([128, 1], F32)
            nc.sync.dma_start(out=g1_sb, in_=g1)
            be1_sb = consts.tile([128, 1], F32)
            nc.sync.dma_start(out=be1_sb, in_=be1)
            g2_sb = consts.tile([64, 1], F32)
            nc.sync.dma_start(out=g2_sb, in_=g2)
            be2_sb = consts.tile([64, 1], F32)
            nc.sync.dma_start(out=be2_sb, in_=be2)
            eps_sb = consts.tile([128, 1], F32)
            nc.vector.memset(eps_sb, BN_EPS)

            stats1 = consts.tile([128, NTILE, 6], F32)
            stats2 = consts.tile([64, NTILE, 6], F32)
            z2sl = [consts.tile([128, 4, 256], F32, name=f"z2sl_{g}")
                    for g in range(NSLAB)]
            table_sb = consts.tile([128, NRANK * 128], BF16)

            z1sp = dram.tile([128, NTILE * 512], F32)

            # ------- phase 0: bf16 projection table in SBUF -----------
            # table row a lives at [a%128, (a//128)*128 : +128]
            for t in range(20):
                c0 = t * 1024
                cw = min(1024, NFULL - c0)
                if cw <= 0:
                    break
                a_sb = atom_pl.tile([64, 1024], F32R, tag="atom")
                nc.sync.dma_start(out=a_sb[:, 0:cw],
                                  in_=atom_full[:, c0:c0 + cw])
                nrk = cw // 128
                for k in range(0, nrk, 2):
                    r = t * 8 + k
                    npair = min(2, nrk - k)
                    ps = psA.tile([128, 512], F32, tag="slot")
                    for q in range(npair):
                        nc.tensor.matmul(ps[:, q * 256:q * 256 + 256],
                                         a_sb[:, (k + q) * 128:(k + q + 1) * 128],
                                         w1nw_sb[:], start=True, stop=True,
                                         skip_group_check=True)
                    nc.vector.tensor_copy(
                        out=table_sb[:, r * 128:(r + npair) * 128],
                        in_=ps[:].rearrange("p (a b) -> p a b", b=256)[:, 0:npair, 0:128])

            # ---------------- phase 1: z1 assembly + stats -------------
            for c in range(NCHUNK):
                valid = CW if c < NCHUNK - 1 else TAIL
                # per-atom base: center + angle -> psB bank -> SBUF (f32r)
                at_sb = atom_pl.tile([64, 1024], F32R, tag="atom")
                nc.sync.dma_start(out=at_sb[:, 0:CW],
                                  in_=atom_loc[:, c * CW:(c + 1) * CW])
                ab = psB.tile([128, 512], F32, tag="psB")
                nc.tensor.matmul(ab[:], w1c_sb[:], at_sb[:, 0:CW],
                                 start=True, stop=False)
                for k in range(KA):
                    an_sb = angle_pl.tile([128, CW], F32R, tag="angle")
                    nc.sync.dma_start(
                        out=an_sb,
                        in_=angle_t[k * 128:(k + 1) * 128, c * CW:(c + 1) * CW])
                    nc.tensor.matmul(ab[:], w1a_sb[:, k, :], an_sb[:],
                                     start=False, stop=(k == KA - 1))
                ab_sb = absb_pl.tile([128, 512], F32R, tag="absb")
                nc.scalar.copy(out=ab_sb[:], in_=ab[:])
                # gathers for this chunk (4 slots per 2048-idx call)
                gts = []
                for g in range(GCALLS):
                    gt = gath_pl.tile([128, 1, 2048], BF16, tag="gath")
                    call = c * GCALLS + g
                    nc.gpsimd.dma_gather(
                        out_ap=gt[:], in_ap=table_sb[:],
                        idxs_ap=idx_sb[:, call * 128:(call + 1) * 128],
                        num_idxs=2048, num_idxs_reg=2048, elem_size=128,
                        transpose=True, single_packet=False,
                        sbuf_tokens_per_rank=128, sbuf_free_dim_per_rank=256)
                    gts.append(gt)
                z1slabs = [slab_pl.tile([128, 2048], F32, tag="slab",
                                        name=f"z1slab_{c}_{gi}")
                           for gi in range(GCALLS)]
                # slot pipeline: iter j loads nbr slot j%12; f->bank[j], r->bank[j-1]
                banks = {}
                for j in range(M + 1):
                    jm = j % M
                    nb_sb = nbr_pl.tile([64, CW], F32R, tag="nbr")
                    nc.sync.dma_start(
                        out=nb_sb, in_=nbr_t[jm, :, c * CW:(c + 1) * CW])
                    if j < M:
                        ps = psA.tile([128, 512], F32, tag="slot")
                        banks[j] = ps
                        nc.tensor.matmul(ps[:], w1f_sb[:], nb_sb[:],
                                         start=True, stop=False)
                        # inject per-atom base via identity matmul
                        nc.tensor.matmul(ps[:], ident_sb[:], ab_sb[:],
                                         start=False, stop=False)
                    if j >= 1:
                        s = j - 1
                        ps = banks.pop(s)
                        nc.tensor.matmul(ps[:], w1r_sb[:], nb_sb[:],
                                         start=False, stop=True)
                        z1t = z1slabs[s // 4][:, (s % 4) * 512:(s % 4 + 1) * 512]
                        nc.vector.scalar_tensor_tensor(
                            out=z1t, in0=ps[:], scalar=1.0,
                            in1=gts[s // 4][:, 0, (s % 4) * 512:(s % 4 + 1) * 512],
                            op0=ALU.mult, op1=ALU.add)
                        nc.vector.bn_stats(out=stats1[:, c * M + s, :],
                                           in_=z1t[:, 0:valid])
                        if s % 4 == 3:
                            g = s // 4
                            nc.sync.dma_start(
                                out=z1sp[:, (c * GCALLS + g) * 2048:
                                         (c * GCALLS + g + 1) * 2048],
                                in_=z1slabs[g][:])

            # ---------------- BN1 stats allreduce ----------------------
            mv1 = consts.tile([128, 2], F32)
            nc.vector.bn_aggr(out=mv1[:], in_=stats1[:])
            pay1 = consts.tile([128, 2], F32)
            msq1 = consts.tile([128, 1], F32)
            nc.scalar.square(out=msq1[:], in_=mv1[:, 0:1])
            nc.vector.tensor_copy(out=pay1[:, 0:1], in_=mv1[:, 0:1])
            nc.vector.tensor_add(out=pay1[:, 1:2], in0=mv1[:, 1:2], in1=msq1[:])
            cc1i = dram.tile([128, 2], F32)
            cc1o = dram.tile([128, 2], F32)
            nc.sync.dma_start(out=cc1i[:], in_=pay1[:])
            nc.gpsimd.collective_compute(
                "AllReduce", ALU.add, replica_groups=[list(range(NCORES))],
                ins=[cc1i[:].opt()], outs=[cc1o[:].opt()])
            S1 = consts.tile([128, 2], F32)
            nc.sync.dma_start(out=S1[:], in_=cc1o[:])
            mean1 = consts.tile([128, 1], F32)
            nc.scalar.mul(out=mean1[:], in_=S1[:, 0:1], mul=1.0 / NCORES)
            mm1 = consts.tile([128, 1], F32)
            nc.scalar.square(out=mm1[:], in_=mean1[:])
            var1 = consts.tile([128, 1], F32)
            nc.vector.scalar_tensor_tensor(
                out=var1[:], in0=S1[:, 1:2], scalar=1.0 / NCORES, in1=mm1[:],
                op0=ALU.mult, op1=ALU.subtract)
            sd1 = consts.tile([128, 1], F32)
            nc.scalar.activation(out=sd1[:], in_=var1[:], func=AF.Sqrt,
                                 bias=eps_sb[:], scale=1.0)
            rs1 = consts.tile([128, 1], F32)
            nc.vector.reciprocal(out=rs1[:], in_=sd1[:])
            scale1 = consts.tile([128, 1], F32)
            nc.vector.tensor_mul(out=scale1[:], in0=rs1[:], in1=g1_sb[:])
            negm1 = consts.tile([128, 1], F32)
            nc.scalar.mul(out=negm1[:], in_=mean1[:], mul=-1.0)
            bias1 = consts.tile([128, 1], F32)
            nc.vector.scalar_tensor_tensor(
                out=bias1[:], in0=scale1[:], scalar=negm1[:], in1=be1_sb[:],
                op0=ALU.mult, op1=ALU.add)

            # ---------------- phase 2: h1, z2, stats2 (4-tile slabs) ---
            for g in range(NSLAB):
                zsl = slab_pl.tile([128, 2048], F32, tag="slab")
                nc.sync.dma_start(out=zsl[:],
                                  in_=z1sp[:, g * 2048:(g + 1) * 2048])
                nc.scalar.activation(out=zsl[:], in_=zsl[:], func=AF.Exp,
                                     bias=bias1[:], scale=scale1[:])
                h1s = h1_pl.tile([128, 2048], F32R, tag="h1")
                nc.scalar.activation(out=h1s[:], in_=zsl[:], func=AF.Ln,
                                     bias=1.0)
                tb = tmpb_pl.tile([64, 4, 256], F32, tag="tmpb")
                for k in range(4):
                    t = g * 4 + k
                    c = t // M
                    valid = 512 if c < NCHUNK - 1 else TAIL
                    ps = psB.tile([128, 512], F32, tag="psB")
                    nc.tensor.matmul(ps[0:64, :], w2t_sb[:],
                                     h1s[:, k * 512:(k + 1) * 512],
                                     start=True, stop=True)
                    nc.vector.bn_stats(out=stats2[:, t, :],
                                       in_=ps[0:64, 0:valid])
                    nc.vector.tensor_copy(
                        out=z2sl[g][0:64, k, :], in_=ps[0:64, 0:256])
                    nc.vector.tensor_copy(out=tb[:, k, :],
                                          in_=ps[0:64, 256:512])
                nc.sync.dma_start(
                    out=z2sl[g][64:128, :, :].rearrange("p a b -> p (a b)"),
                    in_=tb[:].rearrange("p a b -> p (a b)"))

            # ---------------- BN2 stats allreduce ----------------------
            mv2 = consts.tile([64, 2], F32)
            nc.vector.bn_aggr(out=mv2[:], in_=stats2[:])
            pay2 = consts.tile([64, 2], F32)
            msq2 = consts.tile([64, 1], F32)
            nc.scalar.square(out=msq2[:], in_=mv2[:, 0:1])
            nc.vector.tensor_copy(out=pay2[:, 0:1], in_=mv2[:, 0:1])
            nc.vector.tensor_add(out=pay2[:, 1:2], in0=mv2[:, 1:2], in1=msq2[:])
            cc2i = dram.tile([64, 2], F32)
            cc2o = dram.tile([64, 2], F32)
            nc.sync.dma_start(out=cc2i[:], in_=pay2[:])
            nc.gpsimd.collective_compute(
                "AllReduce", ALU.add, replica_groups=[list(range(NCORES))],
                ins=[cc2i[:].opt()], outs=[cc2o[:].opt()])
            S2t = consts.tile([64, 2], F32)
            nc.sync.dma_start(out=S2t[:], in_=cc2o[:])
            mean2 = consts.tile([64, 1], F32)
            nc.scalar.mul(out=mean2[:], in_=S2t[:, 0:1], mul=1.0 / NCORES)
            mm2 = consts.tile([64, 1], F32)
            nc.scalar.square(out=mm2[:], in_=mean2[:])
            var2 = consts.tile([64, 1], F32)
            nc.vector.scalar_tensor_tensor(
                out=var2[:], in0=S2t[:, 1:2], scalar=1.0 / NCORES, in1=mm2[:],
                op0=ALU.mult, op1=ALU.subtract)
            sd2 = consts.tile([64, 1], F32)
            nc.scalar.activation(out=sd2[:], in_=var2[:], func=AF.Sqrt,
                                 bias=eps_sb[0:64, :], scale=1.0)
            rs2 = consts.tile([64, 1], F32)
            nc.vector.reciprocal(out=rs2[:], in_=sd2[:])
            scale2 = consts.tile([64, 1], F32)
            nc.vector.tensor_mul(out=scale2[:], in0=rs2[:], in1=g2_sb[:])
            negm2 = consts.tile([64, 1], F32)
            nc.scalar.mul(out=negm2[:], in_=mean2[:], mul=-1.0)
            bias2 = consts.tile([64, 1], F32)
            nc.vector.scalar_tensor_tensor(
                out=bias2[:], in0=scale2[:], scalar=negm2[:], in1=be2_sb[:],
                op0=ALU.mult, op1=ALU.add)
            scale2r = consts.tile([128, 1], F32)
            nc.sync.dma_start(out=scale2r[0:64, :], in_=scale2[:])
            nc.sync.dma_start(out=scale2r[64:128, :], in_=scale2[:])
            bias2r = consts.tile([128, 1], F32)
            nc.sync.dma_start(out=bias2r[0:64, :], in_=bias2[:])
            nc.sync.dma_start(out=bias2r[64:128, :], in_=bias2[:])

            # ---------------- phase 3: softplus2 * bw -> out -----------
            for g in range(NSLAB):
                c, s0 = divmod(g * 4, M)
                zf = z2sl[g][:].rearrange("p a b -> p (a b)")
                nc.scalar.activation(out=zf, in_=zf, func=AF.Exp,
                                     bias=bias2r[:], scale=scale2r[:])
                nc.scalar.activation(out=zf, in_=zf, func=AF.Ln, bias=1.0)
                bwt = bwbc_pl.tile([128, 4, 256], F32, tag="bwbc")
                off = c * CW + s0 * NPAD
                nc.sync.dma_start(
                    out=bwt[0:64, :, :],
                    in_=bass.AP(tensor=bw.tensor, offset=off,
                                ap=[[0, 64], [NPAD, 4], [1, 256]]))
                nc.sync.dma_start(
                    out=bwt[64:128, :, :],
                    in_=bass.AP(tensor=bw.tensor, offset=off + 256,
                                ap=[[0, 64], [NPAD, 4], [1, 256]]))
                oc = outc_pl.tile([128, 4, 256], F32, tag="outc")
                nc.vector.tensor_mul(
                    out=oc[:].rearrange("p a b -> p (a b)"), in0=zf,
                    in1=bwt[:].rearrange("p a b -> p (a b)"))
                nc.sync.dma_start(
                    out=out_p[:, g * 1024:(g + 1) * 1024],
                    in_=oc[:].rearrange("p a b -> p (a b)"))

    nc.compile()
    _CACHE["nc"] = nc
    return nc


def _prep_core(c, atom_fea, nbr_fea, nbr_fea_idx, angle_fea, bond_weights,
               W1, W2, g1, be1, g2, be2, shared):
    lo = c * NLOC
    hi = lo + NLOC
    angle_t = np.zeros((KA * 128, NPAD), np.float32)
    angle_t[:A * ANG_F, :NLOC] = angle_fea[lo:hi].reshape(NLOC, A * ANG_F).T
    nbr_t = np.zeros((M, 64, NPAD), np.float32)
    nbr_t[:, :, :NLOC] = nbr_fea[lo:hi].transpose(1, 2, 0)
    bw_p = np.zeros((M, NPAD), np.float32)
    bw_p[:, :NLOC] = bond_weights[lo:hi].T

    idxp = np.zeros((NPAD, M), np.int16)
    idxp[:NLOC] = nbr_fea_idx[lo:hi].astype(np.int16)
    idx_w = np.zeros((128, NCHUNK * GCALLS * 128), np.int16)
    for cc in range(NCHUNK):
        blk = idxp[cc * CW:(cc + 1) * CW, :]
        for g in range(GCALLS):
            flat = blk[:, 4 * g:4 * g + 4].T.reshape(-1)
            wr = flat.reshape(128, 16).T
            col = (cc * GCALLS + g) * 128
            idx_w[:, col:col + 128] = np.tile(wr, (8, 1))

    atom_loc = np.zeros((64, NPAD), np.float32)
    atom_loc[:, :NLOC] = atom_fea[lo:hi].T
    d = dict(shared)
    d.update(atom_loc=atom_loc, angle_t=angle_t, nbr_t=nbr_t, idx_w=idx_w,
             bw=bw_p)
    return d


def _make_in_maps(inputs):
    """Build per-core input dicts from the full (unsharded) input dict."""
    atom_fea = np.asarray(inputs["atom_fea"], dtype=np.float32)
    nbr_fea = np.asarray(inputs["nbr_fea"], dtype=np.float32)
    nbr_fea_idx = np.asarray(inputs["nbr_fea_idx"])
    angle_fea = np.asarray(inputs["angle_fea"], dtype=np.float32)
    bond_weights = np.asarray(inputs["bond_weights"], dtype=np.float32)
    W1 = np.asarray(inputs["W1"]); W2 = np.asarray(inputs["W2"])
    g1 = np.asarray(inputs["g1"]); be1 = np.asarray(inputs["be1"])
    g2 = np.asarray(inputs["g2"]); be2 = np.asarray(inputs["be2"])

    atom_full = np.zeros((64, NFULL), np.float32)
    atom_full[:, :N] = atom_fea.T
    w1t = W1.T.astype(np.float32)
    w1t_a = np.zeros((KA * 128, 128), np.float32)
    w1t_a[:A * ANG_F] = w1t[256:1312]
    w1t_nw = np.zeros((64, 256), np.float32)
    w1t_nw[:, :128] = w1t[64:128]
    shared = dict(
        atom_full=atom_full,
        w1t_c=np.ascontiguousarray(w1t[0:64]),
        w1t_nw=w1t_nw,
        w1t_f=np.ascontiguousarray(w1t[128:192]),
        w1t_r=np.ascontiguousarray(w1t[192:256]),
        w1t_a=w1t_a,
        w2t=np.ascontiguousarray(W2.T.astype(np.float32)),
        ident=np.eye(128, dtype=np.float32),
        g1=g1.reshape(128, 1).astype(np.float32),
        be1=be1.reshape(128, 1).astype(np.float32),
        g2=g2.reshape(64, 1).astype(np.float32),
        be2=be2.reshape(64, 1).astype(np.float32),
    )
    return [_prep_core(c, atom_fea, nbr_fea, nbr_fea_idx, angle_fea,
                       bond_weights, W1, W2, g1, be1, g2, be2, shared)
            for c in range(NCORES)]


def kernel(atom_fea, nbr_fea, nbr_fea_idx, angle_fea, bond_weights,
           W1, b1, g1, be1, W2, b2, g2, be2):
    global LAST_EXEC_NS, LAST_RESULTS
    nc = _build()
    in_maps = _make_in_maps(dict(
        atom_fea=atom_fea, nbr_fea=nbr_fea, nbr_fea_idx=nbr_fea_idx,
        angle_fea=angle_fea, bond_weights=bond_weights, W1=W1, W2=W2,
        g1=g1, be1=be1, g2=g2, be2=be2))

    if TRACE:
        _install_ntff_hook()
    br = run_bass_kernel_spmd(nc, in_maps, list(range(NCORES)), trace=TRACE)
    LAST_EXEC_NS = br.exec_time_ns
    LAST_RESULTS = br

    out = np.empty((N, M, NBR_F), np.float32)
    for c in range(NCORES):
        op = br.results[c]["out_p"]
        lo = c * NLOC
        for t in range(NTILE):
            cc, s = divmod(t, M)
            blk = op[:, t * 256:(t + 1) * 256]
            a0 = cc * CW
            nA = min(256, NLOC - a0)
            if nA > 0:
                out[lo + a0:lo + a0 + nA, s, :] = blk[0:64, :nA].T
            b0 = a0 + 256
            nB = min(256, NLOC - b0)
            if nB > 0:
                out[lo + b0:lo + b0 + nB, s, :] = blk[64:128, :nB].T
    return out


def _install_ntff_hook():
    """Inject antenv.axon_hooks (missing in this image) so trace=True works."""
    import types
    if "antenv.axon_hooks" in sys.modules:
        return
    sys.path.insert(0, "/root/.axon_site")
    mod = types.ModuleType("antenv.axon_hooks")
    mod._hook = None
    mod.set_axon_ntff_profile_hook = lambda h: setattr(mod, "_hook", h)
    mod.get_axon_ntff_profile_hook = lambda: mod._hook
    sys.modules["antenv.axon_hooks"] = mod
    try:
        from trn_agent_boot.trn_boot import _ntff_profile_via_ctypes
        h = _ntff_profile_via_ctypes("/opt/axon/libaxon_pjrt.so")
        if h is not None:
            mod.set_axon_ntff_profile_hook(h)
    except Exception as e:
        print("ntff hook install failed:", e)



# revision 4
# speedup vs baseline: 1.2784x; 1.2784x over previous
"""Trainium2 Bass kernel for nn_BondConvLayer (gnn_message_passing).

8-core data-parallel: 2500 atoms (30000 bonds) per core.

out = softplus(bn2(softplus(bn1(cat @ W1.T)) @ W2.T)) * bw  where
cat = [center, gathered_nbr_atom, nbr_fea, rolled_nbr_fea, angle] per bond;
b1/b2 cancel inside training-mode BatchNorm and are dropped.

v2 layout: everything bf16 on the wire and in the PE. Each core projects
the full atom table atom-major ([atom, 128h] rows, one matmul per
128-atom rank) and stores it to DRAM; per-bond rows are then pulled with
a DRAM-source non-transpose dma_gather (contiguous 256B descriptors -
fast path) arriving bond-major, and re-transposed into the feature-major
z1 PSUM accumulation with identity matmuls on the PE (stat=G block,
mov=I, start=False). nbr f/r projections run as one 128-deep stacked
matmul per slot ([W1f;W1r] weights, one 128-partition DMA spanning
adjacent nbr slots via a wraparound row). The per-atom center+angle base
is added on DVE during PSUM evacuation (scalar_tensor_tensor), which
also casts z1 to bf16 slabs kept in SBUF (no DRAM spill). BN batch
stats: bn_stats/bn_aggr per core + tiny AllReduce; phase 2 runs W2 as
two half-partition matmuls per PSUM bank (tile_position) so softplus /
stats / output work on full 128-partition tiles. Softplus = Exp then
Ln(x+1) on ACT with the BN affine fused in.
"""
import sys, os

sys.path.insert(0, "/opt/trn_rl_repo")

import numpy as np

import concourse.bass as bass
import concourse.bacc as bacc
import concourse.tile as tile
from concourse import mybir
from concourse.bass_utils import run_bass_kernel_spmd

F32 = mybir.dt.float32
BF16 = mybir.dt.bfloat16
I16 = mybir.dt.int16
AF = mybir.ActivationFunctionType
ALU = mybir.AluOpType
BF16_NP = mybir.dt.np(BF16)

NCORES = 8
N, M = 20000, 12
ATOM_F, NBR_F, ANG_F, A = 64, 64, 16, 66
H, O = 128, 64
BN_EPS = 1e-5
NLOC = N // NCORES          # 2500
NCHUNK = 5                  # chunks of 512 atoms (last ragged 452)
CW = 512
NPAD = NCHUNK * CW          # 2560
TAIL = NLOC - (NCHUNK - 1) * CW   # 452
NTILE = NCHUNK * M          # 60 bond tiles per core
NPAIR = NTILE // 2          # 30 paired tiles in phase 2/3
NSLAB = NTILE // 4          # 15 slabs of 4 bond-tiles
KA = 9                      # angle K-chunks of 128 (1056 padded to 1152)
NRANK = 160                 # table ranks (20000 atoms padded to 20480)
NFULL = NRANK * 128         # 20480
NIDX = M * CW               # 6144 gather indices per chunk (one call)
TGRP = 20                   # table build groups of 8 ranks

_CACHE = {}
TRACE = bool(int(os.environ.get("BASS_KERNEL_TRACE", "0")))
LAST_EXEC_NS = None
LAST_RESULTS = None


def _pin_act_tables():
    """Restrict the activation-table sets bacc may choose so Exp/Ln/Copy/
    Square all land in natural_log_exp_and_others (one load, no per-op
    table swaps). Set names/order (= act_func_set_id) are preserved."""
    if getattr(bacc, "_act_tables_pinned", False):
        return
    orig = bacc.get_activation_tables

    def pinned(arch):
        tabs = orig(arch)
        keep_all = "natural_log_exp_and_others"
        sqrt_home = "sqrt_and_others"
        strip = {AF.Exp, AF.Ln, AF.Copy, AF.Identity, AF.Square, AF.Sqrt}
        out = {}
        for name, funcs in tabs.items():
            if name == keep_all:
                out[name] = funcs
            elif name == sqrt_home:
                out[name] = {f for f in funcs
                             if f not in (strip - {AF.Sqrt})}
            else:
                out[name] = {f for f in funcs if f not in strip}
        return out

    bacc.get_activation_tables = pinned
    bacc._act_tables_pinned = True


def _build():
    if "nc" in _CACHE:
        return _CACHE["nc"]
    _pin_act_tables()
    import concourse.tile_utils as tile_utils
    tile_utils.max_sbuf_usage = 206 * 1024

    nc = bacc.Bacc("TRN2", target_bir_lowering=False, debug=False,
                   num_devices=NCORES)

    atom_fullT = nc.dram_tensor("atom_fullT", [64, NFULL], BF16, kind="ExternalInput").ap()
    atom_locT = nc.dram_tensor("atom_locT", [64, NPAD], BF16, kind="ExternalInput").ap()
    angle_t = nc.dram_tensor("angle_t", [KA * 128, NPAD], BF16, kind="ExternalInput").ap()
    nbr_t = nc.dram_tensor("nbr_t", [(M + 1) * 64, NPAD], BF16, kind="ExternalInput").ap()
    idx_w = nc.dram_tensor("idx_w", [128, NCHUNK * (NIDX // 16)], I16, kind="ExternalInput").ap()
    bw = nc.dram_tensor("bw", [M, NPAD], BF16, kind="ExternalInput").ap()
    w1t_c = nc.dram_tensor("w1t_c", [64, 128], BF16, kind="ExternalInput").ap()
    w1t_n = nc.dram_tensor("w1t_n", [64, 128], BF16, kind="ExternalInput").ap()
    w1t_fr = nc.dram_tensor("w1t_fr", [128, 128], BF16, kind="ExternalInput").ap()
    w1t_a = nc.dram_tensor("w1t_a", [KA * 128, 128], BF16, kind="ExternalInput").ap()
    w2t = nc.dram_tensor("w2t", [128, 64], BF16, kind="ExternalInput").ap()
    ident = nc.dram_tensor("ident", [128, 128], BF16, kind="ExternalInput").ap()
    g1 = nc.dram_tensor("g1", [128, 1], F32, kind="ExternalInput").ap()
    be1 = nc.dram_tensor("be1", [128, 1], F32, kind="ExternalInput").ap()
    g2 = nc.dram_tensor("g2", [64, 1], F32, kind="ExternalInput").ap()
    be2 = nc.dram_tensor("be2", [64, 1], F32, kind="ExternalInput").ap()
    out_p = nc.dram_tensor("out_p", [128, NPAIR * 512], F32, kind="ExternalOutput").ap()

    with tile.TileContext(nc) as tc:
        with (tc.tile_pool(name="consts", bufs=1) as consts,
              tc.tile_pool(name="astr", bufs=3) as astr_pl,
              tc.tile_pool(name="tb", bufs=2) as tb_pl,
              tc.tile_pool(name="atom", bufs=2) as atom_pl,
              tc.tile_pool(name="angle", bufs=2) as angle_pl,
              tc.tile_pool(name="nbr", bufs=4) as nbr_pl,
              tc.tile_pool(name="gath", bufs=2) as gath_pl,
              tc.tile_pool(name="absb", bufs=2) as absb_pl,
              tc.tile_pool(name="h1", bufs=2) as h1_pl,
              tc.tile_pool(name="sp", bufs=2) as sp_pl,
              tc.tile_pool(name="bwbc", bufs=NSLAB) as bwbc_pl,
              tc.tile_pool(name="psA", bufs=4, space="PSUM") as psA,
              tc.tile_pool(name="psB", bufs=2, space="PSUM") as psB,
              tc.tile_pool(name="dram", bufs=1, space="DRAM") as dram):

            # ---------------- constants -------------------------------
            ident_sb = consts.tile([128, 128], BF16)
            nc.sync.dma_start(out=ident_sb, in_=ident)
            w1c_sb = consts.tile([64, 128], BF16)
            nc.sync.dma_start(out=w1c_sb, in_=w1t_c)
            w1n_sb = consts.tile([64, 128], BF16)
            nc.sync.dma_start(out=w1n_sb, in_=w1t_n)
            w1fr_sb = consts.tile([128, 128], BF16)
            nc.sync.dma_start(out=w1fr_sb, in_=w1t_fr)
            w1a_sb = consts.tile([128, KA, 128], BF16)
            nc.sync.dma_start(
                out=w1a_sb,
                in_=bass.AP(tensor=w1t_a.tensor, offset=0,
                            ap=[[128, 128], [128 * 128, KA], [1, 128]]))
            w2t_sb = consts.tile([128, 64], BF16)
            nc.sync.dma_start(out=w2t_sb, in_=w2t)
            idx_sb = consts.tile([128, NCHUNK * (NIDX // 16)], I16)
            nc.sync.dma_start(out=idx_sb, in_=idx_w)
            g1_sb = consts.tile([128, 1], F32)
            nc.sync.dma_start(out=g1_sb, in_=g1)
            be1_sb = consts.tile([128, 1], F32)
            nc.sync.dma_start(out=be1_sb, in_=be1)
            g2_sb = consts.tile([64, 1], F32)
            nc.sync.dma_start(out=g2_sb, in_=g2)
            be2_sb = consts.tile([64, 1], F32)
            nc.sync.dma_start(out=be2_sb, in_=be2)
            eps_sb = consts.tile([128, 1], F32)
            nc.vector.memset(eps_sb, BN_EPS)

            stats1 = consts.tile([128, NTILE, 6], F32)
            stats2 = consts.tile([128, NPAIR, 6], F32)
            z1_sb = consts.tile([128, NTILE, 512], BF16)
            z2_sb = consts.tile([128, NPAIR, 512], BF16)

            table_d = dram.tile([NFULL, 128], BF16)
            table_ap = bass.AP(tensor=table_d.tensor, offset=0,
                               ap=[[128, NFULL], [1, 128]])

            # ------- phase 0: bf16 projection table in DRAM -----------
            # table row a = atom_fea[a] @ W1n.T, built atom-major: one
            # matmul per 128-atom rank (atoms land on partitions), so the
            # store to DRAM is a plain contiguous-row DMA.
            for grp in range(TGRP):
                a_sb = astr_pl.tile([64, 1024], BF16, tag="astr")
                nc.sync.dma_start(out=a_sb,
                                  in_=atom_fullT[:, grp * 1024:(grp + 1) * 1024])
                tb = tb_pl.tile([128, 8, 128], BF16, tag="tb")
                for half in range(2):
                    ps = psA.tile([128, 512], F32, tag="slot")
                    for k in range(4):
                        nc.tensor.matmul(
                            ps[:, k * 128:(k + 1) * 128],
                            a_sb[:, (half * 4 + k) * 128:(half * 4 + k + 1) * 128],
                            w1n_sb[:], start=True, stop=True,
                            skip_group_check=True)
                    nc.scalar.copy(
                        out=tb[:, half * 4:half * 4 + 4, :].rearrange(
                            "p a b -> p (a b)"),
                        in_=ps[:])
                nc.sync.dma_start(
                    out=bass.AP(tensor=table_d.tensor,
                                offset=grp * 1024 * 128,
                                ap=[[128, 128], [128 * 128, 8], [1, 128]]),
                    in_=tb[:])

            # ---------------- phase 1: z1 assembly + stats -------------
            for c in range(NCHUNK):
                valid = CW if c < NCHUNK - 1 else TAIL
                # per-atom base: center + angle -> psB bank -> SBUF bf16
                at_sb = atom_pl.tile([64, CW], BF16, tag="atom")
                nc.sync.dma_start(out=at_sb,
                                  in_=atom_locT[:, c * CW:(c + 1) * CW])
                ab = psB.tile([128, 512], F32, tag="psB")
                nc.tensor.matmul(ab[:], w1c_sb[:], at_sb[:],
                                 start=True, stop=False)
                an_sb = angle_pl.tile([128, KA, CW], BF16, tag="angle")
                nc.sync.dma_start(
                    out=an_sb,
                    in_=bass.AP(tensor=angle_t.tensor, offset=c * CW,
                                ap=[[NPAD, 128], [128 * NPAD, KA], [1, CW]]))
                for k in range(KA):
                    nc.tensor.matmul(ab[:], w1a_sb[:, k, :], an_sb[:, k, :],
                                     start=False, stop=(k == KA - 1))
                ab_sb = absb_pl.tile([128, 512], BF16, tag="absb")
                nc.scalar.copy(out=ab_sb[:], in_=ab[:])
                # one bond-major gather for the whole chunk (6144 rows)
                gt = gath_pl.tile([128, NIDX // 128, 128], BF16, tag="gath")
                nc.gpsimd.dma_gather(
                    out_ap=gt[:], in_ap=table_ap,
                    idxs_ap=idx_sb[:, c * (NIDX // 16):(c + 1) * (NIDX // 16)],
                    num_idxs=NIDX, num_idxs_reg=NIDX, elem_size=128,
                    transpose=False, single_packet=False)
                for j in range(M):
                    # stacked [nbr_j; nbr_{j+1}] via one 128-partition DMA
                    # (row M is a host-side copy of row 0 for wraparound)
                    nt = nbr_pl.tile([128, CW], BF16, tag="nbr")
                    nc.sync.dma_start(
                        out=nt,
                        in_=bass.AP(tensor=nbr_t.tensor,
                                    offset=j * 64 * NPAD + c * CW,
                                    ap=[[NPAD, 128], [1, CW]]))
                    ps = psA.tile([128, 512], F32, tag="slot")
                    nc.tensor.matmul(ps[:], w1fr_sb[:], nt[:],
                                     start=True, stop=False)
                    # transpose-inject gathered nbr_atom rows: G_block.T
                    for k in range(4):
                        nc.tensor.matmul(ps[:, k * 128:(k + 1) * 128],
                                         gt[:, 4 * j + k, :], ident_sb[:],
                                         start=False, stop=(k == 3))
                    t = c * M + j
                    z1t = z1_sb[:, t, :]
                    nc.vector.scalar_tensor_tensor(
                        out=z1t, in0=ps[:], scalar=1.0, in1=ab_sb[:],
                        op0=ALU.mult, op1=ALU.add)
                    nc.vector.bn_stats(out=stats1[:, t, :],
                                       in_=z1t[:, 0:valid])

            # ---------------- BN1 stats allreduce ----------------------
            mv1 = consts.tile([128, 2], F32)
            nc.vector.bn_aggr(out=mv1[:], in_=stats1[:])
            pay1 = consts.tile([128, 2], F32)
            msq1 = consts.tile([128, 1], F32)
            nc.scalar.square(out=msq1[:], in_=mv1[:, 0:1])
            nc.vector.tensor_copy(out=pay1[:, 0:1], in_=mv1[:, 0:1])
            nc.vector.tensor_add(out=pay1[:, 1:2], in0=mv1[:, 1:2], in1=msq1[:])
            cc1i = dram.tile([128, 2], F32)
            cc1o = dram.tile([128, 2], F32)
            nc.sync.dma_start(out=cc1i[:], in_=pay1[:])
            nc.gpsimd.collective_compute(
                "AllReduce", ALU.add, replica_groups=[list(range(NCORES))],
                ins=[cc1i[:].opt()], outs=[cc1o[:].opt()])
            S1 = consts.tile([128, 2], F32)
            nc.sync.dma_start(out=S1[:], in_=cc1o[:])
            mean1 = consts.tile([128, 1], F32)
            nc.scalar.mul(out=mean1[:], in_=S1[:, 0:1], mul=1.0 / NCORES)
            mm1 = consts.tile([128, 1], F32)
            nc.scalar.square(out=mm1[:], in_=mean1[:])
            var1 = consts.tile([128, 1], F32)
            nc.vector.scalar_tensor_tensor(
                out=var1[:], in0=S1[:, 1:2], scalar=1.0 / NCORES, in1=mm1[:],
                op0=ALU.mult, op1=ALU.subtract)
            sd1 = consts.tile([128, 1], F32)
            nc.scalar.activation(out=sd1[:], in_=var1[:], func=AF.Sqrt,
                                 bias=eps_sb[:], scale=1.0)
            rs1 = consts.tile([128, 1], F32)
            nc.vector.reciprocal(out=rs1[:], in_=sd1[:])
            scale1 = consts.tile([128, 1], F32)
            nc.vector.tensor_mul(out=scale1[:], in0=rs1[:], in1=g1_sb[:])
            negm1 = consts.tile([128, 1], F32)
            nc.scalar.mul(out=negm1[:], in_=mean1[:], mul=-1.0)
            bias1 = consts.tile([128, 1], F32)
            nc.vector.scalar_tensor_tensor(
                out=bias1[:], in0=scale1[:], scalar=negm1[:], in1=be1_sb[:],
                op0=ALU.mult, op1=ALU.add)

            # ---------------- phase 2: h1, z2, stats2 ------------------
            # softplus per 4-tile slab; W2 as two half-partition matmuls
            # per PSUM bank so downstream tiles are full 128 partitions
            # (partitions 0:64 <- even tile features, 64:128 <- odd).
            for g in range(NSLAB):
                zsl = z1_sb[:, 4 * g:4 * g + 4, :].rearrange("p a b -> p (a b)")
                nc.scalar.activation(out=zsl, in_=zsl, func=AF.Exp,
                                     bias=bias1[:], scale=scale1[:])
                h1s = h1_pl.tile([128, 2048], BF16, tag="h1")
                nc.scalar.activation(out=h1s[:], in_=zsl, func=AF.Ln,
                                     bias=1.0)
                for m in range(2):
                    t = 2 * g + m
                    c = (4 * g + 2 * m) // M
                    valid = CW if c < NCHUNK - 1 else TAIL
                    ps = psB.tile([128, 512], F32, tag="psB")
                    nc.tensor.matmul(ps[0:64, :], w2t_sb[:],
                                     h1s[:, (2 * m) * 512:(2 * m + 1) * 512],
                                     start=True, stop=True,
                                     skip_group_check=True)
                    nc.tensor.matmul(ps[64:128, :], w2t_sb[:],
                                     h1s[:, (2 * m + 1) * 512:(2 * m + 2) * 512],
                                     start=True, stop=True,
                                     skip_group_check=True)
                    nc.vector.bn_stats(out=stats2[:, t, :],
                                       in_=ps[:, 0:valid])
                    nc.vector.tensor_copy(out=z2_sb[:, t, :], in_=ps[:])

            # ---------------- BN2 stats allreduce ----------------------
            mv2 = consts.tile([128, 2], F32)
            nc.vector.bn_aggr(out=mv2[:], in_=stats2[:])
            pay2 = consts.tile([128, 2], F32)
            msq2 = consts.tile([128, 1], F32)
            nc.scalar.square(out=msq2[:], in_=mv2[:, 0:1])
            nc.vector.tensor_copy(out=pay2[:, 0:1], in_=mv2[:, 0:1])
            nc.vector.tensor_add(out=pay2[:, 1:2], in0=mv2[:, 1:2], in1=msq2[:])
            cc2i = dram.tile([128, 2], F32)
            cc2o = dram.tile([128, 2], F32)
            nc.sync.dma_start(out=cc2i[:], in_=pay2[:])
            nc.gpsimd.collective_compute(
                "AllReduce", ALU.add, replica_groups=[list(range(NCORES))],
                ins=[cc2i[:].opt()], outs=[cc2o[:].opt()])
            # prefetch phase-3 bond weights during the collective
            bwts = []
            for g in range(NSLAB):
                bwt = bwbc_pl.tile([128, 2, 512], BF16, tag="bwbc",
                                   name=f"bwt_{g}")
                for m in range(2):
                    t = 2 * g + m
                    c, j0 = divmod(2 * t, M)
                    nc.sync.dma_start(
                        out=bwt[0:64, m, :],
                        in_=bass.AP(tensor=bw.tensor,
                                    offset=j0 * NPAD + c * CW,
                                    ap=[[0, 64], [1, CW]]))
                    nc.sync.dma_start(
                        out=bwt[64:128, m, :],
                        in_=bass.AP(tensor=bw.tensor,
                                    offset=(j0 + 1) * NPAD + c * CW,
                                    ap=[[0, 64], [1, CW]]))
                bwts.append(bwt)
            S2 = consts.tile([128, 2], F32)
            nc.sync.dma_start(out=S2[:], in_=cc2o[:])
            # fold: partitions 64:128 hold the odd-tile half of each
            # feature's stats; shift down and add for the global sums
            S2s = consts.tile([64, 2], F32)
            nc.sync.dma_start(out=S2s[:], in_=S2[64:128, :])
            S2t = consts.tile([64, 2], F32)
            nc.vector.tensor_add(out=S2t[:], in0=S2[0:64, :], in1=S2s[:])
            mean2 = consts.tile([64, 1], F32)
            nc.scalar.mul(out=mean2[:], in_=S2t[:, 0:1], mul=1.0 / (2 * NCORES))
            mm2 = consts.tile([64, 1], F32)
            nc.scalar.square(out=mm2[:], in_=mean2[:])
            var2 = consts.tile([64, 1], F32)
            nc.vector.scalar_tensor_tensor(
                out=var2[:], in0=S2t[:, 1:2], scalar=1.0 / (2 * NCORES),
                in1=mm2[:], op0=ALU.mult, op1=ALU.subtract)
            sd2 = consts.tile([64, 1], F32)
            nc.scalar.activation(out=sd2[:], in_=var2[:], func=AF.Sqrt,
                                 bias=eps_sb[0:64, :], scale=1.0)
            rs2 = consts.tile([64, 1], F32)
            nc.vector.reciprocal(out=rs2[:], in_=sd2[:])
            scale2 = consts.tile([64, 1], F32)
            nc.vector.tensor_mul(out=scale2[:], in0=rs2[:], in1=g2_sb[:])
            negm2 = consts.tile([64, 1], F32)
            nc.scalar.mul(out=negm2[:], in_=mean2[:], mul=-1.0)
            bias2 = consts.tile([64, 1], F32)
            nc.vector.scalar_tensor_tensor(
                out=bias2[:], in0=scale2[:], scalar=negm2[:], in1=be2_sb[:],
                op0=ALU.mult, op1=ALU.add)
            scale2r = consts.tile([128, 1], F32)
            nc.sync.dma_start(out=scale2r[0:64, :], in_=scale2[:])
            nc.sync.dma_start(out=scale2r[64:128, :], in_=scale2[:])
            bias2r = consts.tile([128, 1], F32)
            nc.sync.dma_start(out=bias2r[0:64, :], in_=bias2[:])
            nc.sync.dma_start(out=bias2r[64:128, :], in_=bias2[:])

            # ---------------- phase 3: softplus2 * bw -> out -----------
            for g in range(NSLAB):
                zf = z2_sb[:, 2 * g:2 * g + 2, :].rearrange("p a b -> p (a b)")
                nc.scalar.activation(out=zf, in_=zf, func=AF.Exp,
                                     bias=bias2r[:], scale=scale2r[:])
                sp = sp_pl.tile([128, 1024], F32, tag="sp")
                nc.scalar.activation(out=sp[:], in_=zf, func=AF.Ln, bias=1.0)
                nc.vector.tensor_mul(
                    out=sp[:], in0=sp[:],
                    in1=bwts[g][:].rearrange("p a b -> p (a b)"))
                nc.sync.dma_start(
                    out=out_p[:, g * 1024:(g + 1) * 1024], in_=sp[:])

    nc.compile()
    _CACHE["nc"] = nc
    return nc


def _prep_core(c, atom_fea, nbr_fea, nbr_fea_idx, angle_fea, bond_weights,
               shared):
    lo = c * NLOC
    hi = lo + NLOC
    atom_locT = np.zeros((64, NPAD), BF16_NP)
    atom_locT[:, :NLOC] = atom_fea[lo:hi].T.astype(BF16_NP)
    angle_t = np.zeros((KA * 128, NPAD), BF16_NP)
    angle_t[:A * ANG_F, :NLOC] = \
        angle_fea[lo:hi].reshape(NLOC, A * ANG_F).T.astype(BF16_NP)
    nbr_t = np.zeros(((M + 1) * 64, NPAD), BF16_NP)
    nbr_t[:M * 64, :NLOC] = \
        nbr_fea[lo:hi].transpose(1, 2, 0).reshape(M * 64, NLOC).astype(BF16_NP)
    nbr_t[M * 64:, :] = nbr_t[0:64, :]
    bw_p = np.zeros((M, NPAD), BF16_NP)
    bw_p[:, :NLOC] = bond_weights[lo:hi].T.astype(BF16_NP)

    idxp = np.zeros((NPAD, M), np.int16)
    idxp[:NLOC] = nbr_fea_idx[lo:hi].astype(np.int16)
    idx_w = np.zeros((128, NCHUNK * (NIDX // 16)), np.int16)
    for cc in range(NCHUNK):
        flat = idxp[cc * CW:(cc + 1) * CW, :].T.reshape(-1)   # slot-major
        wr = flat.reshape(NIDX // 16, 16).T                   # (16, 384)
        col = cc * (NIDX // 16)
        idx_w[:, col:col + NIDX // 16] = np.tile(wr, (8, 1))

    d = dict(shared)
    d.update(atom_locT=atom_locT, angle_t=angle_t, nbr_t=nbr_t, idx_w=idx_w,
             bw=bw_p)
    return d


def _make_in_maps(inputs):
    """Build per-core input dicts from the full (unsharded) input dict."""
    atom_fea = np.asarray(inputs["atom_fea"], dtype=np.float32)
    nbr_fea = np.asarray(inputs["nbr_fea"], dtype=np.float32)
    nbr_fea_idx = np.asarray(inputs["nbr_fea_idx"])
    angle_fea = np.asarray(inputs["angle_fea"], dtype=np.float32)
    bond_weights = np.asarray(inputs["bond_weights"], dtype=np.float32)
    W1 = np.asarray(inputs["W1"]); W2 = np.asarray(inputs["W2"])
    g1 = np.asarray(inputs["g1"]); be1 = np.asarray(inputs["be1"])
    g2 = np.asarray(inputs["g2"]); be2 = np.asarray(inputs["be2"])

    atom_fullT = np.zeros((64, NFULL), BF16_NP)
    atom_fullT[:, :N] = atom_fea.T.astype(BF16_NP)
    w1t = W1.T.astype(np.float32)
    w1t_a = np.zeros((KA * 128, 128), BF16_NP)
    w1t_a[:A * ANG_F] = w1t[256:1312].astype(BF16_NP)
    shared = dict(
        atom_fullT=atom_fullT,
        w1t_c=np.ascontiguousarray(w1t[0:64]).astype(BF16_NP),
        w1t_n=np.ascontiguousarray(w1t[64:128]).astype(BF16_NP),
        w1t_fr=np.ascontiguousarray(w1t[128:256]).astype(BF16_NP),
        w1t_a=w1t_a,
        w2t=np.ascontiguousarray(W2.T).astype(BF16_NP),
        ident=np.eye(128, dtype=np.float32).astype(BF16_NP),
        g1=g1.reshape(128, 1).astype(np.float32),
        be1=be1.reshape(128, 1).astype(np.float32),
        g2=g2.reshape(64, 1).astype(np.float32),
        be2=be2.reshape(64, 1).astype(np.float32),
    )
    return [_prep_core(c, atom_fea, nbr_fea, nbr_fea_idx, angle_fea,
                       bond_weights, shared)
            for c in range(NCORES)]


def _assemble(results):
    """Per-core out_p buffers -> full (N, M, NBR_F) output."""
    out = np.empty((N, M, NBR_F), np.float32)
    for c in range(NCORES):
        op = results[c]["out_p"]
        lo = c * NLOC
        for t in range(NPAIR):
            cc, j0 = divmod(2 * t, M)
            blk = op[:, t * 512:(t + 1) * 512]
            a0 = cc * CW
            nA = min(CW, NLOC - a0)
            out[lo + a0:lo + a0 + nA, j0, :] = blk[0:64, :nA].T
            out[lo + a0:lo + a0 + nA, j0 + 1, :] = blk[64:128, :nA].T
    return out


def kernel(atom_fea, nbr_fea, nbr_fea_idx, angle_fea, bond_weights,
           W1, b1, g1, be1, W2, b2, g2, be2):
    global LAST_EXEC_NS, LAST_RESULTS
    nc = _build()
    in_maps = _make_in_maps(dict(
        atom_fea=atom_fea, nbr_fea=nbr_fea, nbr_fea_idx=nbr_fea_idx,
        angle_fea=angle_fea, bond_weights=bond_weights, W1=W1, W2=W2,
        g1=g1, be1=be1, g2=g2, be2=be2))

    if TRACE:
        _install_ntff_hook()
    br = run_bass_kernel_spmd(nc, in_maps, list(range(NCORES)), trace=TRACE)
    LAST_EXEC_NS = br.exec_time_ns
    LAST_RESULTS = br
    return _assemble(br.results)


def _install_ntff_hook():
    """Inject antenv.axon_hooks (missing in this image) so trace=True works."""
    import types
    if "antenv.axon_hooks" in sys.modules:
        return
    sys.path.insert(0, "/root/.axon_site")
    mod = types.ModuleType("antenv.axon_hooks")
    mod._hook = None
    mod.set_axon_ntff_profile_hook = lambda h: setattr(mod, "_hook", h)
    mod.get_axon_ntff_profile_hook = lambda: mod._hook
    sys.modules["antenv.axon_hooks"] = mod
    try:
        from trn_agent_boot.trn_boot import _ntff_profile_via_ctypes
        h = _ntff_profile_via_ctypes("/opt/axon/libaxon_pjrt.so")
        if h is not None:
            mod.set_axon_ntff_profile_hook(h)
    except Exception as e:
        print("ntff hook install failed:", e)


# revision 8
# speedup vs baseline: 1.6337x; 1.2778x over previous
"""Trainium2 Bass kernel for nn_BondConvLayer (gnn_message_passing).

8-core data-parallel: 2500 atoms (30000 bonds) per core.

out = softplus(bn2(softplus(bn1(cat @ W1.T)) @ W2.T)) * bw  where
cat = [center, gathered_nbr_atom, nbr_fea, rolled_nbr_fea, angle] per bond;
b1/b2 cancel inside training-mode BatchNorm and are dropped.

v2 layout: everything bf16 on the wire and in the PE. Each core projects
the full atom table atom-major ([atom, 128h] rows, one matmul per
128-atom rank) and stores it to DRAM; per-bond rows are then pulled with
a DRAM-source non-transpose dma_gather (contiguous 256B descriptors -
fast path) arriving bond-major, and re-transposed into the feature-major
z1 PSUM accumulation with identity matmuls on the PE (stat=G block,
mov=I, start=False). nbr f/r projections run as one 128-deep stacked
matmul per slot ([W1f;W1r] weights, one 128-partition DMA spanning
adjacent nbr slots via a wraparound row). The per-atom center+angle base
is added on DVE during PSUM evacuation (scalar_tensor_tensor), which
also casts z1 to bf16 slabs kept in SBUF (no DRAM spill). BN batch
stats: bn_stats/bn_aggr per core + tiny AllReduce; phase 2 runs W2 as
two half-partition matmuls per PSUM bank (tile_position) so softplus /
stats / output work on full 128-partition tiles. Softplus = Exp then
Ln(x+1) on ACT with the BN affine fused in.
"""
import sys, os

sys.path.insert(0, "/opt/trn_rl_repo")

import numpy as np

import concourse.bass as bass
import concourse.bacc as bacc
import concourse.tile as tile
from concourse import mybir
from concourse.bass_utils import run_bass_kernel_spmd

F32 = mybir.dt.float32
BF16 = mybir.dt.bfloat16
I16 = mybir.dt.int16
AF = mybir.ActivationFunctionType
ALU = mybir.AluOpType
BF16_NP = mybir.dt.np(BF16)

NCORES = 8
N, M = 20000, 12
ATOM_F, NBR_F, ANG_F, A = 64, 64, 16, 66
H, O = 128, 64
BN_EPS = 1e-5
NLOC = N // NCORES          # 2500
NCHUNK = 5                  # chunks of 512 atoms (last ragged 452)
CW = 512
NPAD = NCHUNK * CW          # 2560
TAIL = NLOC - (NCHUNK - 1) * CW   # 452
NTILE = NCHUNK * M          # 60 bond tiles per core
NPAIR = NTILE // 2          # 30 paired tiles in phase 2/3
NSLAB = NTILE // 4          # 15 slabs of 4 bond-tiles
KA = 9                      # angle K-chunks of 128 (1056 padded to 1152)
NRANK = 160                 # table ranks (20000 atoms padded to 20480)
NFULL = NRANK * 128         # 20480
NIDX = M * CW               # 6144 gather indices per chunk (one call)
TGRP = 20                   # table build groups of 8 ranks

_CACHE = {}
TRACE = bool(int(os.environ.get("BASS_KERNEL_TRACE", "0")))
LAST_EXEC_NS = None
LAST_RESULTS = None


def _pin_act_tables():
    """Restrict the activation-table sets bacc may choose so Exp/Ln/Copy/
    Square all land in natural_log_exp_and_others (one load, no per-op
    table swaps). Set names/order (= act_func_set_id) are preserved."""
    if getattr(bacc, "_act_tables_pinned", False):
        return
    orig = bacc.get_activation_tables

    def pinned(arch):
        tabs = orig(arch)
        keep_all = "natural_log_exp_and_others"
        sqrt_home = "sqrt_and_others"
        strip = {AF.Exp, AF.Ln, AF.Copy, AF.Identity, AF.Square, AF.Sqrt}
        out = {}
        for name, funcs in tabs.items():
            if name == keep_all:
                out[name] = funcs
            elif name == sqrt_home:
                out[name] = {f for f in funcs
                             if f not in (strip - {AF.Sqrt})}
            else:
                out[name] = {f for f in funcs if f not in strip}
        return out

    bacc.get_activation_tables = pinned
    bacc._act_tables_pinned = True


def _build():
    if "nc" in _CACHE:
        return _CACHE["nc"]
    _pin_act_tables()
    import concourse.tile_utils as tile_utils
    tile_utils.max_sbuf_usage = 206 * 1024

    nc = bacc.Bacc("TRN2", target_bir_lowering=False, debug=False,
                   num_devices=NCORES, num_swdge_queues=4)

    atom_fullT = nc.dram_tensor("atom_fullT", [64, NFULL], BF16, kind="ExternalInput").ap()
    atom_locT = nc.dram_tensor("atom_locT", [64, NPAD], BF16, kind="ExternalInput").ap()
    angle_t = nc.dram_tensor("angle_t", [KA * 128, NPAD], BF16, kind="ExternalInput").ap()
    nbr_t = nc.dram_tensor("nbr_t", [(M + 1) * 64, NPAD], BF16, kind="ExternalInput").ap()
    idx_w = nc.dram_tensor("idx_w", [128, NCHUNK * (NIDX // 16)], I16, kind="ExternalInput").ap()
    bw = nc.dram_tensor("bw", [M, NPAD], BF16, kind="ExternalInput").ap()
    w1t_c = nc.dram_tensor("w1t_c", [64, 128], BF16, kind="ExternalInput").ap()
    w1t_n = nc.dram_tensor("w1t_n", [64, 128], BF16, kind="ExternalInput").ap()
    w1t_fr = nc.dram_tensor("w1t_fr", [128, 128], BF16, kind="ExternalInput").ap()
    w1t_a = nc.dram_tensor("w1t_a", [KA * 128, 128], BF16, kind="ExternalInput").ap()
    w2t = nc.dram_tensor("w2t", [128, 64], BF16, kind="ExternalInput").ap()
    ident = nc.dram_tensor("ident", [128, 128], BF16, kind="ExternalInput").ap()
    g1 = nc.dram_tensor("g1", [128, 1], F32, kind="ExternalInput").ap()
    be1 = nc.dram_tensor("be1", [128, 1], F32, kind="ExternalInput").ap()
    g2 = nc.dram_tensor("g2", [64, 1], F32, kind="ExternalInput").ap()
    be2 = nc.dram_tensor("be2", [64, 1], F32, kind="ExternalInput").ap()
    out_p = nc.dram_tensor("out_p", [128, NPAIR * 512], F32, kind="ExternalOutput").ap()

    with tile.TileContext(nc) as tc:
        with (tc.tile_pool(name="consts", bufs=1) as consts,
              tc.tile_pool(name="astr", bufs=3) as astr_pl,
              tc.tile_pool(name="tb", bufs=2) as tb_pl,
              tc.tile_pool(name="atom", bufs=2) as atom_pl,
              tc.tile_pool(name="angle", bufs=2) as angle_pl,
              tc.tile_pool(name="nbr", bufs=4) as nbr_pl,
              tc.tile_pool(name="gath", bufs=2) as gath_pl,
              tc.tile_pool(name="absb", bufs=2) as absb_pl,
              tc.tile_pool(name="h1", bufs=2) as h1_pl,
              tc.tile_pool(name="sp", bufs=2) as sp_pl,
              tc.tile_pool(name="bwbc", bufs=NSLAB) as bwbc_pl,
              tc.tile_pool(name="psA", bufs=4, space="PSUM") as psA,
              tc.tile_pool(name="psB", bufs=2, space="PSUM") as psB,
              tc.tile_pool(name="dram", bufs=1, space="DRAM") as dram):

            # ---------------- constants -------------------------------
            ident_sb = consts.tile([128, 128], BF16)
            nc.sync.dma_start(out=ident_sb, in_=ident)
            w1c_sb = consts.tile([64, 128], BF16)
            nc.sync.dma_start(out=w1c_sb, in_=w1t_c)
            w1n_sb = consts.tile([64, 128], BF16)
            nc.sync.dma_start(out=w1n_sb, in_=w1t_n)
            w1fr_sb = consts.tile([128, 128], BF16)
            nc.sync.dma_start(out=w1fr_sb, in_=w1t_fr)
            w1a_sb = consts.tile([128, KA, 128], BF16)
            nc.sync.dma_start(
                out=w1a_sb,
                in_=bass.AP(tensor=w1t_a.tensor, offset=0,
                            ap=[[128, 128], [128 * 128, KA], [1, 128]]))
            w2t_sb = consts.tile([128, 64], BF16)
            nc.sync.dma_start(out=w2t_sb, in_=w2t)
            idx_sb = consts.tile([128, NCHUNK * (NIDX // 16)], I16)
            nc.sync.dma_start(out=idx_sb, in_=idx_w)
            g1_sb = consts.tile([128, 1], F32)
            nc.sync.dma_start(out=g1_sb, in_=g1)
            be1_sb = consts.tile([128, 1], F32)
            nc.sync.dma_start(out=be1_sb, in_=be1)
            g2_sb = consts.tile([64, 1], F32)
            nc.sync.dma_start(out=g2_sb, in_=g2)
            be2_sb = consts.tile([64, 1], F32)
            nc.sync.dma_start(out=be2_sb, in_=be2)
            eps_sb = consts.tile([128, 1], F32)
            nc.vector.memset(eps_sb, BN_EPS)

            stats1 = consts.tile([128, NTILE, 6], F32)
            stats2 = consts.tile([128, NPAIR, 6], F32)
            z1_sb = consts.tile([128, NTILE, 512], BF16)
            z2_sb = consts.tile([128, NPAIR, 512], BF16)

            table_d = dram.tile([NFULL, 128], BF16)
            table_ap = bass.AP(tensor=table_d.tensor, offset=0,
                               ap=[[128, NFULL], [1, 128]])

            # ------- phase 0: bf16 projection table in DRAM -----------
            # table row a = atom_fea[a] @ W1n.T, built atom-major: one
            # matmul per 128-atom rank (atoms land on partitions), so the
            # store to DRAM is a plain contiguous-row DMA.
            for grp in range(TGRP):
                a_sb = astr_pl.tile([64, 1024], BF16, tag="astr")
                nc.sync.dma_start(out=a_sb,
                                  in_=atom_fullT[:, grp * 1024:(grp + 1) * 1024])
                tb = tb_pl.tile([128, 8, 128], BF16, tag="tb")
                for half in range(2):
                    ps = psA.tile([128, 512], F32, tag="slot")
                    for k in range(4):
                        nc.tensor.matmul(
                            ps[:, k * 128:(k + 1) * 128],
                            a_sb[:, (half * 4 + k) * 128:(half * 4 + k + 1) * 128],
                            w1n_sb[:], start=True, stop=True,
                            skip_group_check=True)
                    nc.vector.tensor_copy(
                        out=tb[:, half * 4:half * 4 + 4, :].rearrange(
                            "p a b -> p (a b)"),
                        in_=ps[:])
                nc.sync.dma_start(
                    out=bass.AP(tensor=table_d.tensor,
                                offset=grp * 1024 * 128,
                                ap=[[128, 128], [128 * 128, 8], [1, 128]]),
                    in_=tb[:])

            # ---------------- phase 1: z1 assembly + stats -------------
            for c in range(NCHUNK):
                valid = CW if c < NCHUNK - 1 else TAIL
                # per-atom base: center + angle -> psB bank -> SBUF bf16
                at_sb = atom_pl.tile([64, CW], BF16, tag="atom")
                nc.sync.dma_start(out=at_sb,
                                  in_=atom_locT[:, c * CW:(c + 1) * CW])
                ab = psB.tile([128, 512], F32, tag="psB")
                nc.tensor.matmul(ab[:], w1c_sb[:], at_sb[:],
                                 start=True, stop=False)
                an_sb = angle_pl.tile([128, KA, CW], BF16, tag="angle")
                nc.sync.dma_start(
                    out=an_sb,
                    in_=bass.AP(tensor=angle_t.tensor, offset=c * CW,
                                ap=[[NPAD, 128], [128 * NPAD, KA], [1, CW]]))
                for k in range(KA):
                    nc.tensor.matmul(ab[:], w1a_sb[:, k, :], an_sb[:, k, :],
                                     start=False, stop=(k == KA - 1))
                ab_sb = absb_pl.tile([128, 512], BF16, tag="absb")
                nc.scalar.copy(out=ab_sb[:], in_=ab[:])
                # bond-major gathers, split across the 4 SWDGE queues so
                # 4 DMA rings pull table rows concurrently (3 slots each)
                gts = []
                for q in range(4):
                    gt = gath_pl.tile([128, NIDX // 512, 128], BF16,
                                      tag=f"gath{q}")
                    col = c * (NIDX // 16) + q * (NIDX // 64)
                    nc.gpsimd.dma_gather(
                        out_ap=gt[:], in_ap=table_ap,
                        idxs_ap=idx_sb[:, col:col + NIDX // 64],
                        num_idxs=NIDX // 4, num_idxs_reg=NIDX // 4,
                        elem_size=128, transpose=False, single_packet=False,
                        queue_num=q)
                    gts.append(gt)
                for j in range(M):
                    # stacked [nbr_j; nbr_{j+1}] via one 128-partition DMA
                    # (row M is a host-side copy of row 0 for wraparound)
                    nt = nbr_pl.tile([128, CW], BF16, tag="nbr")
                    nc.sync.dma_start(
                        out=nt,
                        in_=bass.AP(tensor=nbr_t.tensor,
                                    offset=j * 64 * NPAD + c * CW,
                                    ap=[[NPAD, 128], [1, CW]]))
                    ps = psA.tile([128, 512], F32, tag="slot")
                    nc.tensor.matmul(ps[:], w1fr_sb[:], nt[:],
                                     start=True, stop=False)
                    # transpose-inject gathered nbr_atom rows: G_block.T
                    for k in range(4):
                        nc.tensor.matmul(ps[:, k * 128:(k + 1) * 128],
                                         gts[j // 3][:, 4 * (j % 3) + k, :],
                                         ident_sb[:],
                                         start=False, stop=(k == 3))
                    t = c * M + j
                    z1t = z1_sb[:, t, :]
                    nc.vector.scalar_tensor_tensor(
                        out=z1t, in0=ps[:], scalar=1.0, in1=ab_sb[:],
                        op0=ALU.mult, op1=ALU.add)
                    nc.vector.bn_stats(out=stats1[:, t, :],
                                       in_=z1t[:, 0:valid])

            # ---------------- BN1 stats allreduce ----------------------
            mv1 = consts.tile([128, 2], F32)
            nc.vector.bn_aggr(out=mv1[:], in_=stats1[:])
            pay1 = consts.tile([128, 2], F32)
            msq1 = consts.tile([128, 1], F32)
            nc.scalar.square(out=msq1[:], in_=mv1[:, 0:1])
            nc.vector.tensor_copy(out=pay1[:, 0:1], in_=mv1[:, 0:1])
            nc.vector.tensor_add(out=pay1[:, 1:2], in0=mv1[:, 1:2], in1=msq1[:])
            cc1i = dram.tile([128, 2], F32)
            cc1o = dram.tile([128, 2], F32)
            nc.sync.dma_start(out=cc1i[:], in_=pay1[:])
            nc.gpsimd.collective_compute(
                "AllReduce", ALU.add, replica_groups=[list(range(NCORES))],
                ins=[cc1i[:].opt()], outs=[cc1o[:].opt()])
            S1 = consts.tile([128, 2], F32)
            nc.sync.dma_start(out=S1[:], in_=cc1o[:])
            mean1 = consts.tile([128, 1], F32)
            nc.scalar.mul(out=mean1[:], in_=S1[:, 0:1], mul=1.0 / NCORES)
            mm1 = consts.tile([128, 1], F32)
            nc.scalar.square(out=mm1[:], in_=mean1[:])
            var1 = consts.tile([128, 1], F32)
            nc.vector.scalar_tensor_tensor(
                out=var1[:], in0=S1[:, 1:2], scalar=1.0 / NCORES, in1=mm1[:],
                op0=ALU.mult, op1=ALU.subtract)
            sd1 = consts.tile([128, 1], F32)
            nc.scalar.activation(out=sd1[:], in_=var1[:], func=AF.Sqrt,
                                 bias=eps_sb[:], scale=1.0)
            rs1 = consts.tile([128, 1], F32)
            nc.vector.reciprocal(out=rs1[:], in_=sd1[:])
            scale1 = consts.tile([128, 1], F32)
            nc.vector.tensor_mul(out=scale1[:], in0=rs1[:], in1=g1_sb[:])
            negm1 = consts.tile([128, 1], F32)
            nc.scalar.mul(out=negm1[:], in_=mean1[:], mul=-1.0)
            bias1 = consts.tile([128, 1], F32)
            nc.vector.scalar_tensor_tensor(
                out=bias1[:], in0=scale1[:], scalar=negm1[:], in1=be1_sb[:],
                op0=ALU.mult, op1=ALU.add)

            # ---------------- phase 2: h1, z2, stats2 ------------------
            # softplus per 4-tile slab; W2 as two half-partition matmuls
            # per PSUM bank so downstream tiles are full 128 partitions
            # (partitions 0:64 <- even tile features, 64:128 <- odd).
            for g in range(NSLAB):
                zsl = z1_sb[:, 4 * g:4 * g + 4, :].rearrange("p a b -> p (a b)")
                nc.scalar.activation(out=zsl, in_=zsl, func=AF.Exp,
                                     bias=bias1[:], scale=scale1[:])
                h1s = h1_pl.tile([128, 2048], BF16, tag="h1")
                nc.scalar.activation(out=h1s[:], in_=zsl, func=AF.Ln,
                                     bias=1.0)
                for m in range(2):
                    t = 2 * g + m
                    c = (4 * g + 2 * m) // M
                    valid = CW if c < NCHUNK - 1 else TAIL
                    ps = psB.tile([128, 512], F32, tag="psB")
                    nc.tensor.matmul(ps[0:64, :], w2t_sb[:],
                                     h1s[:, (2 * m) * 512:(2 * m + 1) * 512],
                                     start=True, stop=True,
                                     skip_group_check=True)
                    nc.tensor.matmul(ps[64:128, :], w2t_sb[:],
                                     h1s[:, (2 * m + 1) * 512:(2 * m + 2) * 512],
                                     start=True, stop=True,
                                     skip_group_check=True)
                    nc.vector.bn_stats(out=stats2[:, t, :],
                                       in_=ps[:, 0:valid])
                    nc.vector.tensor_copy(out=z2_sb[:, t, :], in_=ps[:])

            # ---------------- BN2 stats allreduce ----------------------
            mv2 = consts.tile([128, 2], F32)
            nc.vector.bn_aggr(out=mv2[:], in_=stats2[:])
            pay2 = consts.tile([128, 2], F32)
            msq2 = consts.tile([128, 1], F32)
            nc.scalar.square(out=msq2[:], in_=mv2[:, 0:1])
            nc.vector.tensor_copy(out=pay2[:, 0:1], in_=mv2[:, 0:1])
            nc.vector.tensor_add(out=pay2[:, 1:2], in0=mv2[:, 1:2], in1=msq2[:])
            cc2i = dram.tile([128, 2], F32)
            cc2o = dram.tile([128, 2], F32)
            nc.sync.dma_start(out=cc2i[:], in_=pay2[:])
            nc.gpsimd.collective_compute(
                "AllReduce", ALU.add, replica_groups=[list(range(NCORES))],
                ins=[cc2i[:].opt()], outs=[cc2o[:].opt()])
            # prefetch phase-3 bond weights during the collective
            bwts = []
            for g in range(NSLAB):
                bwt = bwbc_pl.tile([128, 2, 512], BF16, tag="bwbc",
                                   name=f"bwt_{g}")
                for m in range(2):
                    t = 2 * g + m
                    c, j0 = divmod(2 * t, M)
                    nc.sync.dma_start(
                        out=bwt[0:64, m, :],
                        in_=bass.AP(tensor=bw.tensor,
                                    offset=j0 * NPAD + c * CW,
                                    ap=[[0, 64], [1, CW]]))
                    nc.sync.dma_start(
                        out=bwt[64:128, m, :],
                        in_=bass.AP(tensor=bw.tensor,
                                    offset=(j0 + 1) * NPAD + c * CW,
                                    ap=[[0, 64], [1, CW]]))
                bwts.append(bwt)
            S2 = consts.tile([128, 2], F32)
            nc.sync.dma_start(out=S2[:], in_=cc2o[:])
            # fold: partitions 64:128 hold the odd-tile half of each
            # feature's stats; shift down and add for the global sums
            S2s = consts.tile([64, 2], F32)
            nc.sync.dma_start(out=S2s[:], in_=S2[64:128, :])
            S2t = consts.tile([64, 2], F32)
            nc.vector.tensor_add(out=S2t[:], in0=S2[0:64, :], in1=S2s[:])
            mean2 = consts.tile([64, 1], F32)
            nc.scalar.mul(out=mean2[:], in_=S2t[:, 0:1], mul=1.0 / (2 * NCORES))
            mm2 = consts.tile([64, 1], F32)
            nc.scalar.square(out=mm2[:], in_=mean2[:])
            var2 = consts.tile([64, 1], F32)
            nc.vector.scalar_tensor_tensor(
                out=var2[:], in0=S2t[:, 1:2], scalar=1.0 / (2 * NCORES),
                in1=mm2[:], op0=ALU.mult, op1=ALU.subtract)
            sd2 = consts.tile([64, 1], F32)
            nc.scalar.activation(out=sd2[:], in_=var2[:], func=AF.Sqrt,
                                 bias=eps_sb[0:64, :], scale=1.0)
            rs2 = consts.tile([64, 1], F32)
            nc.vector.reciprocal(out=rs2[:], in_=sd2[:])
            scale2 = consts.tile([64, 1], F32)
            nc.vector.tensor_mul(out=scale2[:], in0=rs2[:], in1=g2_sb[:])
            negm2 = consts.tile([64, 1], F32)
            nc.scalar.mul(out=negm2[:], in_=mean2[:], mul=-1.0)
            bias2 = consts.tile([64, 1], F32)
            nc.vector.scalar_tensor_tensor(
                out=bias2[:], in0=scale2[:], scalar=negm2[:], in1=be2_sb[:],
                op0=ALU.mult, op1=ALU.add)
            scale2r = consts.tile([128, 1], F32)
            nc.sync.dma_start(out=scale2r[0:64, :], in_=scale2[:])
            nc.sync.dma_start(out=scale2r[64:128, :], in_=scale2[:])
            bias2r = consts.tile([128, 1], F32)
            nc.sync.dma_start(out=bias2r[0:64, :], in_=bias2[:])
            nc.sync.dma_start(out=bias2r[64:128, :], in_=bias2[:])

            # ---------------- phase 3: softplus2 * bw -> out -----------
            for g in range(NSLAB):
                zf = z2_sb[:, 2 * g:2 * g + 2, :].rearrange("p a b -> p (a b)")
                nc.scalar.activation(out=zf, in_=zf, func=AF.Exp,
                                     bias=bias2r[:], scale=scale2r[:])
                sp = sp_pl.tile([128, 1024], F32, tag="sp")
                nc.scalar.activation(out=sp[:], in_=zf, func=AF.Ln, bias=1.0)
                nc.vector.tensor_mul(
                    out=sp[:], in0=sp[:],
                    in1=bwts[g][:].rearrange("p a b -> p (a b)"))
                nc.sync.dma_start(
                    out=out_p[:, g * 1024:(g + 1) * 1024], in_=sp[:])

    nc.compile()
    _CACHE["nc"] = nc
    return nc


def _prep_core(c, atom_fea, nbr_fea, nbr_fea_idx, angle_fea, bond_weights,
               shared):
    lo = c * NLOC
    hi = lo + NLOC
    atom_locT = np.zeros((64, NPAD), BF16_NP)
    atom_locT[:, :NLOC] = atom_fea[lo:hi].T.astype(BF16_NP)
    angle_t = np.zeros((KA * 128, NPAD), BF16_NP)
    angle_t[:A * ANG_F, :NLOC] = \
        angle_fea[lo:hi].reshape(NLOC, A * ANG_F).T.astype(BF16_NP)
    nbr_t = np.zeros(((M + 1) * 64, NPAD), BF16_NP)
    nbr_t[:M * 64, :NLOC] = \
        nbr_fea[lo:hi].transpose(1, 2, 0).reshape(M * 64, NLOC).astype(BF16_NP)
    nbr_t[M * 64:, :] = nbr_t[0:64, :]
    bw_p = np.zeros((M, NPAD), BF16_NP)
    bw_p[:, :NLOC] = bond_weights[lo:hi].T.astype(BF16_NP)

    idxp = np.zeros((NPAD, M), np.int16)
    idxp[:NLOC] = nbr_fea_idx[lo:hi].astype(np.int16)
    idx_w = np.zeros((128, NCHUNK * (NIDX // 16)), np.int16)
    for cc in range(NCHUNK):
        flat = idxp[cc * CW:(cc + 1) * CW, :].T.reshape(-1)   # slot-major
        wr = flat.reshape(NIDX // 16, 16).T                   # (16, 384)
        col = cc * (NIDX // 16)
        idx_w[:, col:col + NIDX // 16] = np.tile(wr, (8, 1))

    d = dict(shared)
    d.update(atom_locT=atom_locT, angle_t=angle_t, nbr_t=nbr_t, idx_w=idx_w,
             bw=bw_p)
    return d


def _make_in_maps(inputs):
    """Build per-core input dicts from the full (unsharded) input dict."""
    atom_fea = np.asarray(inputs["atom_fea"], dtype=np.float32)
    nbr_fea = np.asarray(inputs["nbr_fea"], dtype=np.float32)
    nbr_fea_idx = np.asarray(inputs["nbr_fea_idx"])
    angle_fea = np.asarray(inputs["angle_fea"], dtype=np.float32)
    bond_weights = np.asarray(inputs["bond_weights"], dtype=np.float32)
    W1 = np.asarray(inputs["W1"]); W2 = np.asarray(inputs["W2"])
    g1 = np.asarray(inputs["g1"]); be1 = np.asarray(inputs["be1"])
    g2 = np.asarray(inputs["g2"]); be2 = np.asarray(inputs["be2"])

    atom_fullT = np.zeros((64, NFULL), BF16_NP)
    atom_fullT[:, :N] = atom_fea.T.astype(BF16_NP)
    w1t = W1.T.astype(np.float32)
    w1t_a = np.zeros((KA * 128, 128), BF16_NP)
    w1t_a[:A * ANG_F] = w1t[256:1312].astype(BF16_NP)
    shared = dict(
        atom_fullT=atom_fullT,
        w1t_c=np.ascontiguousarray(w1t[0:64]).astype(BF16_NP),
        w1t_n=np.ascontiguousarray(w1t[64:128]).astype(BF16_NP),
        w1t_fr=np.ascontiguousarray(w1t[128:256]).astype(BF16_NP),
        w1t_a=w1t_a,
        w2t=np.ascontiguousarray(W2.T).astype(BF16_NP),
        ident=np.eye(128, dtype=np.float32).astype(BF16_NP),
        g1=g1.reshape(128, 1).astype(np.float32),
        be1=be1.reshape(128, 1).astype(np.float32),
        g2=g2.reshape(64, 1).astype(np.float32),
        be2=be2.reshape(64, 1).astype(np.float32),
    )
    return [_prep_core(c, atom_fea, nbr_fea, nbr_fea_idx, angle_fea,
                       bond_weights, shared)
            for c in range(NCORES)]


def _assemble(results):
    """Per-core out_p buffers -> full (N, M, NBR_F) output."""
    out = np.empty((N, M, NBR_F), np.float32)
    for c in range(NCORES):
        op = results[c]["out_p"]
        lo = c * NLOC
        for t in range(NPAIR):
            cc, j0 = divmod(2 * t, M)
            blk = op[:, t * 512:(t + 1) * 512]
            a0 = cc * CW
            nA = min(CW, NLOC - a0)
            out[lo + a0:lo + a0 + nA, j0, :] = blk[0:64, :nA].T
            out[lo + a0:lo + a0 + nA, j0 + 1, :] = blk[64:128, :nA].T
    return out


def kernel(atom_fea, nbr_fea, nbr_fea_idx, angle_fea, bond_weights,
           W1, b1, g1, be1, W2, b2, g2, be2):
    global LAST_EXEC_NS, LAST_RESULTS
    nc = _build()
    in_maps = _make_in_maps(dict(
        atom_fea=atom_fea, nbr_fea=nbr_fea, nbr_fea_idx=nbr_fea_idx,
        angle_fea=angle_fea, bond_weights=bond_weights, W1=W1, W2=W2,
        g1=g1, be1=be1, g2=g2, be2=be2))

    if TRACE:
        _install_ntff_hook()
    br = run_bass_kernel_spmd(nc, in_maps, list(range(NCORES)), trace=TRACE)
    LAST_EXEC_NS = br.exec_time_ns
    LAST_RESULTS = br
    return _assemble(br.results)


def _install_ntff_hook():
    """Inject antenv.axon_hooks (missing in this image) so trace=True works."""
    import types
    if "antenv.axon_hooks" in sys.modules:
        return
    sys.path.insert(0, "/root/.axon_site")
    mod = types.ModuleType("antenv.axon_hooks")
    mod._hook = None
    mod.set_axon_ntff_profile_hook = lambda h: setattr(mod, "_hook", h)
    mod.get_axon_ntff_profile_hook = lambda: mod._hook
    sys.modules["antenv.axon_hooks"] = mod
    try:
        from trn_agent_boot.trn_boot import _ntff_profile_via_ctypes
        h = _ntff_profile_via_ctypes("/opt/axon/libaxon_pjrt.so")
        if h is not None:
            mod.set_axon_ntff_profile_hook(h)
    except Exception as e:
        print("ntff hook install failed:", e)


# revision 19
# speedup vs baseline: 1.7116x; 1.0477x over previous
"""Trainium2 Bass kernel for nn_BondConvLayer (gnn_message_passing).

8-core data-parallel: 2500 atoms (30000 bonds) per core.

out = softplus(bn2(softplus(bn1(cat @ W1.T)) @ W2.T)) * bw  where
cat = [center, gathered_nbr_atom, nbr_fea, rolled_nbr_fea, angle] per bond;
b1/b2 cancel inside training-mode BatchNorm and are dropped.

v2 layout: everything bf16 on the wire and in the PE. Each core projects
the full atom table atom-major ([atom, 128h] rows, one matmul per
128-atom rank) and stores it to DRAM; per-bond rows are then pulled with
a DRAM-source non-transpose dma_gather (contiguous 256B descriptors -
fast path) arriving bond-major, and re-transposed into the feature-major
z1 PSUM accumulation with identity matmuls on the PE (stat=G block,
mov=I, start=False). nbr f/r projections run as one 128-deep stacked
matmul per slot ([W1f;W1r] weights, one 128-partition DMA spanning
adjacent nbr slots via a wraparound row). The per-atom center+angle base
is added on DVE during PSUM evacuation (scalar_tensor_tensor), which
also casts z1 to bf16 slabs kept in SBUF (no DRAM spill). BN batch
stats: bn_stats/bn_aggr per core + tiny AllReduce; phase 2 runs W2 as
two half-partition matmuls per PSUM bank (tile_position) so softplus /
stats / output work on full 128-partition tiles. Softplus = Exp then
Ln(x+1) on ACT with the BN affine fused in.
"""
import sys, os

sys.path.insert(0, "/opt/trn_rl_repo")

import numpy as np

import concourse.bass as bass
import concourse.bacc as bacc
import concourse.tile as tile
from concourse import mybir
from concourse.bass_utils import run_bass_kernel_spmd

F32 = mybir.dt.float32
BF16 = mybir.dt.bfloat16
I16 = mybir.dt.int16
AF = mybir.ActivationFunctionType
ALU = mybir.AluOpType
BF16_NP = mybir.dt.np(BF16)

NCORES = 8
N, M = 20000, 12
ATOM_F, NBR_F, ANG_F, A = 64, 64, 16, 66
H, O = 128, 64
BN_EPS = 1e-5
NLOC = N // NCORES          # 2500
NCHUNK = 5                  # chunks of 512 atoms (last ragged 452)
CW = 512
NPAD = NCHUNK * CW          # 2560
TAIL = NLOC - (NCHUNK - 1) * CW   # 452
NTILE = NCHUNK * M          # 60 bond tiles per core
NPAIR = NTILE // 2          # 30 paired tiles in phase 2/3
NSLAB = NTILE // 4          # 15 slabs of 4 bond-tiles
KA = 9                      # angle K-chunks of 128 (1056 padded to 1152)
NRANK = 160                 # table ranks (20000 atoms padded to 20480)
NFULL = NRANK * 128         # 20480
NIDX = M * CW               # 6144 gather indices per chunk (one call)
TGRP = 20                   # table build groups of 8 ranks

_CACHE = {}
TRACE = bool(int(os.environ.get("BASS_KERNEL_TRACE", "0")))
LAST_EXEC_NS = None
LAST_RESULTS = None


def _pin_act_tables():
    """Restrict the activation-table sets bacc may choose so Exp/Ln/Copy/
    Square all land in natural_log_exp_and_others (one load, no per-op
    table swaps). Set names/order (= act_func_set_id) are preserved."""
    if getattr(bacc, "_act_tables_pinned", False):
        return
    orig = bacc.get_activation_tables

    def pinned(arch):
        tabs = orig(arch)
        keep_all = "natural_log_exp_and_others"
        sqrt_home = "sqrt_and_others"
        strip = {AF.Exp, AF.Ln, AF.Copy, AF.Identity, AF.Square, AF.Sqrt}
        out = {}
        for name, funcs in tabs.items():
            if name == keep_all:
                out[name] = funcs
            elif name == sqrt_home:
                out[name] = {f for f in funcs
                             if f not in (strip - {AF.Sqrt})}
            else:
                out[name] = {f for f in funcs if f not in strip}
        return out

    bacc.get_activation_tables = pinned
    bacc._act_tables_pinned = True


def _build():
    if "nc" in _CACHE:
        return _CACHE["nc"]
    _pin_act_tables()
    import concourse.tile_utils as tile_utils
    tile_utils.max_sbuf_usage = 206 * 1024

    nc = bacc.Bacc("TRN2", target_bir_lowering=False, debug=False,
                   num_devices=NCORES, num_swdge_queues=4)

    atom_fullT = nc.dram_tensor("atom_fullT", [64, NFULL], BF16, kind="ExternalInput").ap()
    atom_locT = nc.dram_tensor("atom_locT", [64, NPAD], BF16, kind="ExternalInput").ap()
    angle_t = nc.dram_tensor("angle_t", [KA * 128, NPAD], BF16, kind="ExternalInput").ap()
    nbr_t = nc.dram_tensor("nbr_t", [(M + 1) * 64, NPAD], BF16, kind="ExternalInput").ap()
    idx_w = nc.dram_tensor("idx_w", [128, NCHUNK * (NIDX // 16)], I16, kind="ExternalInput").ap()
    bw = nc.dram_tensor("bw", [M, NPAD], BF16, kind="ExternalInput").ap()
    w1t_c = nc.dram_tensor("w1t_c", [64, 128], BF16, kind="ExternalInput").ap()
    w1t_n = nc.dram_tensor("w1t_n", [64, 128], BF16, kind="ExternalInput").ap()
    w1t_fr = nc.dram_tensor("w1t_fr", [128, 128], BF16, kind="ExternalInput").ap()
    w1t_a = nc.dram_tensor("w1t_a", [KA * 128, 128], BF16, kind="ExternalInput").ap()
    w2t = nc.dram_tensor("w2t", [128, 64], BF16, kind="ExternalInput").ap()
    ident = nc.dram_tensor("ident", [128, 128], BF16, kind="ExternalInput").ap()
    g1 = nc.dram_tensor("g1", [128, 1], F32, kind="ExternalInput").ap()
    be1 = nc.dram_tensor("be1", [128, 1], F32, kind="ExternalInput").ap()
    g2 = nc.dram_tensor("g2", [64, 1], F32, kind="ExternalInput").ap()
    be2 = nc.dram_tensor("be2", [64, 1], F32, kind="ExternalInput").ap()
    out_p = nc.dram_tensor("out_p", [128, NPAIR * 512], BF16, kind="ExternalOutput").ap()

    with tile.TileContext(nc) as tc:
        with (tc.tile_pool(name="consts", bufs=1) as consts,
              tc.tile_pool(name="astr", bufs=2) as astr_pl,
              tc.tile_pool(name="tb", bufs=2) as tb_pl,
              tc.tile_pool(name="atom", bufs=2) as atom_pl,
              tc.tile_pool(name="angle", bufs=2) as angle_pl,
              tc.tile_pool(name="nbr", bufs=2) as nbr_pl,
              tc.tile_pool(name="gath", bufs=2) as gath_pl,
              tc.tile_pool(name="absb", bufs=2) as absb_pl,
              tc.tile_pool(name="h1", bufs=2) as h1_pl,
              tc.tile_pool(name="sp", bufs=4) as sp_pl,
              tc.tile_pool(name="bwbc", bufs=6) as bwbc_pl,
              tc.tile_pool(name="psA", bufs=4, space="PSUM") as psA,
              tc.tile_pool(name="psB", bufs=2, space="PSUM") as psB,
              tc.tile_pool(name="dram", bufs=1, space="DRAM") as dram):

            # ---------------- constants -------------------------------
            ident_sb = consts.tile([128, 128], BF16)
            nc.sync.dma_start(out=ident_sb, in_=ident)
            w1c_sb = consts.tile([64, 128], BF16)
            nc.sync.dma_start(out=w1c_sb, in_=w1t_c)
            w1n_sb = consts.tile([64, 128], BF16)
            nc.sync.dma_start(out=w1n_sb, in_=w1t_n)
            w1fr_sb = consts.tile([128, 128], BF16)
            nc.sync.dma_start(out=w1fr_sb, in_=w1t_fr)
            w1a_sb = consts.tile([128, KA, 128], BF16)
            nc.sync.dma_start(
                out=w1a_sb,
                in_=bass.AP(tensor=w1t_a.tensor, offset=0,
                            ap=[[128, 128], [128 * 128, KA], [1, 128]]))
            w2t_sb = consts.tile([128, 64], BF16)
            nc.sync.dma_start(out=w2t_sb, in_=w2t)
            idx_sb = consts.tile([128, NCHUNK * (NIDX // 16)], I16)
            nc.sync.dma_start(out=idx_sb, in_=idx_w)
            g1_sb = consts.tile([128, 1], F32)
            nc.sync.dma_start(out=g1_sb, in_=g1)
            be1_sb = consts.tile([128, 1], F32)
            nc.sync.dma_start(out=be1_sb, in_=be1)
            g2_sb = consts.tile([64, 1], F32)
            nc.sync.dma_start(out=g2_sb, in_=g2)
            be2_sb = consts.tile([64, 1], F32)
            nc.sync.dma_start(out=be2_sb, in_=be2)
            eps_sb = consts.tile([128, 1], F32)
            nc.vector.memset(eps_sb, BN_EPS)

            stats1 = consts.tile([128, NTILE, 6], F32)
            stats2 = consts.tile([128, NPAIR, 6], F32)
            # z2 pair tiles are overlaid into z1 slots 4g / 4g+1, which are
            # dead once slab g's h1 is computed
            z1_sb = consts.tile([128, NTILE, 512], BF16)

            table_d = dram.tile([NFULL, 128], BF16)
            table_ap = bass.AP(tensor=table_d.tensor, offset=0,
                               ap=[[128, NFULL], [1, 128]])

            # ------- phase 0: bf16 projection table in DRAM -----------
            # table row a = atom_fea[a] @ W1n.T, built atom-major: one
            # matmul per 128-atom rank (atoms land on partitions), so the
            # store to DRAM is a plain contiguous-row DMA. Atom features
            # stream in 5 big pieces to amortize DMA latency.
            a_pieces = []
            for grp in range(TGRP):
                if grp % 4 == 0:
                    a_sb = astr_pl.tile([64, 4096], BF16, tag="astr")
                    nc.sync.dma_start(
                        out=a_sb,
                        in_=atom_fullT[:, grp * 1024:(grp + 4) * 1024])
                    a_pieces.append(a_sb)
                a_sb = a_pieces[-1]
                base = (grp % 4) * 1024
                tb = tb_pl.tile([128, 8, 128], BF16, tag="tb")
                for half in range(2):
                    ps = psA.tile([128, 512], F32, tag="slot")
                    for k in range(4):
                        col = base + (half * 4 + k) * 128
                        nc.tensor.matmul(
                            ps[:, k * 128:(k + 1) * 128],
                            a_sb[:, col:col + 128],
                            w1n_sb[:], start=True, stop=True,
                            skip_group_check=True)
                    nc.vector.tensor_copy(
                        out=tb[:, half * 4:half * 4 + 4, :].rearrange(
                            "p a b -> p (a b)"),
                        in_=ps[:])
                nc.scalar.dma_start(
                    out=bass.AP(tensor=table_d.tensor,
                                offset=grp * 1024 * 128,
                                ap=[[128, 128], [128 * 128, 8], [1, 128]]),
                    in_=tb[:])

            # ---------------- phase 1: z1 assembly + stats -------------
            for c in range(NCHUNK):
                valid = CW if c < NCHUNK - 1 else TAIL
                # per-atom base: center + angle -> psB bank -> SBUF bf16
                at_sb = atom_pl.tile([64, CW], BF16, tag="atom")
                nc.sync.dma_start(out=at_sb,
                                  in_=atom_locT[:, c * CW:(c + 1) * CW])
                ab = psB.tile([128, 512], F32, tag="psB")
                nc.tensor.matmul(ab[:], w1c_sb[:], at_sb[:],
                                 start=True, stop=False)
                an_sb = angle_pl.tile([128, KA, CW], BF16, tag="angle")
                nc.sync.dma_start(
                    out=an_sb,
                    in_=bass.AP(tensor=angle_t.tensor, offset=c * CW,
                                ap=[[NPAD, 128], [128 * NPAD, KA], [1, CW]]))
                for k in range(KA):
                    nc.tensor.matmul(ab[:], w1a_sb[:, k, :], an_sb[:, k, :],
                                     start=False, stop=(k == KA - 1))
                ab_sb = absb_pl.tile([128, 512], BF16, tag="absb")
                nc.scalar.copy(out=ab_sb[:], in_=ab[:])
                # bond-major gathers, split across the 4 SWDGE queues so
                # 4 DMA rings pull table rows concurrently (3 slots each)
                gts = []
                for q in range(4):
                    gt = gath_pl.tile([128, NIDX // 512, 128], BF16,
                                      tag=f"gath{q}")
                    col = c * (NIDX // 16) + q * (NIDX // 64)
                    nc.gpsimd.dma_gather(
                        out_ap=gt[:], in_ap=table_ap,
                        idxs_ap=idx_sb[:, col:col + NIDX // 64],
                        num_idxs=NIDX // 4, num_idxs_reg=NIDX // 4,
                        elem_size=128, transpose=False, single_packet=False,
                        queue_num=q)
                    gts.append(gt)
                # all 12 stacked [nbr_j; nbr_{j+1}] tiles in one DMA
                # (row M is a host-side copy of row 0 for wraparound)
                nbA = nbr_pl.tile([128, M, CW], BF16, tag="nbr")
                nc.sync.dma_start(
                    out=nbA,
                    in_=bass.AP(tensor=nbr_t.tensor, offset=c * CW,
                                ap=[[NPAD, 128], [64 * NPAD, M], [1, CW]]))
                for j in range(M):
                    ps = psA.tile([128, 512], F32, tag="slot")
                    nc.tensor.matmul(ps[:], w1fr_sb[:], nbA[:, j, :],
                                     start=True, stop=False)
                    # transpose-inject gathered nbr_atom rows: G_block.T
                    for k in range(4):
                        nc.tensor.matmul(ps[:, k * 128:(k + 1) * 128],
                                         gts[j // 3][:, 4 * (j % 3) + k, :],
                                         ident_sb[:],
                                         start=False, stop=(k == 3))
                    t = c * M + j
                    z1t = z1_sb[:, t, :]
                    nc.vector.scalar_tensor_tensor(
                        out=z1t, in0=ps[:], scalar=1.0, in1=ab_sb[:],
                        op0=ALU.mult, op1=ALU.add)
                    nc.vector.bn_stats(out=stats1[:, t, :],
                                       in_=z1t[:, 0:valid])

            # ---------------- BN1 stats allreduce ----------------------
            mv1 = consts.tile([128, 2], F32)
            nc.vector.bn_aggr(out=mv1[:], in_=stats1[:])
            pay1 = consts.tile([128, 2], F32)
            msq1 = consts.tile([128, 1], F32)
            nc.scalar.square(out=msq1[:], in_=mv1[:, 0:1])
            nc.vector.tensor_copy(out=pay1[:, 0:1], in_=mv1[:, 0:1])
            nc.vector.tensor_add(out=pay1[:, 1:2], in0=mv1[:, 1:2], in1=msq1[:])
            cc1i = dram.tile([128, 2], F32)
            cc1o = dram.tile([128, 2], F32)
            nc.sync.dma_start(out=cc1i[:], in_=pay1[:])
            nc.gpsimd.collective_compute(
                "AllReduce", ALU.add, replica_groups=[list(range(NCORES))],
                ins=[cc1i[:].opt()], outs=[cc1o[:].opt()])
            S1 = consts.tile([128, 2], F32)
            nc.sync.dma_start(out=S1[:], in_=cc1o[:])
            mean1 = consts.tile([128, 1], F32)
            nc.scalar.mul(out=mean1[:], in_=S1[:, 0:1], mul=1.0 / NCORES)
            mm1 = consts.tile([128, 1], F32)
            nc.scalar.square(out=mm1[:], in_=mean1[:])
            var1 = consts.tile([128, 1], F32)
            nc.vector.scalar_tensor_tensor(
                out=var1[:], in0=S1[:, 1:2], scalar=1.0 / NCORES, in1=mm1[:],
                op0=ALU.mult, op1=ALU.subtract)
            sd1 = consts.tile([128, 1], F32)
            nc.scalar.activation(out=sd1[:], in_=var1[:], func=AF.Sqrt,
                                 bias=eps_sb[:], scale=1.0)
            rs1 = consts.tile([128, 1], F32)
            nc.vector.reciprocal(out=rs1[:], in_=sd1[:])
            scale1 = consts.tile([128, 1], F32)
            nc.vector.tensor_mul(out=scale1[:], in0=rs1[:], in1=g1_sb[:])
            negm1 = consts.tile([128, 1], F32)
            nc.scalar.mul(out=negm1[:], in_=mean1[:], mul=-1.0)
            bias1 = consts.tile([128, 1], F32)
            nc.vector.scalar_tensor_tensor(
                out=bias1[:], in0=scale1[:], scalar=negm1[:], in1=be1_sb[:],
                op0=ALU.mult, op1=ALU.add)

            # ---------------- phase 2: h1, z2, stats2 ------------------
            # softplus per 4-tile slab; W2 as two half-partition matmuls
            # per PSUM bank so downstream tiles are full 128 partitions
            # (partitions 0:64 <- even tile features, 64:128 <- odd).
            for g in range(NSLAB):
                zsl = z1_sb[:, 4 * g:4 * g + 4, :].rearrange("p a b -> p (a b)")
                nc.scalar.activation(out=zsl, in_=zsl, func=AF.Exp,
                                     bias=bias1[:], scale=scale1[:])
                h1s = h1_pl.tile([128, 2048], BF16, tag="h1")
                nc.scalar.activation(out=h1s[:], in_=zsl, func=AF.Ln,
                                     bias=1.0)
                for m in range(2):
                    t = 2 * g + m
                    c = (4 * g + 2 * m) // M
                    valid = CW if c < NCHUNK - 1 else TAIL
                    ps = psB.tile([128, 512], F32, tag="psB")
                    nc.tensor.matmul(ps[0:64, :], w2t_sb[:],
                                     h1s[:, (2 * m) * 512:(2 * m + 1) * 512],
                                     start=True, stop=True,
                                     skip_group_check=True)
                    nc.tensor.matmul(ps[64:128, :], w2t_sb[:],
                                     h1s[:, (2 * m + 1) * 512:(2 * m + 2) * 512],
                                     start=True, stop=True,
                                     skip_group_check=True)
                    nc.vector.bn_stats(out=stats2[:, t, :],
                                       in_=ps[:, 0:valid])
                    nc.vector.tensor_copy(out=z1_sb[:, 4 * g + m, :],
                                          in_=ps[:])

            # ---------------- BN2 stats allreduce ----------------------
            mv2 = consts.tile([128, 2], F32)
            nc.vector.bn_aggr(out=mv2[:], in_=stats2[:])
            pay2 = consts.tile([128, 2], F32)
            msq2 = consts.tile([128, 1], F32)
            nc.scalar.square(out=msq2[:], in_=mv2[:, 0:1])
            nc.vector.tensor_copy(out=pay2[:, 0:1], in_=mv2[:, 0:1])
            nc.vector.tensor_add(out=pay2[:, 1:2], in0=mv2[:, 1:2], in1=msq2[:])
            cc2i = dram.tile([128, 2], F32)
            cc2o = dram.tile([128, 2], F32)
            nc.sync.dma_start(out=cc2i[:], in_=pay2[:])
            nc.gpsimd.collective_compute(
                "AllReduce", ALU.add, replica_groups=[list(range(NCORES))],
                ins=[cc2i[:].opt()], outs=[cc2o[:].opt()])
            # prefetch phase-3 bond weights during the collective (Pool
            # queue: idle here and dispatch is cheap)
            bwts = []
            for g in range(NSLAB):
                bwt = bwbc_pl.tile([128, 2, 512], BF16, tag="bwbc")
                for m in range(2):
                    t = 2 * g + m
                    c, j0 = divmod(2 * t, M)
                    nc.scalar.dma_start(
                        out=bwt[0:64, m, :],
                        in_=bass.AP(tensor=bw.tensor,
                                    offset=j0 * NPAD + c * CW,
                                    ap=[[0, 64], [1, CW]]))
                    nc.scalar.dma_start(
                        out=bwt[64:128, m, :],
                        in_=bass.AP(tensor=bw.tensor,
                                    offset=(j0 + 1) * NPAD + c * CW,
                                    ap=[[0, 64], [1, CW]]))
                bwts.append(bwt)
            S2 = consts.tile([128, 2], F32)
            nc.sync.dma_start(out=S2[:], in_=cc2o[:])
            # fold: partitions 64:128 hold the odd-tile half of each
            # feature's stats; shift down and add for the global sums
            S2s = consts.tile([64, 2], F32)
            nc.sync.dma_start(out=S2s[:], in_=S2[64:128, :])
            S2t = consts.tile([64, 2], F32)
            nc.vector.tensor_add(out=S2t[:], in0=S2[0:64, :], in1=S2s[:])
            mean2 = consts.tile([64, 1], F32)
            nc.scalar.mul(out=mean2[:], in_=S2t[:, 0:1], mul=1.0 / (2 * NCORES))
            mm2 = consts.tile([64, 1], F32)
            nc.scalar.square(out=mm2[:], in_=mean2[:])
            var2 = consts.tile([64, 1], F32)
            nc.vector.scalar_tensor_tensor(
                out=var2[:], in0=S2t[:, 1:2], scalar=1.0 / (2 * NCORES),
                in1=mm2[:], op0=ALU.mult, op1=ALU.subtract)
            sd2 = consts.tile([64, 1], F32)
            nc.scalar.activation(out=sd2[:], in_=var2[:], func=AF.Sqrt,
                                 bias=eps_sb[0:64, :], scale=1.0)
            rs2 = consts.tile([64, 1], F32)
            nc.vector.reciprocal(out=rs2[:], in_=sd2[:])
            scale2 = consts.tile([64, 1], F32)
            nc.vector.tensor_mul(out=scale2[:], in0=rs2[:], in1=g2_sb[:])
            negm2 = consts.tile([64, 1], F32)
            nc.scalar.mul(out=negm2[:], in_=mean2[:], mul=-1.0)
            bias2 = consts.tile([64, 1], F32)
            nc.vector.scalar_tensor_tensor(
                out=bias2[:], in0=scale2[:], scalar=negm2[:], in1=be2_sb[:],
                op0=ALU.mult, op1=ALU.add)
            scale2r = consts.tile([128, 1], F32)
            nc.sync.dma_start(out=scale2r[0:64, :], in_=scale2[:])
            nc.sync.dma_start(out=scale2r[64:128, :], in_=scale2[:])
            bias2r = consts.tile([128, 1], F32)
            nc.sync.dma_start(out=bias2r[0:64, :], in_=bias2[:])
            nc.sync.dma_start(out=bias2r[64:128, :], in_=bias2[:])

            # ---------------- phase 3: softplus2 * bw -> out -----------
            for g in range(NSLAB):
                zf = z1_sb[:, 4 * g:4 * g + 2, :].rearrange("p a b -> p (a b)")
                nc.scalar.activation(out=zf, in_=zf, func=AF.Exp,
                                     bias=bias2r[:], scale=scale2r[:])
                sp = sp_pl.tile([128, 1024], BF16, tag="sp")
                nc.scalar.activation(out=sp[:], in_=zf, func=AF.Ln, bias=1.0)
                nc.vector.tensor_mul(
                    out=sp[:], in0=sp[:],
                    in1=bwts[g][:].rearrange("p a b -> p (a b)"))
                nc.sync.dma_start(
                    out=out_p[:, g * 1024:(g + 1) * 1024], in_=sp[:])

    nc.compile()
    _CACHE["nc"] = nc
    return nc


def _prep_core(c, atom_fea, nbr_fea, nbr_fea_idx, angle_fea, bond_weights,
               shared):
    lo = c * NLOC
    hi = lo + NLOC
    atom_locT = np.zeros((64, NPAD), BF16_NP)
    atom_locT[:, :NLOC] = atom_fea[lo:hi].T.astype(BF16_NP)
    angle_t = np.zeros((KA * 128, NPAD), BF16_NP)
    angle_t[:A * ANG_F, :NLOC] = \
        angle_fea[lo:hi].reshape(NLOC, A * ANG_F).T.astype(BF16_NP)
    nbr_t = np.zeros(((M + 1) * 64, NPAD), BF16_NP)
    nbr_t[:M * 64, :NLOC] = \
        nbr_fea[lo:hi].transpose(1, 2, 0).reshape(M * 64, NLOC).astype(BF16_NP)
    nbr_t[M * 64:, :] = nbr_t[0:64, :]
    bw_p = np.zeros((M, NPAD), BF16_NP)
    bw_p[:, :NLOC] = bond_weights[lo:hi].T.astype(BF16_NP)

    idxp = np.zeros((NPAD, M), np.int16)
    idxp[:NLOC] = nbr_fea_idx[lo:hi].astype(np.int16)
    idx_w = np.zeros((128, NCHUNK * (NIDX // 16)), np.int16)
    for cc in range(NCHUNK):
        flat = idxp[cc * CW:(cc + 1) * CW, :].T.reshape(-1)   # slot-major
        wr = flat.reshape(NIDX // 16, 16).T                   # (16, 384)
        col = cc * (NIDX // 16)
        idx_w[:, col:col + NIDX // 16] = np.tile(wr, (8, 1))

    d = dict(shared)
    d.update(atom_locT=atom_locT, angle_t=angle_t, nbr_t=nbr_t, idx_w=idx_w,
             bw=bw_p)
    return d


def _make_in_maps(inputs):
    """Build per-core input dicts from the full (unsharded) input dict."""
    atom_fea = np.asarray(inputs["atom_fea"], dtype=np.float32)
    nbr_fea = np.asarray(inputs["nbr_fea"], dtype=np.float32)
    nbr_fea_idx = np.asarray(inputs["nbr_fea_idx"])
    angle_fea = np.asarray(inputs["angle_fea"], dtype=np.float32)
    bond_weights = np.asarray(inputs["bond_weights"], dtype=np.float32)
    W1 = np.asarray(inputs["W1"]); W2 = np.asarray(inputs["W2"])
    g1 = np.asarray(inputs["g1"]); be1 = np.asarray(inputs["be1"])
    g2 = np.asarray(inputs["g2"]); be2 = np.asarray(inputs["be2"])

    atom_fullT = np.zeros((64, NFULL), BF16_NP)
    atom_fullT[:, :N] = atom_fea.T.astype(BF16_NP)
    w1t = W1.T.astype(np.float32)
    w1t_a = np.zeros((KA * 128, 128), BF16_NP)
    w1t_a[:A * ANG_F] = w1t[256:1312].astype(BF16_NP)
    shared = dict(
        atom_fullT=atom_fullT,
        w1t_c=np.ascontiguousarray(w1t[0:64]).astype(BF16_NP),
        w1t_n=np.ascontiguousarray(w1t[64:128]).astype(BF16_NP),
        w1t_fr=np.ascontiguousarray(w1t[128:256]).astype(BF16_NP),
        w1t_a=w1t_a,
        w2t=np.ascontiguousarray(W2.T).astype(BF16_NP),
        ident=np.eye(128, dtype=np.float32).astype(BF16_NP),
        g1=g1.reshape(128, 1).astype(np.float32),
        be1=be1.reshape(128, 1).astype(np.float32),
        g2=g2.reshape(64, 1).astype(np.float32),
        be2=be2.reshape(64, 1).astype(np.float32),
    )
    return [_prep_core(c, atom_fea, nbr_fea, nbr_fea_idx, angle_fea,
                       bond_weights, shared)
            for c in range(NCORES)]


def _assemble(results):
    """Per-core out_p buffers -> full (N, M, NBR_F) output."""
    out = np.empty((N, M, NBR_F), np.float32)
    for c in range(NCORES):
        op = np.asarray(results[c]["out_p"]).astype(np.float32)
        lo = c * NLOC
        for t in range(NPAIR):
            cc, j0 = divmod(2 * t, M)
            blk = op[:, t * 512:(t + 1) * 512]
            a0 = cc * CW
            nA = min(CW, NLOC - a0)
            out[lo + a0:lo + a0 + nA, j0, :] = blk[0:64, :nA].T
            out[lo + a0:lo + a0 + nA, j0 + 1, :] = blk[64:128, :nA].T
    return out


def kernel(atom_fea, nbr_fea, nbr_fea_idx, angle_fea, bond_weights,
           W1, b1, g1, be1, W2, b2, g2, be2):
    global LAST_EXEC_NS, LAST_RESULTS
    nc = _build()
    in_maps = _make_in_maps(dict(
        atom_fea=atom_fea, nbr_fea=nbr_fea, nbr_fea_idx=nbr_fea_idx,
        angle_fea=angle_fea, bond_weights=bond_weights, W1=W1, W2=W2,
        g1=g1, be1=be1, g2=g2, be2=be2))

    if TRACE:
        _install_ntff_hook()
    br = run_bass_kernel_spmd(nc, in_maps, list(range(NCORES)), trace=TRACE)
    LAST_EXEC_NS = br.exec_time_ns
    LAST_RESULTS = br
    return _assemble(br.results)


def _install_ntff_hook():
    """Inject antenv.axon_hooks (missing in this image) so trace=True works."""
    import types
    if "antenv.axon_hooks" in sys.modules:
        return
    sys.path.insert(0, "/root/.axon_site")
    mod = types.ModuleType("antenv.axon_hooks")
    mod._hook = None
    mod.set_axon_ntff_profile_hook = lambda h: setattr(mod, "_hook", h)
    mod.get_axon_ntff_profile_hook = lambda: mod._hook
    sys.modules["antenv.axon_hooks"] = mod
    try:
        from trn_agent_boot.trn_boot import _ntff_profile_via_ctypes
        h = _ntff_profile_via_ctypes("/opt/axon/libaxon_pjrt.so")
        if h is not None:
            mod.set_axon_ntff_profile_hook(h)
    except Exception as e:
        print("ntff hook install failed:", e)


# revision 20
# speedup vs baseline: 1.7626x; 1.0298x over previous
"""Trainium2 Bass kernel for nn_BondConvLayer (gnn_message_passing).

8-core data-parallel: 2500 atoms (30000 bonds) per core.

out = softplus(bn2(softplus(bn1(cat @ W1.T)) @ W2.T)) * bw  where
cat = [center, gathered_nbr_atom, nbr_fea, rolled_nbr_fea, angle] per bond;
b1/b2 cancel inside training-mode BatchNorm and are dropped.

v2 layout: everything bf16 on the wire and in the PE. Each core projects
the full atom table atom-major ([atom, 128h] rows, one matmul per
128-atom rank) and stores it to DRAM; per-bond rows are then pulled with
a DRAM-source non-transpose dma_gather (contiguous 256B descriptors -
fast path) arriving bond-major, and re-transposed into the feature-major
z1 PSUM accumulation with identity matmuls on the PE (stat=G block,
mov=I, start=False). nbr f/r projections run as one 128-deep stacked
matmul per slot ([W1f;W1r] weights, one 128-partition DMA spanning
adjacent nbr slots via a wraparound row). The per-atom center+angle base
is added on DVE during PSUM evacuation (scalar_tensor_tensor), which
also casts z1 to bf16 slabs kept in SBUF (no DRAM spill). BN batch
stats: bn_stats/bn_aggr per core + tiny AllReduce; phase 2 runs W2 as
two half-partition matmuls per PSUM bank (tile_position) so softplus /
stats / output work on full 128-partition tiles. Softplus = Exp then
Ln(x+1) on ACT with the BN affine fused in.
"""
import sys, os

sys.path.insert(0, "/opt/trn_rl_repo")

import numpy as np

import concourse.bass as bass
import concourse.bacc as bacc
import concourse.tile as tile
from concourse import mybir
from concourse.bass_utils import run_bass_kernel_spmd

F32 = mybir.dt.float32
BF16 = mybir.dt.bfloat16
I16 = mybir.dt.int16
AF = mybir.ActivationFunctionType
ALU = mybir.AluOpType
BF16_NP = mybir.dt.np(BF16)

NCORES = 8
N, M = 20000, 12
ATOM_F, NBR_F, ANG_F, A = 64, 64, 16, 66
H, O = 128, 64
BN_EPS = 1e-5
NLOC = N // NCORES          # 2500
NCHUNK = 5                  # chunks of 512 atoms (last ragged 452)
CW = 512
NPAD = NCHUNK * CW          # 2560
TAIL = NLOC - (NCHUNK - 1) * CW   # 452
NTILE = NCHUNK * M          # 60 bond tiles per core
NPAIR = NTILE // 2          # 30 paired tiles in phase 2/3
NSLAB = NTILE // 4          # 15 slabs of 4 bond-tiles
KA = 9                      # angle K-chunks of 128 (1056 padded to 1152)
NRANK = 160                 # table ranks (20000 atoms padded to 20480)
NFULL = NRANK * 128         # 20480
NIDX = M * CW               # 6144 gather indices per chunk (one call)
TGRP = 20                   # table build groups of 8 ranks

_CACHE = {}
TRACE = bool(int(os.environ.get("BASS_KERNEL_TRACE", "0")))
LAST_EXEC_NS = None
LAST_RESULTS = None


def _pin_act_tables():
    """Restrict the activation-table sets bacc may choose so Exp/Ln/Copy/
    Square all land in natural_log_exp_and_others (one load, no per-op
    table swaps). Set names/order (= act_func_set_id) are preserved."""
    if getattr(bacc, "_act_tables_pinned", False):
        return
    orig = bacc.get_activation_tables

    def pinned(arch):
        tabs = orig(arch)
        keep_all = "natural_log_exp_and_others"
        sqrt_home = "sqrt_and_others"
        strip = {AF.Exp, AF.Ln, AF.Copy, AF.Identity, AF.Square, AF.Sqrt}
        out = {}
        for name, funcs in tabs.items():
            if name == keep_all:
                out[name] = funcs
            elif name == sqrt_home:
                out[name] = {f for f in funcs
                             if f not in (strip - {AF.Sqrt})}
            else:
                out[name] = {f for f in funcs if f not in strip}
        return out

    bacc.get_activation_tables = pinned
    bacc._act_tables_pinned = True


def _build():
    if "nc" in _CACHE:
        return _CACHE["nc"]
    _pin_act_tables()
    import concourse.tile_utils as tile_utils
    tile_utils.max_sbuf_usage = 206 * 1024

    nc = bacc.Bacc("TRN2", target_bir_lowering=False, debug=False,
                   num_devices=NCORES, num_swdge_queues=4)

    atom_fullT = nc.dram_tensor("atom_fullT", [64, NFULL], BF16, kind="ExternalInput").ap()
    atom_locT = nc.dram_tensor("atom_locT", [64, NPAD], BF16, kind="ExternalInput").ap()
    angle_t = nc.dram_tensor("angle_t", [KA * 128, NPAD], BF16, kind="ExternalInput").ap()
    nbr_t = nc.dram_tensor("nbr_t", [(M + 1) * 64, NPAD], BF16, kind="ExternalInput").ap()
    idx_w = nc.dram_tensor("idx_w", [128, NCHUNK * (NIDX // 16)], I16, kind="ExternalInput").ap()
    bw = nc.dram_tensor("bw", [M, NPAD], BF16, kind="ExternalInput").ap()
    w1t_c = nc.dram_tensor("w1t_c", [64, 128], BF16, kind="ExternalInput").ap()
    w1t_n = nc.dram_tensor("w1t_n", [64, 128], BF16, kind="ExternalInput").ap()
    w1t_fr = nc.dram_tensor("w1t_fr", [128, 128], BF16, kind="ExternalInput").ap()
    w1t_a = nc.dram_tensor("w1t_a", [KA * 128, 128], BF16, kind="ExternalInput").ap()
    w2t = nc.dram_tensor("w2t", [128, 64], BF16, kind="ExternalInput").ap()
    ident = nc.dram_tensor("ident", [128, 128], BF16, kind="ExternalInput").ap()
    g1 = nc.dram_tensor("g1", [128, 1], F32, kind="ExternalInput").ap()
    be1 = nc.dram_tensor("be1", [128, 1], F32, kind="ExternalInput").ap()
    g2 = nc.dram_tensor("g2", [64, 1], F32, kind="ExternalInput").ap()
    be2 = nc.dram_tensor("be2", [64, 1], F32, kind="ExternalInput").ap()
    out_p = nc.dram_tensor("out_p", [128, NPAIR * 512], BF16, kind="ExternalOutput").ap()

    with tile.TileContext(nc) as tc:
        with (tc.tile_pool(name="consts", bufs=1) as consts,
              tc.tile_pool(name="astr", bufs=3) as astr_pl,
              tc.tile_pool(name="tb", bufs=2) as tb_pl,
              tc.tile_pool(name="atom", bufs=2) as atom_pl,
              tc.tile_pool(name="angle", bufs=2) as angle_pl,
              tc.tile_pool(name="nbr", bufs=2) as nbr_pl,
              tc.tile_pool(name="gath", bufs=2) as gath_pl,
              tc.tile_pool(name="absb", bufs=2) as absb_pl,
              tc.tile_pool(name="h1", bufs=2) as h1_pl,
              tc.tile_pool(name="sp", bufs=4) as sp_pl,
              tc.tile_pool(name="sq", bufs=2) as sq_pl,
              tc.tile_pool(name="bwbc", bufs=6) as bwbc_pl,
              tc.tile_pool(name="psA", bufs=4, space="PSUM") as psA,
              tc.tile_pool(name="psB", bufs=2, space="PSUM") as psB,
              tc.tile_pool(name="dram", bufs=1, space="DRAM") as dram):

            # ---------------- constants -------------------------------
            ident_sb = consts.tile([128, 128], BF16)
            nc.sync.dma_start(out=ident_sb, in_=ident)
            w1c_sb = consts.tile([64, 128], BF16)
            nc.sync.dma_start(out=w1c_sb, in_=w1t_c)
            w1n_sb = consts.tile([64, 128], BF16)
            nc.sync.dma_start(out=w1n_sb, in_=w1t_n)
            w1fr_sb = consts.tile([128, 128], BF16)
            nc.sync.dma_start(out=w1fr_sb, in_=w1t_fr)
            w1a_sb = consts.tile([128, KA, 128], BF16)
            nc.sync.dma_start(
                out=w1a_sb,
                in_=bass.AP(tensor=w1t_a.tensor, offset=0,
                            ap=[[128, 128], [128 * 128, KA], [1, 128]]))
            w2t_sb = consts.tile([128, 64], BF16)
            nc.sync.dma_start(out=w2t_sb, in_=w2t)
            idx_sb = consts.tile([128, NCHUNK * (NIDX // 16)], I16)
            nc.sync.dma_start(out=idx_sb, in_=idx_w)
            g1_sb = consts.tile([128, 1], F32)
            nc.sync.dma_start(out=g1_sb, in_=g1)
            be1_sb = consts.tile([128, 1], F32)
            nc.sync.dma_start(out=be1_sb, in_=be1)
            g2_sb = consts.tile([64, 1], F32)
            nc.sync.dma_start(out=g2_sb, in_=g2)
            be2_sb = consts.tile([64, 1], F32)
            nc.sync.dma_start(out=be2_sb, in_=be2)
            eps_sb = consts.tile([128, 1], F32)
            nc.vector.memset(eps_sb, BN_EPS)

            s1 = consts.tile([128, NTILE], F32)
            q1 = consts.tile([128, NTILE], F32)
            stats2 = consts.tile([128, NPAIR, 6], F32)
            # z2 pair tiles are overlaid into z1 slots 4g / 4g+1, which are
            # dead once slab g's h1 is computed
            z1_sb = consts.tile([128, NTILE, 512], BF16)

            table_d = dram.tile([NFULL, 128], BF16)
            table_ap = bass.AP(tensor=table_d.tensor, offset=0,
                               ap=[[128, NFULL], [1, 128]])

            # ------- phase 0: bf16 projection table in DRAM -----------
            # table row a = atom_fea[a] @ W1n.T, built atom-major: one
            # matmul per 128-atom rank (atoms land on partitions), so the
            # store to DRAM is a plain contiguous-row DMA. Atom features
            # stream in 5 big pieces to amortize DMA latency.
            a_sb = None
            for grp in range(TGRP):
                if grp % 2 == 0:
                    a_sb = astr_pl.tile([64, 2048], BF16, tag="astr")
                    nc.sync.dma_start(
                        out=a_sb,
                        in_=atom_fullT[:, grp * 1024:(grp + 2) * 1024])
                base = (grp % 2) * 1024
                tb = tb_pl.tile([128, 8, 128], BF16, tag="tb")
                for half in range(2):
                    ps = psA.tile([128, 512], F32, tag="slot")
                    for k in range(4):
                        col = base + (half * 4 + k) * 128
                        nc.tensor.matmul(
                            ps[:, k * 128:(k + 1) * 128],
                            a_sb[:, col:col + 128],
                            w1n_sb[:], start=True, stop=True,
                            skip_group_check=True)
                    nc.vector.tensor_copy(
                        out=tb[:, half * 4:half * 4 + 4, :].rearrange(
                            "p a b -> p (a b)"),
                        in_=ps[:])
                nc.scalar.dma_start(
                    out=bass.AP(tensor=table_d.tensor,
                                offset=grp * 1024 * 128,
                                ap=[[128, 128], [128 * 128, 8], [1, 128]]),
                    in_=tb[:])

            # ---------------- phase 1: z1 assembly + stats -------------
            for c in range(NCHUNK):
                valid = CW if c < NCHUNK - 1 else TAIL
                # per-atom base: center + angle -> psB bank -> SBUF bf16
                at_sb = atom_pl.tile([64, CW], BF16, tag="atom")
                nc.sync.dma_start(out=at_sb,
                                  in_=atom_locT[:, c * CW:(c + 1) * CW])
                ab = psB.tile([128, 512], F32, tag="psB")
                nc.tensor.matmul(ab[:], w1c_sb[:], at_sb[:],
                                 start=True, stop=False)
                an_sb = angle_pl.tile([128, KA, CW], BF16, tag="angle")
                nc.sync.dma_start(
                    out=an_sb,
                    in_=bass.AP(tensor=angle_t.tensor, offset=c * CW,
                                ap=[[NPAD, 128], [128 * NPAD, KA], [1, CW]]))
                for k in range(KA):
                    nc.tensor.matmul(ab[:], w1a_sb[:, k, :], an_sb[:, k, :],
                                     start=False, stop=(k == KA - 1))
                ab_sb = absb_pl.tile([128, 512], BF16, tag="absb")
                nc.scalar.copy(out=ab_sb[:], in_=ab[:])
                # bond-major gathers, split across the 4 SWDGE queues so
                # 4 DMA rings pull table rows concurrently (3 slots each)
                gts = []
                for q in range(4):
                    gt = gath_pl.tile([128, NIDX // 512, 128], BF16,
                                      tag=f"gath{q}")
                    col = c * (NIDX // 16) + q * (NIDX // 64)
                    nc.gpsimd.dma_gather(
                        out_ap=gt[:], in_ap=table_ap,
                        idxs_ap=idx_sb[:, col:col + NIDX // 64],
                        num_idxs=NIDX // 4, num_idxs_reg=NIDX // 4,
                        elem_size=128, transpose=False, single_packet=False,
                        queue_num=q)
                    gts.append(gt)
                # all 12 stacked [nbr_j; nbr_{j+1}] tiles in one DMA
                # (row M is a host-side copy of row 0 for wraparound)
                nbA = nbr_pl.tile([128, M, CW], BF16, tag="nbr")
                nc.sync.dma_start(
                    out=nbA,
                    in_=bass.AP(tensor=nbr_t.tensor, offset=c * CW,
                                ap=[[NPAD, 128], [64 * NPAD, M], [1, CW]]))
                for j in range(M):
                    ps = psA.tile([128, 512], F32, tag="slot")
                    nc.tensor.matmul(ps[:], w1fr_sb[:], nbA[:, j, :],
                                     start=True, stop=False)
                    # transpose-inject gathered nbr_atom rows: G_block.T
                    for k in range(4):
                        nc.tensor.matmul(ps[:, k * 128:(k + 1) * 128],
                                         gts[j // 3][:, 4 * (j % 3) + k, :],
                                         ident_sb[:],
                                         start=False, stop=(k == 3))
                    t = c * M + j
                    z1t = z1_sb[:, t, :]
                    nc.vector.scalar_tensor_tensor(
                        out=z1t, in0=ps[:], scalar=1.0, in1=ab_sb[:],
                        op0=ALU.mult, op1=ALU.add,
                        accum_out=s1[:, t:t + 1])
                    sq = sq_pl.tile([128, 512], BF16, tag="sq")
                    nc.scalar.activation(out=sq[:], in_=z1t, func=AF.Square,
                                         accum_out=q1[:, t:t + 1])

            # ---------------- BN1 stats allreduce ----------------------
            # payload is plain [sum(x), sum(x^2)] per feature; pad bonds
            # gather the zero table row so full-width accums are exact
            pay1 = consts.tile([128, 2], F32)
            nc.vector.reduce_sum(out=pay1[:, 0:1], in_=s1[:],
                                 axis=mybir.AxisListType.X)
            nc.vector.reduce_sum(out=pay1[:, 1:2], in_=q1[:],
                                 axis=mybir.AxisListType.X)
            cc1i = dram.tile([128, 2], F32)
            cc1o = dram.tile([128, 2], F32)
            nc.sync.dma_start(out=cc1i[:], in_=pay1[:])
            nc.gpsimd.collective_compute(
                "AllReduce", ALU.add, replica_groups=[list(range(NCORES))],
                ins=[cc1i[:].opt()], outs=[cc1o[:].opt()])
            S1 = consts.tile([128, 2], F32)
            nc.sync.dma_start(out=S1[:], in_=cc1o[:])
            mean1 = consts.tile([128, 1], F32)
            nc.scalar.mul(out=mean1[:], in_=S1[:, 0:1], mul=1.0 / (N * M))
            mm1 = consts.tile([128, 1], F32)
            nc.scalar.square(out=mm1[:], in_=mean1[:])
            var1 = consts.tile([128, 1], F32)
            nc.vector.scalar_tensor_tensor(
                out=var1[:], in0=S1[:, 1:2], scalar=1.0 / (N * M), in1=mm1[:],
                op0=ALU.mult, op1=ALU.subtract)
            sd1 = consts.tile([128, 1], F32)
            nc.scalar.activation(out=sd1[:], in_=var1[:], func=AF.Sqrt,
                                 bias=eps_sb[:], scale=1.0)
            rs1 = consts.tile([128, 1], F32)
            nc.vector.reciprocal(out=rs1[:], in_=sd1[:])
            scale1 = consts.tile([128, 1], F32)
            nc.vector.tensor_mul(out=scale1[:], in0=rs1[:], in1=g1_sb[:])
            negm1 = consts.tile([128, 1], F32)
            nc.scalar.mul(out=negm1[:], in_=mean1[:], mul=-1.0)
            bias1 = consts.tile([128, 1], F32)
            nc.vector.scalar_tensor_tensor(
                out=bias1[:], in0=scale1[:], scalar=negm1[:], in1=be1_sb[:],
                op0=ALU.mult, op1=ALU.add)

            # ---------------- phase 2: h1, z2, stats2 ------------------
            # softplus per 4-tile slab; W2 as two half-partition matmuls
            # per PSUM bank so downstream tiles are full 128 partitions
            # (partitions 0:64 <- even tile features, 64:128 <- odd).
            for g in range(NSLAB):
                zsl = z1_sb[:, 4 * g:4 * g + 4, :].rearrange("p a b -> p (a b)")
                nc.scalar.activation(out=zsl, in_=zsl, func=AF.Exp,
                                     bias=bias1[:], scale=scale1[:])
                h1s = h1_pl.tile([128, 2048], BF16, tag="h1")
                nc.scalar.activation(out=h1s[:], in_=zsl, func=AF.Ln,
                                     bias=1.0)
                for m in range(2):
                    t = 2 * g + m
                    c = (4 * g + 2 * m) // M
                    valid = CW if c < NCHUNK - 1 else TAIL
                    ps = psB.tile([128, 512], F32, tag="psB")
                    nc.tensor.matmul(ps[0:64, :], w2t_sb[:],
                                     h1s[:, (2 * m) * 512:(2 * m + 1) * 512],
                                     start=True, stop=True,
                                     skip_group_check=True)
                    nc.tensor.matmul(ps[64:128, :], w2t_sb[:],
                                     h1s[:, (2 * m + 1) * 512:(2 * m + 2) * 512],
                                     start=True, stop=True,
                                     skip_group_check=True)
                    nc.vector.bn_stats(out=stats2[:, t, :],
                                       in_=ps[:, 0:valid])
                    nc.vector.tensor_copy(out=z1_sb[:, 4 * g + m, :],
                                          in_=ps[:])

            # ---------------- BN2 stats allreduce ----------------------
            mv2 = consts.tile([128, 2], F32)
            nc.vector.bn_aggr(out=mv2[:], in_=stats2[:])
            pay2 = consts.tile([128, 2], F32)
            msq2 = consts.tile([128, 1], F32)
            nc.scalar.square(out=msq2[:], in_=mv2[:, 0:1])
            nc.vector.tensor_copy(out=pay2[:, 0:1], in_=mv2[:, 0:1])
            nc.vector.tensor_add(out=pay2[:, 1:2], in0=mv2[:, 1:2], in1=msq2[:])
            cc2i = dram.tile([128, 2], F32)
            cc2o = dram.tile([128, 2], F32)
            nc.sync.dma_start(out=cc2i[:], in_=pay2[:])
            nc.gpsimd.collective_compute(
                "AllReduce", ALU.add, replica_groups=[list(range(NCORES))],
                ins=[cc2i[:].opt()], outs=[cc2o[:].opt()])
            # prefetch phase-3 bond weights during the collective (Pool
            # queue: idle here and dispatch is cheap)
            bwts = []
            for g in range(NSLAB):
                bwt = bwbc_pl.tile([128, 2, 512], BF16, tag="bwbc")
                for m in range(2):
                    t = 2 * g + m
                    c, j0 = divmod(2 * t, M)
                    nc.scalar.dma_start(
                        out=bwt[0:64, m, :],
                        in_=bass.AP(tensor=bw.tensor,
                                    offset=j0 * NPAD + c * CW,
                                    ap=[[0, 64], [1, CW]]))
                    nc.scalar.dma_start(
                        out=bwt[64:128, m, :],
                        in_=bass.AP(tensor=bw.tensor,
                                    offset=(j0 + 1) * NPAD + c * CW,
                                    ap=[[0, 64], [1, CW]]))
                bwts.append(bwt)
            S2 = consts.tile([128, 2], F32)
            nc.sync.dma_start(out=S2[:], in_=cc2o[:])
            # fold: partitions 64:128 hold the odd-tile half of each
            # feature's stats; shift down and add for the global sums
            S2s = consts.tile([64, 2], F32)
            nc.sync.dma_start(out=S2s[:], in_=S2[64:128, :])
            S2t = consts.tile([64, 2], F32)
            nc.vector.tensor_add(out=S2t[:], in0=S2[0:64, :], in1=S2s[:])
            mean2 = consts.tile([64, 1], F32)
            nc.scalar.mul(out=mean2[:], in_=S2t[:, 0:1], mul=1.0 / (2 * NCORES))
            mm2 = consts.tile([64, 1], F32)
            nc.scalar.square(out=mm2[:], in_=mean2[:])
            var2 = consts.tile([64, 1], F32)
            nc.vector.scalar_tensor_tensor(
                out=var2[:], in0=S2t[:, 1:2], scalar=1.0 / (2 * NCORES),
                in1=mm2[:], op0=ALU.mult, op1=ALU.subtract)
            sd2 = consts.tile([64, 1], F32)
            nc.scalar.activation(out=sd2[:], in_=var2[:], func=AF.Sqrt,
                                 bias=eps_sb[0:64, :], scale=1.0)
            rs2 = consts.tile([64, 1], F32)
            nc.vector.reciprocal(out=rs2[:], in_=sd2[:])
            scale2 = consts.tile([64, 1], F32)
            nc.vector.tensor_mul(out=scale2[:], in0=rs2[:], in1=g2_sb[:])
            negm2 = consts.tile([64, 1], F32)
            nc.scalar.mul(out=negm2[:], in_=mean2[:], mul=-1.0)
            bias2 = consts.tile([64, 1], F32)
            nc.vector.scalar_tensor_tensor(
                out=bias2[:], in0=scale2[:], scalar=negm2[:], in1=be2_sb[:],
                op0=ALU.mult, op1=ALU.add)
            scale2r = consts.tile([128, 1], F32)
            nc.sync.dma_start(out=scale2r[0:64, :], in_=scale2[:])
            nc.sync.dma_start(out=scale2r[64:128, :], in_=scale2[:])
            bias2r = consts.tile([128, 1], F32)
            nc.sync.dma_start(out=bias2r[0:64, :], in_=bias2[:])
            nc.sync.dma_start(out=bias2r[64:128, :], in_=bias2[:])

            # ---------------- phase 3: softplus2 * bw -> out -----------
            for g in range(NSLAB):
                zf = z1_sb[:, 4 * g:4 * g + 2, :].rearrange("p a b -> p (a b)")
                nc.scalar.activation(out=zf, in_=zf, func=AF.Exp,
                                     bias=bias2r[:], scale=scale2r[:])
                sp = sp_pl.tile([128, 1024], BF16, tag="sp")
                nc.scalar.activation(out=sp[:], in_=zf, func=AF.Ln, bias=1.0)
                nc.vector.tensor_mul(
                    out=sp[:], in0=sp[:],
                    in1=bwts[g][:].rearrange("p a b -> p (a b)"))
                nc.sync.dma_start(
                    out=out_p[:, g * 1024:(g + 1) * 1024], in_=sp[:])

    nc.compile()
    _CACHE["nc"] = nc
    return nc


def _prep_core(c, atom_fea, nbr_fea, nbr_fea_idx, angle_fea, bond_weights,
               shared):
    lo = c * NLOC
    hi = lo + NLOC
    atom_locT = np.zeros((64, NPAD), BF16_NP)
    atom_locT[:, :NLOC] = atom_fea[lo:hi].T.astype(BF16_NP)
    angle_t = np.zeros((KA * 128, NPAD), BF16_NP)
    angle_t[:A * ANG_F, :NLOC] = \
        angle_fea[lo:hi].reshape(NLOC, A * ANG_F).T.astype(BF16_NP)
    nbr_t = np.zeros(((M + 1) * 64, NPAD), BF16_NP)
    nbr_t[:M * 64, :NLOC] = \
        nbr_fea[lo:hi].transpose(1, 2, 0).reshape(M * 64, NLOC).astype(BF16_NP)
    nbr_t[M * 64:, :] = nbr_t[0:64, :]
    bw_p = np.zeros((M, NPAD), BF16_NP)
    bw_p[:, :NLOC] = bond_weights[lo:hi].T.astype(BF16_NP)

    idxp = np.full((NPAD, M), N, np.int16)   # pad bonds -> zero table row
    idxp[:NLOC] = nbr_fea_idx[lo:hi].astype(np.int16)
    idx_w = np.zeros((128, NCHUNK * (NIDX // 16)), np.int16)
    for cc in range(NCHUNK):
        flat = idxp[cc * CW:(cc + 1) * CW, :].T.reshape(-1)   # slot-major
        wr = flat.reshape(NIDX // 16, 16).T                   # (16, 384)
        col = cc * (NIDX // 16)
        idx_w[:, col:col + NIDX // 16] = np.tile(wr, (8, 1))

    d = dict(shared)
    d.update(atom_locT=atom_locT, angle_t=angle_t, nbr_t=nbr_t, idx_w=idx_w,
             bw=bw_p)
    return d


def _make_in_maps(inputs):
    """Build per-core input dicts from the full (unsharded) input dict."""
    atom_fea = np.asarray(inputs["atom_fea"], dtype=np.float32)
    nbr_fea = np.asarray(inputs["nbr_fea"], dtype=np.float32)
    nbr_fea_idx = np.asarray(inputs["nbr_fea_idx"])
    angle_fea = np.asarray(inputs["angle_fea"], dtype=np.float32)
    bond_weights = np.asarray(inputs["bond_weights"], dtype=np.float32)
    W1 = np.asarray(inputs["W1"]); W2 = np.asarray(inputs["W2"])
    g1 = np.asarray(inputs["g1"]); be1 = np.asarray(inputs["be1"])
    g2 = np.asarray(inputs["g2"]); be2 = np.asarray(inputs["be2"])

    atom_fullT = np.zeros((64, NFULL), BF16_NP)
    atom_fullT[:, :N] = atom_fea.T.astype(BF16_NP)
    w1t = W1.T.astype(np.float32)
    w1t_a = np.zeros((KA * 128, 128), BF16_NP)
    w1t_a[:A * ANG_F] = w1t[256:1312].astype(BF16_NP)
    shared = dict(
        atom_fullT=atom_fullT,
        w1t_c=np.ascontiguousarray(w1t[0:64]).astype(BF16_NP),
        w1t_n=np.ascontiguousarray(w1t[64:128]).astype(BF16_NP),
        w1t_fr=np.ascontiguousarray(w1t[128:256]).astype(BF16_NP),
        w1t_a=w1t_a,
        w2t=np.ascontiguousarray(W2.T).astype(BF16_NP),
        ident=np.eye(128, dtype=np.float32).astype(BF16_NP),
        g1=g1.reshape(128, 1).astype(np.float32),
        be1=be1.reshape(128, 1).astype(np.float32),
        g2=g2.reshape(64, 1).astype(np.float32),
        be2=be2.reshape(64, 1).astype(np.float32),
    )
    return [_prep_core(c, atom_fea, nbr_fea, nbr_fea_idx, angle_fea,
                       bond_weights, shared)
            for c in range(NCORES)]


def _assemble(results):
    """Per-core out_p buffers -> full (N, M, NBR_F) output."""
    out = np.empty((N, M, NBR_F), np.float32)
    for c in range(NCORES):
        op = np.asarray(results[c]["out_p"]).astype(np.float32)
        lo = c * NLOC
        for t in range(NPAIR):
            cc, j0 = divmod(2 * t, M)
            blk = op[:, t * 512:(t + 1) * 512]
            a0 = cc * CW
            nA = min(CW, NLOC - a0)
            out[lo + a0:lo + a0 + nA, j0, :] = blk[0:64, :nA].T
            out[lo + a0:lo + a0 + nA, j0 + 1, :] = blk[64:128, :nA].T
    return out


def kernel(atom_fea, nbr_fea, nbr_fea_idx, angle_fea, bond_weights,
           W1, b1, g1, be1, W2, b2, g2, be2):
    global LAST_EXEC_NS, LAST_RESULTS
    nc = _build()
    in_maps = _make_in_maps(dict(
        atom_fea=atom_fea, nbr_fea=nbr_fea, nbr_fea_idx=nbr_fea_idx,
        angle_fea=angle_fea, bond_weights=bond_weights, W1=W1, W2=W2,
        g1=g1, be1=be1, g2=g2, be2=be2))

    if TRACE:
        _install_ntff_hook()
    br = run_bass_kernel_spmd(nc, in_maps, list(range(NCORES)), trace=TRACE)
    LAST_EXEC_NS = br.exec_time_ns
    LAST_RESULTS = br
    return _assemble(br.results)


def _install_ntff_hook():
    """Inject antenv.axon_hooks (missing in this image) so trace=True works."""
    import types
    if "antenv.axon_hooks" in sys.modules:
        return
    sys.path.insert(0, "/root/.axon_site")
    mod = types.ModuleType("antenv.axon_hooks")
    mod._hook = None
    mod.set_axon_ntff_profile_hook = lambda h: setattr(mod, "_hook", h)
    mod.get_axon_ntff_profile_hook = lambda: mod._hook
    sys.modules["antenv.axon_hooks"] = mod
    try:
        from trn_agent_boot.trn_boot import _ntff_profile_via_ctypes
        h = _ntff_profile_via_ctypes("/opt/axon/libaxon_pjrt.so")
        if h is not None:
            mod.set_axon_ntff_profile_hook(h)
    except Exception as e:
        print("ntff hook install failed:", e)


# revision 21
# speedup vs baseline: 1.8324x; 1.0396x over previous
"""Trainium2 Bass kernel for nn_BondConvLayer (gnn_message_passing).

8-core data-parallel: 2500 atoms (30000 bonds) per core.

out = softplus(bn2(softplus(bn1(cat @ W1.T)) @ W2.T)) * bw  where
cat = [center, gathered_nbr_atom, nbr_fea, rolled_nbr_fea, angle] per bond;
b1/b2 cancel inside training-mode BatchNorm and are dropped.

v2 layout: everything bf16 on the wire and in the PE. Each core projects
the full atom table atom-major ([atom, 128h] rows, one matmul per
128-atom rank) and stores it to DRAM; per-bond rows are then pulled with
a DRAM-source non-transpose dma_gather (contiguous 256B descriptors -
fast path) arriving bond-major, and re-transposed into the feature-major
z1 PSUM accumulation with identity matmuls on the PE (stat=G block,
mov=I, start=False). nbr f/r projections run as one 128-deep stacked
matmul per slot ([W1f;W1r] weights, one 128-partition DMA spanning
adjacent nbr slots via a wraparound row). The per-atom center+angle base
is added on DVE during PSUM evacuation (scalar_tensor_tensor), which
also casts z1 to bf16 slabs kept in SBUF (no DRAM spill). BN batch
stats: bn_stats/bn_aggr per core + tiny AllReduce; phase 2 runs W2 as
two half-partition matmuls per PSUM bank (tile_position) so softplus /
stats / output work on full 128-partition tiles. Softplus = Exp then
Ln(x+1) on ACT with the BN affine fused in.
"""
import sys, os

sys.path.insert(0, "/opt/trn_rl_repo")

import numpy as np

import concourse.bass as bass
import concourse.bacc as bacc
import concourse.tile as tile
from concourse import mybir
from concourse.bass_utils import run_bass_kernel_spmd

F32 = mybir.dt.float32
BF16 = mybir.dt.bfloat16
I16 = mybir.dt.int16
AF = mybir.ActivationFunctionType
ALU = mybir.AluOpType
BF16_NP = mybir.dt.np(BF16)

NCORES = 8
N, M = 20000, 12
ATOM_F, NBR_F, ANG_F, A = 64, 64, 16, 66
H, O = 128, 64
BN_EPS = 1e-5
NLOC = N // NCORES          # 2500
NCHUNK = 5                  # chunks of 512 atoms (last ragged 452)
CW = 512
NPAD = NCHUNK * CW          # 2560
TAIL = NLOC - (NCHUNK - 1) * CW   # 452
NTILE = NCHUNK * M          # 60 bond tiles per core
NPAIR = NTILE // 2          # 30 paired tiles in phase 2/3
NSLAB = NTILE // 4          # 15 slabs of 4 bond-tiles
KA = 9                      # angle K-chunks of 128 (1056 padded to 1152)
NRANK = 160                 # table ranks (20000 atoms padded to 20480)
NFULL = NRANK * 128         # 20480
NIDX = M * CW               # 6144 gather indices per chunk (one call)
TGRP = 20                   # table build groups of 8 ranks

_CACHE = {}
TRACE = bool(int(os.environ.get("BASS_KERNEL_TRACE", "0")))
LAST_EXEC_NS = None
LAST_RESULTS = None


def _pin_act_tables():
    """Restrict the activation-table sets bacc may choose so Exp/Ln/Copy/
    Square all land in natural_log_exp_and_others (one load, no per-op
    table swaps). Set names/order (= act_func_set_id) are preserved."""
    if getattr(bacc, "_act_tables_pinned", False):
        return
    orig = bacc.get_activation_tables

    def pinned(arch):
        tabs = orig(arch)
        keep_all = "natural_log_exp_and_others"
        sqrt_home = "sqrt_and_others"
        strip = {AF.Exp, AF.Ln, AF.Copy, AF.Identity, AF.Square, AF.Sqrt}
        out = {}
        for name, funcs in tabs.items():
            if name == keep_all:
                out[name] = funcs
            elif name == sqrt_home:
                out[name] = {f for f in funcs
                             if f not in (strip - {AF.Sqrt})}
            else:
                out[name] = {f for f in funcs if f not in strip}
        return out

    bacc.get_activation_tables = pinned
    bacc._act_tables_pinned = True


def _build():
    if "nc" in _CACHE:
        return _CACHE["nc"]
    _pin_act_tables()
    import concourse.tile_utils as tile_utils
    tile_utils.max_sbuf_usage = 206 * 1024

    nc = bacc.Bacc("TRN2", target_bir_lowering=False, debug=False,
                   num_devices=NCORES, num_swdge_queues=4)

    atom_fullT = nc.dram_tensor("atom_fullT", [64, NFULL], BF16, kind="ExternalInput").ap()
    atom_locT = nc.dram_tensor("atom_locT", [64, NPAD], BF16, kind="ExternalInput").ap()
    angle_t = nc.dram_tensor("angle_t", [KA * 128, NPAD], BF16, kind="ExternalInput").ap()
    nbr_t = nc.dram_tensor("nbr_t", [(M + 1) * 64, NPAD], BF16, kind="ExternalInput").ap()
    idx_w = nc.dram_tensor("idx_w", [128, NCHUNK * (NIDX // 16)], I16, kind="ExternalInput").ap()
    bw = nc.dram_tensor("bw", [M, NPAD], BF16, kind="ExternalInput").ap()
    w1t_c = nc.dram_tensor("w1t_c", [64, 128], BF16, kind="ExternalInput").ap()
    w1t_n = nc.dram_tensor("w1t_n", [64, 128], BF16, kind="ExternalInput").ap()
    w1t_fr = nc.dram_tensor("w1t_fr", [128, 128], BF16, kind="ExternalInput").ap()
    w1t_a = nc.dram_tensor("w1t_a", [KA * 128, 128], BF16, kind="ExternalInput").ap()
    w2t = nc.dram_tensor("w2t", [128, 64], BF16, kind="ExternalInput").ap()
    ident = nc.dram_tensor("ident", [128, 128], BF16, kind="ExternalInput").ap()
    g1 = nc.dram_tensor("g1", [128, 1], F32, kind="ExternalInput").ap()
    be1 = nc.dram_tensor("be1", [128, 1], F32, kind="ExternalInput").ap()
    g2 = nc.dram_tensor("g2", [64, 1], F32, kind="ExternalInput").ap()
    be2 = nc.dram_tensor("be2", [64, 1], F32, kind="ExternalInput").ap()
    out_p = nc.dram_tensor("out_p", [128, NPAIR * 512], BF16, kind="ExternalOutput").ap()

    with tile.TileContext(nc) as tc:
        with (tc.tile_pool(name="consts", bufs=1) as consts,
              tc.tile_pool(name="astr", bufs=4) as astr_pl,
              tc.tile_pool(name="tb", bufs=6) as tb_pl,
              tc.tile_pool(name="atom", bufs=2) as atom_pl,
              tc.tile_pool(name="angle", bufs=2) as angle_pl,
              tc.tile_pool(name="nbr", bufs=2) as nbr_pl,
              tc.tile_pool(name="gath", bufs=2) as gath_pl,
              tc.tile_pool(name="absb", bufs=2) as absb_pl,
              tc.tile_pool(name="h1", bufs=2) as h1_pl,
              tc.tile_pool(name="sp", bufs=4) as sp_pl,
              tc.tile_pool(name="sq", bufs=2) as sq_pl,
              tc.tile_pool(name="bwbc", bufs=6) as bwbc_pl,
              tc.tile_pool(name="psA", bufs=4, space="PSUM") as psA,
              tc.tile_pool(name="psB", bufs=2, space="PSUM") as psB,
              tc.tile_pool(name="dram", bufs=1, space="DRAM") as dram):

            # ---------------- constants -------------------------------
            # w1n first: the table build (critical path to the gathers)
            # needs only it plus the first atom piece
            w1n_sb = consts.tile([64, 128], BF16)
            nc.sync.dma_start(out=w1n_sb, in_=w1t_n)
            ident_sb = consts.tile([128, 128], BF16)
            nc.sync.dma_start(out=ident_sb, in_=ident)
            w1c_sb = consts.tile([64, 128], BF16)
            nc.sync.dma_start(out=w1c_sb, in_=w1t_c)
            w1fr_sb = consts.tile([128, 128], BF16)
            nc.sync.dma_start(out=w1fr_sb, in_=w1t_fr)
            w1a_sb = consts.tile([128, KA, 128], BF16)
            nc.sync.dma_start(
                out=w1a_sb,
                in_=bass.AP(tensor=w1t_a.tensor, offset=0,
                            ap=[[128, 128], [128 * 128, KA], [1, 128]]))
            w2t_sb = consts.tile([128, 64], BF16)
            nc.sync.dma_start(out=w2t_sb, in_=w2t)
            idx_sb = consts.tile([128, NCHUNK * (NIDX // 16)], I16)
            nc.sync.dma_start(out=idx_sb, in_=idx_w)
            g1_sb = consts.tile([128, 1], F32)
            nc.sync.dma_start(out=g1_sb, in_=g1)
            be1_sb = consts.tile([128, 1], F32)
            nc.sync.dma_start(out=be1_sb, in_=be1)
            g2_sb = consts.tile([64, 1], F32)
            nc.sync.dma_start(out=g2_sb, in_=g2)
            be2_sb = consts.tile([64, 1], F32)
            nc.sync.dma_start(out=be2_sb, in_=be2)
            eps_sb = consts.tile([128, 1], F32)
            nc.vector.memset(eps_sb, BN_EPS)

            s1 = consts.tile([128, NTILE], F32)
            q1 = consts.tile([128, NTILE], F32)
            stats2 = consts.tile([128, NPAIR, 6], F32)
            # z2 pair tiles are overlaid into z1 slots 4g / 4g+1, which are
            # dead once slab g's h1 is computed
            z1_sb = consts.tile([128, NTILE, 512], BF16)

            table_d = dram.tile([NFULL, 128], BF16)
            table_ap = bass.AP(tensor=table_d.tensor, offset=0,
                               ap=[[128, NFULL], [1, 128]])

            # ------- phase 0: bf16 projection table in DRAM -----------
            # table row a = atom_fea[a] @ W1n.T, built atom-major: one
            # matmul per 128-atom rank (atoms land on partitions), so the
            # store to DRAM is a plain contiguous-row DMA. Atom features
            # stream in 5 big pieces to amortize DMA latency.
            a_sb = None
            for grp in range(TGRP):
                if grp % 2 == 0:
                    a_sb = astr_pl.tile([64, 2048], BF16, tag="astr")
                    nc.sync.dma_start(
                        out=a_sb,
                        in_=atom_fullT[:, grp * 1024:(grp + 2) * 1024])
                base = (grp % 2) * 1024
                tb = tb_pl.tile([128, 8, 128], BF16, tag="tb")
                for half in range(2):
                    ps = psA.tile([128, 512], F32, tag="slot")
                    for k in range(4):
                        col = base + (half * 4 + k) * 128
                        nc.tensor.matmul(
                            ps[:, k * 128:(k + 1) * 128],
                            a_sb[:, col:col + 128],
                            w1n_sb[:], start=True, stop=True,
                            skip_group_check=True)
                    nc.vector.tensor_copy(
                        out=tb[:, half * 4:half * 4 + 4, :].rearrange(
                            "p a b -> p (a b)"),
                        in_=ps[:])
                nc.scalar.dma_start(
                    out=bass.AP(tensor=table_d.tensor,
                                offset=grp * 1024 * 128,
                                ap=[[128, 128], [128 * 128, 8], [1, 128]]),
                    in_=tb[:])

            # ---------------- phase 1: z1 assembly + stats -------------
            for c in range(NCHUNK):
                valid = CW if c < NCHUNK - 1 else TAIL
                # per-atom base: center + angle -> psB bank -> SBUF bf16
                at_sb = atom_pl.tile([64, CW], BF16, tag="atom")
                nc.sync.dma_start(out=at_sb,
                                  in_=atom_locT[:, c * CW:(c + 1) * CW])
                ab = psB.tile([128, 512], F32, tag="psB")
                nc.tensor.matmul(ab[:], w1c_sb[:], at_sb[:],
                                 start=True, stop=False)
                an_sb = angle_pl.tile([128, KA, CW], BF16, tag="angle")
                nc.sync.dma_start(
                    out=an_sb,
                    in_=bass.AP(tensor=angle_t.tensor, offset=c * CW,
                                ap=[[NPAD, 128], [128 * NPAD, KA], [1, CW]]))
                for k in range(KA):
                    nc.tensor.matmul(ab[:], w1a_sb[:, k, :], an_sb[:, k, :],
                                     start=False, stop=(k == KA - 1))
                ab_sb = absb_pl.tile([128, 512], BF16, tag="absb")
                nc.scalar.copy(out=ab_sb[:], in_=ab[:])
                # bond-major gathers, split across the 4 SWDGE queues so
                # 4 DMA rings pull table rows concurrently (3 slots each)
                gts = []
                for q in range(4):
                    gt = gath_pl.tile([128, NIDX // 512, 128], BF16,
                                      tag=f"gath{q}")
                    col = c * (NIDX // 16) + q * (NIDX // 64)
                    nc.gpsimd.dma_gather(
                        out_ap=gt[:], in_ap=table_ap,
                        idxs_ap=idx_sb[:, col:col + NIDX // 64],
                        num_idxs=NIDX // 4, num_idxs_reg=NIDX // 4,
                        elem_size=128, transpose=False, single_packet=False,
                        queue_num=q)
                    gts.append(gt)
                # all 12 stacked [nbr_j; nbr_{j+1}] tiles in one DMA
                # (row M is a host-side copy of row 0 for wraparound)
                nbA = nbr_pl.tile([128, M, CW], BF16, tag="nbr")
                nc.sync.dma_start(
                    out=nbA,
                    in_=bass.AP(tensor=nbr_t.tensor, offset=c * CW,
                                ap=[[NPAD, 128], [64 * NPAD, M], [1, CW]]))
                for j in range(M):
                    ps = psA.tile([128, 512], F32, tag="slot")
                    nc.tensor.matmul(ps[:], w1fr_sb[:], nbA[:, j, :],
                                     start=True, stop=False)
                    # transpose-inject gathered nbr_atom rows: G_block.T
                    for k in range(4):
                        nc.tensor.matmul(ps[:, k * 128:(k + 1) * 128],
                                         gts[j // 3][:, 4 * (j % 3) + k, :],
                                         ident_sb[:],
                                         start=False, stop=(k == 3))
                    t = c * M + j
                    z1t = z1_sb[:, t, :]
                    nc.vector.scalar_tensor_tensor(
                        out=z1t, in0=ps[:], scalar=1.0, in1=ab_sb[:],
                        op0=ALU.mult, op1=ALU.add,
                        accum_out=s1[:, t:t + 1])
                    sq = sq_pl.tile([128, 512], BF16, tag="sq")
                    nc.scalar.activation(out=sq[:], in_=z1t, func=AF.Square,
                                         accum_out=q1[:, t:t + 1])

            # ---------------- BN1 stats allreduce ----------------------
            # payload is plain [sum(x), sum(x^2)] per feature; pad bonds
            # gather the zero table row so full-width accums are exact
            pay1 = consts.tile([128, 2], F32)
            nc.vector.reduce_sum(out=pay1[:, 0:1], in_=s1[:],
                                 axis=mybir.AxisListType.X)
            nc.vector.reduce_sum(out=pay1[:, 1:2], in_=q1[:],
                                 axis=mybir.AxisListType.X)
            cc1i = dram.tile([128, 2], F32)
            cc1o = dram.tile([128, 2], F32)
            nc.sync.dma_start(out=cc1i[:], in_=pay1[:])
            nc.gpsimd.collective_compute(
                "AllReduce", ALU.add, replica_groups=[list(range(NCORES))],
                ins=[cc1i[:].opt()], outs=[cc1o[:].opt()])
            S1 = consts.tile([128, 2], F32)
            nc.sync.dma_start(out=S1[:], in_=cc1o[:])
            mean1 = consts.tile([128, 1], F32)
            nc.scalar.mul(out=mean1[:], in_=S1[:, 0:1], mul=1.0 / (N * M))
            mm1 = consts.tile([128, 1], F32)
            nc.scalar.square(out=mm1[:], in_=mean1[:])
            var1 = consts.tile([128, 1], F32)
            nc.vector.scalar_tensor_tensor(
                out=var1[:], in0=S1[:, 1:2], scalar=1.0 / (N * M), in1=mm1[:],
                op0=ALU.mult, op1=ALU.subtract)
            sd1 = consts.tile([128, 1], F32)
            nc.scalar.activation(out=sd1[:], in_=var1[:], func=AF.Sqrt,
                                 bias=eps_sb[:], scale=1.0)
            rs1 = consts.tile([128, 1], F32)
            nc.vector.reciprocal(out=rs1[:], in_=sd1[:])
            scale1 = consts.tile([128, 1], F32)
            nc.vector.tensor_mul(out=scale1[:], in0=rs1[:], in1=g1_sb[:])
            negm1 = consts.tile([128, 1], F32)
            nc.scalar.mul(out=negm1[:], in_=mean1[:], mul=-1.0)
            bias1 = consts.tile([128, 1], F32)
            nc.vector.scalar_tensor_tensor(
                out=bias1[:], in0=scale1[:], scalar=negm1[:], in1=be1_sb[:],
                op0=ALU.mult, op1=ALU.add)

            # ---------------- phase 2: h1, z2, stats2 ------------------
            # softplus per 4-tile slab; W2 as two half-partition matmuls
            # per PSUM bank so downstream tiles are full 128 partitions
            # (partitions 0:64 <- even tile features, 64:128 <- odd).
            for g in range(NSLAB):
                zsl = z1_sb[:, 4 * g:4 * g + 4, :].rearrange("p a b -> p (a b)")
                nc.scalar.activation(out=zsl, in_=zsl, func=AF.Exp,
                                     bias=bias1[:], scale=scale1[:])
                h1s = h1_pl.tile([128, 2048], BF16, tag="h1")
                nc.scalar.activation(out=h1s[:], in_=zsl, func=AF.Ln,
                                     bias=1.0)
                for m in range(2):
                    t = 2 * g + m
                    c = (4 * g + 2 * m) // M
                    valid = CW if c < NCHUNK - 1 else TAIL
                    ps = psB.tile([128, 512], F32, tag="psB")
                    nc.tensor.matmul(ps[0:64, :], w2t_sb[:],
                                     h1s[:, (2 * m) * 512:(2 * m + 1) * 512],
                                     start=True, stop=True,
                                     skip_group_check=True)
                    nc.tensor.matmul(ps[64:128, :], w2t_sb[:],
                                     h1s[:, (2 * m + 1) * 512:(2 * m + 2) * 512],
                                     start=True, stop=True,
                                     skip_group_check=True)
                    nc.vector.bn_stats(out=stats2[:, t, :],
                                       in_=ps[:, 0:valid])
                    nc.vector.tensor_copy(out=z1_sb[:, 4 * g + m, :],
                                          in_=ps[:])

            # ---------------- BN2 stats allreduce ----------------------
            mv2 = consts.tile([128, 2], F32)
            nc.vector.bn_aggr(out=mv2[:], in_=stats2[:])
            pay2 = consts.tile([128, 2], F32)
            msq2 = consts.tile([128, 1], F32)
            nc.scalar.square(out=msq2[:], in_=mv2[:, 0:1])
            nc.vector.tensor_copy(out=pay2[:, 0:1], in_=mv2[:, 0:1])
            nc.vector.tensor_add(out=pay2[:, 1:2], in0=mv2[:, 1:2], in1=msq2[:])
            cc2i = dram.tile([128, 2], F32)
            cc2o = dram.tile([128, 2], F32)
            nc.sync.dma_start(out=cc2i[:], in_=pay2[:])
            nc.gpsimd.collective_compute(
                "AllReduce", ALU.add, replica_groups=[list(range(NCORES))],
                ins=[cc2i[:].opt()], outs=[cc2o[:].opt()])
            # prefetch phase-3 bond weights during the collective (Pool
            # queue: idle here and dispatch is cheap)
            bwts = []
            for g in range(NSLAB):
                bwt = bwbc_pl.tile([128, 2, 512], BF16, tag="bwbc")
                for m in range(2):
                    t = 2 * g + m
                    c, j0 = divmod(2 * t, M)
                    nc.sync.dma_start(
                        out=bwt[0:64, m, :],
                        in_=bass.AP(tensor=bw.tensor,
                                    offset=j0 * NPAD + c * CW,
                                    ap=[[0, 64], [1, CW]]))
                    nc.sync.dma_start(
                        out=bwt[64:128, m, :],
                        in_=bass.AP(tensor=bw.tensor,
                                    offset=(j0 + 1) * NPAD + c * CW,
                                    ap=[[0, 64], [1, CW]]))
                bwts.append(bwt)
            S2 = consts.tile([128, 2], F32)
            nc.sync.dma_start(out=S2[:], in_=cc2o[:])
            # fold: partitions 64:128 hold the odd-tile half of each
            # feature's stats; shift down and add for the global sums
            S2s = consts.tile([64, 2], F32)
            nc.sync.dma_start(out=S2s[:], in_=S2[64:128, :])
            S2t = consts.tile([64, 2], F32)
            nc.vector.tensor_add(out=S2t[:], in0=S2[0:64, :], in1=S2s[:])
            mean2 = consts.tile([64, 1], F32)
            nc.scalar.mul(out=mean2[:], in_=S2t[:, 0:1], mul=1.0 / (2 * NCORES))
            mm2 = consts.tile([64, 1], F32)
            nc.scalar.square(out=mm2[:], in_=mean2[:])
            var2 = consts.tile([64, 1], F32)
            nc.vector.scalar_tensor_tensor(
                out=var2[:], in0=S2t[:, 1:2], scalar=1.0 / (2 * NCORES),
                in1=mm2[:], op0=ALU.mult, op1=ALU.subtract)
            sd2 = consts.tile([64, 1], F32)
            nc.scalar.activation(out=sd2[:], in_=var2[:], func=AF.Sqrt,
                                 bias=eps_sb[0:64, :], scale=1.0)
            rs2 = consts.tile([64, 1], F32)
            nc.vector.reciprocal(out=rs2[:], in_=sd2[:])
            scale2 = consts.tile([64, 1], F32)
            nc.vector.tensor_mul(out=scale2[:], in0=rs2[:], in1=g2_sb[:])
            negm2 = consts.tile([64, 1], F32)
            nc.scalar.mul(out=negm2[:], in_=mean2[:], mul=-1.0)
            bias2 = consts.tile([64, 1], F32)
            nc.vector.scalar_tensor_tensor(
                out=bias2[:], in0=scale2[:], scalar=negm2[:], in1=be2_sb[:],
                op0=ALU.mult, op1=ALU.add)
            scale2r = consts.tile([128, 1], F32)
            nc.sync.dma_start(out=scale2r[0:64, :], in_=scale2[:])
            nc.sync.dma_start(out=scale2r[64:128, :], in_=scale2[:])
            bias2r = consts.tile([128, 1], F32)
            nc.sync.dma_start(out=bias2r[0:64, :], in_=bias2[:])
            nc.sync.dma_start(out=bias2r[64:128, :], in_=bias2[:])

            # ---------------- phase 3: softplus2 * bw -> out -----------
            for g in range(NSLAB):
                zf = z1_sb[:, 4 * g:4 * g + 2, :].rearrange("p a b -> p (a b)")
                nc.scalar.activation(out=zf, in_=zf, func=AF.Exp,
                                     bias=bias2r[:], scale=scale2r[:])
                sp = sp_pl.tile([128, 1024], BF16, tag="sp")
                nc.scalar.activation(out=sp[:], in_=zf, func=AF.Ln, bias=1.0)
                nc.vector.tensor_mul(
                    out=sp[:], in0=sp[:],
                    in1=bwts[g][:].rearrange("p a b -> p (a b)"))
                nc.sync.dma_start(
                    out=out_p[:, g * 1024:(g + 1) * 1024], in_=sp[:])

    nc.compile()
    _CACHE["nc"] = nc
    return nc


def _prep_core(c, atom_fea, nbr_fea, nbr_fea_idx, angle_fea, bond_weights,
               shared):
    lo = c * NLOC
    hi = lo + NLOC
    atom_locT = np.zeros((64, NPAD), BF16_NP)
    atom_locT[:, :NLOC] = atom_fea[lo:hi].T.astype(BF16_NP)
    angle_t = np.zeros((KA * 128, NPAD), BF16_NP)
    angle_t[:A * ANG_F, :NLOC] = \
        angle_fea[lo:hi].reshape(NLOC, A * ANG_F).T.astype(BF16_NP)
    nbr_t = np.zeros(((M + 1) * 64, NPAD), BF16_NP)
    nbr_t[:M * 64, :NLOC] = \
        nbr_fea[lo:hi].transpose(1, 2, 0).reshape(M * 64, NLOC).astype(BF16_NP)
    nbr_t[M * 64:, :] = nbr_t[0:64, :]
    bw_p = np.zeros((M, NPAD), BF16_NP)
    bw_p[:, :NLOC] = bond_weights[lo:hi].T.astype(BF16_NP)

    idxp = np.full((NPAD, M), N, np.int16)   # pad bonds -> zero table row
    idxp[:NLOC] = nbr_fea_idx[lo:hi].astype(np.int16)
    idx_w = np.zeros((128, NCHUNK * (NIDX // 16)), np.int16)
    for cc in range(NCHUNK):
        flat = idxp[cc * CW:(cc + 1) * CW, :].T.reshape(-1)   # slot-major
        wr = flat.reshape(NIDX // 16, 16).T                   # (16, 384)
        col = cc * (NIDX // 16)
        idx_w[:, col:col + NIDX // 16] = np.tile(wr, (8, 1))

    d = dict(shared)
    d.update(atom_locT=atom_locT, angle_t=angle_t, nbr_t=nbr_t, idx_w=idx_w,
             bw=bw_p)
    return d


def _make_in_maps(inputs):
    """Build per-core input dicts from the full (unsharded) input dict."""
    atom_fea = np.asarray(inputs["atom_fea"], dtype=np.float32)
    nbr_fea = np.asarray(inputs["nbr_fea"], dtype=np.float32)
    nbr_fea_idx = np.asarray(inputs["nbr_fea_idx"])
    angle_fea = np.asarray(inputs["angle_fea"], dtype=np.float32)
    bond_weights = np.asarray(inputs["bond_weights"], dtype=np.float32)
    W1 = np.asarray(inputs["W1"]); W2 = np.asarray(inputs["W2"])
    g1 = np.asarray(inputs["g1"]); be1 = np.asarray(inputs["be1"])
    g2 = np.asarray(inputs["g2"]); be2 = np.asarray(inputs["be2"])

    atom_fullT = np.zeros((64, NFULL), BF16_NP)
    atom_fullT[:, :N] = atom_fea.T.astype(BF16_NP)
    w1t = W1.T.astype(np.float32)
    w1t_a = np.zeros((KA * 128, 128), BF16_NP)
    w1t_a[:A * ANG_F] = w1t[256:1312].astype(BF16_NP)
    shared = dict(
        atom_fullT=atom_fullT,
        w1t_c=np.ascontiguousarray(w1t[0:64]).astype(BF16_NP),
        w1t_n=np.ascontiguousarray(w1t[64:128]).astype(BF16_NP),
        w1t_fr=np.ascontiguousarray(w1t[128:256]).astype(BF16_NP),
        w1t_a=w1t_a,
        w2t=np.ascontiguousarray(W2.T).astype(BF16_NP),
        ident=np.eye(128, dtype=np.float32).astype(BF16_NP),
        g1=g1.reshape(128, 1).astype(np.float32),
        be1=be1.reshape(128, 1).astype(np.float32),
        g2=g2.reshape(64, 1).astype(np.float32),
        be2=be2.reshape(64, 1).astype(np.float32),
    )
    return [_prep_core(c, atom_fea, nbr_fea, nbr_fea_idx, angle_fea,
                       bond_weights, shared)
            for c in range(NCORES)]


def _assemble(results):
    """Per-core out_p buffers -> full (N, M, NBR_F) output."""
    out = np.empty((N, M, NBR_F), np.float32)
    for c in range(NCORES):
        op = np.asarray(results[c]["out_p"]).astype(np.float32)
        lo = c * NLOC
        for t in range(NPAIR):
            cc, j0 = divmod(2 * t, M)
            blk = op[:, t * 512:(t + 1) * 512]
            a0 = cc * CW
            nA = min(CW, NLOC - a0)
            out[lo + a0:lo + a0 + nA, j0, :] = blk[0:64, :nA].T
            out[lo + a0:lo + a0 + nA, j0 + 1, :] = blk[64:128, :nA].T
    return out


def kernel(atom_fea, nbr_fea, nbr_fea_idx, angle_fea, bond_weights,
           W1, b1, g1, be1, W2, b2, g2, be2):
    global LAST_EXEC_NS, LAST_RESULTS
    nc = _build()
    in_maps = _make_in_maps(dict(
        atom_fea=atom_fea, nbr_fea=nbr_fea, nbr_fea_idx=nbr_fea_idx,
        angle_fea=angle_fea, bond_weights=bond_weights, W1=W1, W2=W2,
        g1=g1, be1=be1, g2=g2, be2=be2))

    if TRACE:
        _install_ntff_hook()
    br = run_bass_kernel_spmd(nc, in_maps, list(range(NCORES)), trace=TRACE)
    LAST_EXEC_NS = br.exec_time_ns
    LAST_RESULTS = br
    return _assemble(br.results)


def _install_ntff_hook():
    """Inject antenv.axon_hooks (missing in this image) so trace=True works."""
    import types
    if "antenv.axon_hooks" in sys.modules:
        return
    sys.path.insert(0, "/root/.axon_site")
    mod = types.ModuleType("antenv.axon_hooks")
    mod._hook = None
    mod.set_axon_ntff_profile_hook = lambda h: setattr(mod, "_hook", h)
    mod.get_axon_ntff_profile_hook = lambda: mod._hook
    sys.modules["antenv.axon_hooks"] = mod
    try:
        from trn_agent_boot.trn_boot import _ntff_profile_via_ctypes
        h = _ntff_profile_via_ctypes("/opt/axon/libaxon_pjrt.so")
        if h is not None:
            mod.set_axon_ntff_profile_hook(h)
    except Exception as e:
        print("ntff hook install failed:", e)


# revision 23
# speedup vs baseline: 1.9432x; 1.0605x over previous
"""Trainium2 Bass kernel for nn_BondConvLayer (gnn_message_passing).

8-core data-parallel: 2500 atoms (30000 bonds) per core.

out = softplus(bn2(softplus(bn1(cat @ W1.T)) @ W2.T)) * bw  where
cat = [center, gathered_nbr_atom, nbr_fea, rolled_nbr_fea, angle] per bond;
b1/b2 cancel inside training-mode BatchNorm and are dropped.

v2 layout: everything bf16 on the wire and in the PE. Each core projects
the full atom table atom-major ([atom, 128h] rows, one matmul per
128-atom rank) and stores it to DRAM; per-bond rows are then pulled with
a DRAM-source non-transpose dma_gather (contiguous 256B descriptors -
fast path) arriving bond-major, and re-transposed into the feature-major
z1 PSUM accumulation with identity matmuls on the PE (stat=G block,
mov=I, start=False). nbr f/r projections run as one 128-deep stacked
matmul per slot ([W1f;W1r] weights, one 128-partition DMA spanning
adjacent nbr slots via a wraparound row). The per-atom center+angle base
is added on DVE during PSUM evacuation (scalar_tensor_tensor), which
also casts z1 to bf16 slabs kept in SBUF (no DRAM spill). BN batch
stats: bn_stats/bn_aggr per core + tiny AllReduce; phase 2 runs W2 as
two half-partition matmuls per PSUM bank (tile_position) so softplus /
stats / output work on full 128-partition tiles. Softplus = Exp then
Ln(x+1) on ACT with the BN affine fused in.
"""
import sys, os

sys.path.insert(0, "/opt/trn_rl_repo")

import numpy as np

import concourse.bass as bass
import concourse.bacc as bacc
import concourse.tile as tile
from concourse import mybir
from concourse.bass_utils import run_bass_kernel_spmd

F32 = mybir.dt.float32
BF16 = mybir.dt.bfloat16
I16 = mybir.dt.int16
AF = mybir.ActivationFunctionType
ALU = mybir.AluOpType
BF16_NP = mybir.dt.np(BF16)

NCORES = 8
N, M = 20000, 12
ATOM_F, NBR_F, ANG_F, A = 64, 64, 16, 66
H, O = 128, 64
BN_EPS = 1e-5
NLOC = N // NCORES          # 2500
NCHUNK = 5                  # chunks of 512 atoms (last ragged 452)
CW = 512
NPAD = NCHUNK * CW          # 2560
TAIL = NLOC - (NCHUNK - 1) * CW   # 452
NTILE = NCHUNK * M          # 60 bond tiles per core
NPAIR = NTILE // 2          # 30 paired tiles in phase 2/3
NSLAB = NTILE // 4          # 15 slabs of 4 bond-tiles
KA = 9                      # angle K-chunks of 128 (1056 padded to 1152)
NRANK = 160                 # table ranks (20000 atoms padded to 20480)
NFULL = NRANK * 128         # 20480
NIDX = M * CW               # 6144 gather indices per chunk (one call)
TGRP = 20                   # table build groups of 8 ranks

_CACHE = {}
TRACE = bool(int(os.environ.get("BASS_KERNEL_TRACE", "0")))
LAST_EXEC_NS = None
LAST_RESULTS = None


def _pin_act_tables():
    """Restrict the activation-table sets bacc may choose so Exp/Ln/Copy/
    Square all land in natural_log_exp_and_others (one load, no per-op
    table swaps). Set names/order (= act_func_set_id) are preserved."""
    if getattr(bacc, "_act_tables_pinned", False):
        return
    orig = bacc.get_activation_tables

    def pinned(arch):
        tabs = orig(arch)
        keep_all = "natural_log_exp_and_others"
        sqrt_home = "sqrt_and_others"
        strip = {AF.Exp, AF.Ln, AF.Copy, AF.Identity, AF.Square, AF.Sqrt}
        out = {}
        for name, funcs in tabs.items():
            if name == keep_all:
                out[name] = funcs
            elif name == sqrt_home:
                out[name] = {f for f in funcs
                             if f not in (strip - {AF.Sqrt})}
            else:
                out[name] = {f for f in funcs if f not in strip}
        return out

    bacc.get_activation_tables = pinned
    bacc._act_tables_pinned = True


def _build():
    if "nc" in _CACHE:
        return _CACHE["nc"]
    _pin_act_tables()
    import concourse.tile_utils as tile_utils
    tile_utils.max_sbuf_usage = 206 * 1024

    nc = bacc.Bacc("TRN2", target_bir_lowering=False, debug=False,
                   num_devices=NCORES, num_swdge_queues=4)

    atom_fullT = nc.dram_tensor("atom_fullT", [64, NFULL], BF16, kind="ExternalInput").ap()
    atom_locT = nc.dram_tensor("atom_locT", [64, NPAD], BF16, kind="ExternalInput").ap()
    angle_t = nc.dram_tensor("angle_t", [128, NCHUNK * KA * CW], BF16, kind="ExternalInput").ap()
    nbr_t = nc.dram_tensor("nbr_t", [128, NCHUNK * M * CW], BF16, kind="ExternalInput").ap()
    idx_w = nc.dram_tensor("idx_w", [128, NCHUNK * (NIDX // 16)], I16, kind="ExternalInput").ap()
    bw = nc.dram_tensor("bw", [128, NSLAB * 1024], BF16, kind="ExternalInput").ap()
    w1t_c = nc.dram_tensor("w1t_c", [64, 128], BF16, kind="ExternalInput").ap()
    w1t_n = nc.dram_tensor("w1t_n", [64, 128], BF16, kind="ExternalInput").ap()
    w1t_fr = nc.dram_tensor("w1t_fr", [128, 128], BF16, kind="ExternalInput").ap()
    w1t_a = nc.dram_tensor("w1t_a", [KA * 128, 128], BF16, kind="ExternalInput").ap()
    w2t = nc.dram_tensor("w2t", [128, 64], BF16, kind="ExternalInput").ap()
    ident = nc.dram_tensor("ident", [128, 128], BF16, kind="ExternalInput").ap()
    g1 = nc.dram_tensor("g1", [128, 1], F32, kind="ExternalInput").ap()
    be1 = nc.dram_tensor("be1", [128, 1], F32, kind="ExternalInput").ap()
    g2 = nc.dram_tensor("g2", [64, 1], F32, kind="ExternalInput").ap()
    be2 = nc.dram_tensor("be2", [64, 1], F32, kind="ExternalInput").ap()
    out_p = nc.dram_tensor("out_p", [128, NPAIR * 512], BF16, kind="ExternalOutput").ap()

    with tile.TileContext(nc) as tc:
        with (tc.tile_pool(name="consts", bufs=1) as consts,
              tc.tile_pool(name="astr", bufs=4) as astr_pl,
              tc.tile_pool(name="tb", bufs=6) as tb_pl,
              tc.tile_pool(name="atom", bufs=2) as atom_pl,
              tc.tile_pool(name="angle", bufs=2) as angle_pl,
              tc.tile_pool(name="nbr", bufs=2) as nbr_pl,
              tc.tile_pool(name="gath", bufs=2) as gath_pl,
              tc.tile_pool(name="absb", bufs=2) as absb_pl,
              tc.tile_pool(name="h1", bufs=2) as h1_pl,
              tc.tile_pool(name="sp", bufs=4) as sp_pl,
              tc.tile_pool(name="sq", bufs=2) as sq_pl,
              tc.tile_pool(name="bwbc", bufs=6) as bwbc_pl,
              tc.tile_pool(name="psA", bufs=4, space="PSUM") as psA,
              tc.tile_pool(name="psB", bufs=2, space="PSUM") as psB,
              tc.tile_pool(name="dram", bufs=1, space="DRAM") as dram):

            # ---------------- constants -------------------------------
            # w1n first: the table build (critical path to the gathers)
            # needs only it plus the first atom piece
            w1n_sb = consts.tile([64, 128], BF16)
            nc.sync.dma_start(out=w1n_sb, in_=w1t_n)
            ident_sb = consts.tile([128, 128], BF16)
            nc.sync.dma_start(out=ident_sb, in_=ident)
            w1c_sb = consts.tile([64, 128], BF16)
            nc.sync.dma_start(out=w1c_sb, in_=w1t_c)
            w1fr_sb = consts.tile([128, 128], BF16)
            nc.sync.dma_start(out=w1fr_sb, in_=w1t_fr)
            w1a_sb = consts.tile([128, KA, 128], BF16)
            nc.sync.dma_start(
                out=w1a_sb,
                in_=bass.AP(tensor=w1t_a.tensor, offset=0,
                            ap=[[128, 128], [128 * 128, KA], [1, 128]]))
            w2t_sb = consts.tile([128, 64], BF16)
            nc.sync.dma_start(out=w2t_sb, in_=w2t)
            idx_sb = consts.tile([128, NCHUNK * (NIDX // 16)], I16)
            nc.sync.dma_start(out=idx_sb, in_=idx_w)
            g1_sb = consts.tile([128, 1], F32)
            nc.sync.dma_start(out=g1_sb, in_=g1)
            be1_sb = consts.tile([128, 1], F32)
            nc.sync.dma_start(out=be1_sb, in_=be1)
            g2_sb = consts.tile([64, 1], F32)
            nc.sync.dma_start(out=g2_sb, in_=g2)
            be2_sb = consts.tile([64, 1], F32)
            nc.sync.dma_start(out=be2_sb, in_=be2)
            eps_sb = consts.tile([128, 1], F32)
            nc.vector.memset(eps_sb, BN_EPS)

            s1 = consts.tile([128, NTILE], F32)
            q1 = consts.tile([128, NTILE], F32)
            stats2 = consts.tile([128, NPAIR, 6], F32)
            # z2 pair tiles are overlaid into z1 slots 4g / 4g+1, which are
            # dead once slab g's h1 is computed
            z1_sb = consts.tile([128, NTILE, 512], BF16)

            table_d = dram.tile([NFULL, 128], BF16)
            table_ap = bass.AP(tensor=table_d.tensor, offset=0,
                               ap=[[128, NFULL], [1, 128]])

            # ------- phase 0: bf16 projection table in DRAM -----------
            # table row a = atom_fea[a] @ W1n.T, built atom-major: one
            # matmul per 128-atom rank (atoms land on partitions), so the
            # store to DRAM is a plain contiguous-row DMA. Atom features
            # stream in 5 big pieces to amortize DMA latency.
            a_sb = None
            for grp in range(TGRP):
                if grp % 2 == 0:
                    a_sb = astr_pl.tile([64, 2048], BF16, tag="astr")
                    nc.sync.dma_start(
                        out=a_sb,
                        in_=atom_fullT[:, grp * 1024:(grp + 2) * 1024])
                base = (grp % 2) * 1024
                tb = tb_pl.tile([128, 8, 128], BF16, tag="tb")
                for half in range(2):
                    ps = psA.tile([128, 512], F32, tag="slot")
                    for k in range(4):
                        col = base + (half * 4 + k) * 128
                        nc.tensor.matmul(
                            ps[:, k * 128:(k + 1) * 128],
                            a_sb[:, col:col + 128],
                            w1n_sb[:], start=True, stop=True,
                            skip_group_check=True)
                    nc.vector.tensor_copy(
                        out=tb[:, half * 4:half * 4 + 4, :].rearrange(
                            "p a b -> p (a b)"),
                        in_=ps[:])
                nc.scalar.dma_start(
                    out=bass.AP(tensor=table_d.tensor,
                                offset=grp * 8 * 128,
                                ap=[[NRANK * 128, 128], [1, 8 * 128]]),
                    in_=tb[:].rearrange("p a b -> p (a b)"))

            # ---------------- phase 1: z1 assembly + stats -------------
            for c in range(NCHUNK):
                valid = CW if c < NCHUNK - 1 else TAIL
                # per-atom base: center + angle -> psB bank -> SBUF bf16
                at_sb = atom_pl.tile([64, CW], BF16, tag="atom")
                nc.sync.dma_start(out=at_sb,
                                  in_=atom_locT[:, c * CW:(c + 1) * CW])
                ab = psB.tile([128, 512], F32, tag="psB")
                nc.tensor.matmul(ab[:], w1c_sb[:], at_sb[:],
                                 start=True, stop=False)
                an_sb = angle_pl.tile([128, KA, CW], BF16, tag="angle")
                nc.sync.dma_start(
                    out=an_sb[:].rearrange("p a b -> p (a b)"),
                    in_=angle_t[:, c * KA * CW:(c + 1) * KA * CW])
                for k in range(KA):
                    nc.tensor.matmul(ab[:], w1a_sb[:, k, :], an_sb[:, k, :],
                                     start=False, stop=(k == KA - 1))
                ab_sb = absb_pl.tile([128, 512], BF16, tag="absb")
                nc.scalar.copy(out=ab_sb[:], in_=ab[:])
                # bond-major gathers, split across the 4 SWDGE queues so
                # 4 DMA rings pull table rows concurrently (3 slots each)
                gts = []
                for q in range(4):
                    gt = gath_pl.tile([128, NIDX // 512, 128], BF16,
                                      tag=f"gath{q}")
                    col = c * (NIDX // 16) + q * (NIDX // 64)
                    nc.gpsimd.dma_gather(
                        out_ap=gt[:], in_ap=table_ap,
                        idxs_ap=idx_sb[:, col:col + NIDX // 64],
                        num_idxs=NIDX // 4, num_idxs_reg=NIDX // 4,
                        elem_size=128, transpose=False, single_packet=False,
                        queue_num=q)
                    gts.append(gt)
                # all 12 stacked [nbr_j; nbr_{j+1}] tiles in one DMA
                # (row M is a host-side copy of row 0 for wraparound)
                nbA = nbr_pl.tile([128, M, CW], BF16, tag="nbr")
                nc.sync.dma_start(
                    out=nbA[:].rearrange("p a b -> p (a b)"),
                    in_=nbr_t[:, c * M * CW:(c + 1) * M * CW])
                for j in range(M):
                    ps = psA.tile([128, 512], F32, tag="slot")
                    nc.tensor.matmul(ps[:], w1fr_sb[:], nbA[:, j, :],
                                     start=True, stop=False)
                    # transpose-inject gathered nbr_atom rows: G_block.T
                    for k in range(4):
                        nc.tensor.matmul(ps[:, k * 128:(k + 1) * 128],
                                         gts[j // 3][:, 4 * (j % 3) + k, :],
                                         ident_sb[:],
                                         start=False, stop=(k == 3))
                    t = c * M + j
                    z1t = z1_sb[:, t, :]
                    nc.vector.scalar_tensor_tensor(
                        out=z1t, in0=ps[:], scalar=1.0, in1=ab_sb[:],
                        op0=ALU.mult, op1=ALU.add,
                        accum_out=s1[:, t:t + 1])
                    sq = sq_pl.tile([128, 512], BF16, tag="sq")
                    nc.scalar.activation(out=sq[:], in_=z1t, func=AF.Square,
                                         accum_out=q1[:, t:t + 1])

            # ---------------- BN1 stats allreduce ----------------------
            # payload is plain [sum(x), sum(x^2)] per feature; pad bonds
            # gather the zero table row so full-width accums are exact
            pay1 = consts.tile([128, 2], F32)
            nc.vector.reduce_sum(out=pay1[:, 0:1], in_=s1[:],
                                 axis=mybir.AxisListType.X)
            nc.vector.reduce_sum(out=pay1[:, 1:2], in_=q1[:],
                                 axis=mybir.AxisListType.X)
            cc1i = dram.tile([128, 2], F32)
            cc1o = dram.tile([128, 2], F32)
            nc.sync.dma_start(out=cc1i[:], in_=pay1[:])
            nc.gpsimd.collective_compute(
                "AllReduce", ALU.add, replica_groups=[list(range(NCORES))],
                ins=[cc1i[:].opt()], outs=[cc1o[:].opt()])
            S1 = consts.tile([128, 2], F32)
            nc.sync.dma_start(out=S1[:], in_=cc1o[:])
            mean1 = consts.tile([128, 1], F32)
            nc.scalar.mul(out=mean1[:], in_=S1[:, 0:1], mul=1.0 / (N * M))
            mm1 = consts.tile([128, 1], F32)
            nc.scalar.square(out=mm1[:], in_=mean1[:])
            var1 = consts.tile([128, 1], F32)
            nc.vector.scalar_tensor_tensor(
                out=var1[:], in0=S1[:, 1:2], scalar=1.0 / (N * M), in1=mm1[:],
                op0=ALU.mult, op1=ALU.subtract)
            sd1 = consts.tile([128, 1], F32)
            nc.scalar.activation(out=sd1[:], in_=var1[:], func=AF.Sqrt,
                                 bias=eps_sb[:], scale=1.0)
            rs1 = consts.tile([128, 1], F32)
            nc.vector.reciprocal(out=rs1[:], in_=sd1[:])
            scale1 = consts.tile([128, 1], F32)
            nc.vector.tensor_mul(out=scale1[:], in0=rs1[:], in1=g1_sb[:])
            negm1 = consts.tile([128, 1], F32)
            nc.scalar.mul(out=negm1[:], in_=mean1[:], mul=-1.0)
            bias1 = consts.tile([128, 1], F32)
            nc.vector.scalar_tensor_tensor(
                out=bias1[:], in0=scale1[:], scalar=negm1[:], in1=be1_sb[:],
                op0=ALU.mult, op1=ALU.add)

            # ---------------- phase 2: h1, z2, stats2 ------------------
            # softplus per 4-tile slab; W2 as two half-partition matmuls
            # per PSUM bank so downstream tiles are full 128 partitions
            # (partitions 0:64 <- even tile features, 64:128 <- odd).
            for g in range(NSLAB):
                zsl = z1_sb[:, 4 * g:4 * g + 4, :].rearrange("p a b -> p (a b)")
                nc.scalar.activation(out=zsl, in_=zsl, func=AF.Exp,
                                     bias=bias1[:], scale=scale1[:])
                h1s = h1_pl.tile([128, 2048], BF16, tag="h1")
                nc.scalar.activation(out=h1s[:], in_=zsl, func=AF.Ln,
                                     bias=1.0)
                for m in range(2):
                    t = 2 * g + m
                    c = (4 * g + 2 * m) // M
                    valid = CW if c < NCHUNK - 1 else TAIL
                    ps = psB.tile([128, 512], F32, tag="psB")
                    nc.tensor.matmul(ps[0:64, :], w2t_sb[:],
                                     h1s[:, (2 * m) * 512:(2 * m + 1) * 512],
                                     start=True, stop=True,
                                     skip_group_check=True)
                    nc.tensor.matmul(ps[64:128, :], w2t_sb[:],
                                     h1s[:, (2 * m + 1) * 512:(2 * m + 2) * 512],
                                     start=True, stop=True,
                                     skip_group_check=True)
                    nc.vector.bn_stats(out=stats2[:, t, :],
                                       in_=ps[:, 0:valid])
                    nc.vector.tensor_copy(out=z1_sb[:, 4 * g + m, :],
                                          in_=ps[:])

            # ---------------- BN2 stats allreduce ----------------------
            mv2 = consts.tile([128, 2], F32)
            nc.vector.bn_aggr(out=mv2[:], in_=stats2[:])
            pay2 = consts.tile([128, 2], F32)
            msq2 = consts.tile([128, 1], F32)
            nc.scalar.square(out=msq2[:], in_=mv2[:, 0:1])
            nc.vector.tensor_copy(out=pay2[:, 0:1], in_=mv2[:, 0:1])
            nc.vector.tensor_add(out=pay2[:, 1:2], in0=mv2[:, 1:2], in1=msq2[:])
            cc2i = dram.tile([128, 2], F32)
            cc2o = dram.tile([128, 2], F32)
            nc.sync.dma_start(out=cc2i[:], in_=pay2[:])
            nc.gpsimd.collective_compute(
                "AllReduce", ALU.add, replica_groups=[list(range(NCORES))],
                ins=[cc2i[:].opt()], outs=[cc2o[:].opt()])
            # prefetch phase-3 bond weights during the collective (Pool
            # queue: idle here and dispatch is cheap)
            bwts = []
            for g in range(NSLAB):
                bwt = bwbc_pl.tile([128, 2, 512], BF16, tag="bwbc")
                nc.sync.dma_start(
                    out=bwt[:].rearrange("p a b -> p (a b)"),
                    in_=bw[:, g * 1024:(g + 1) * 1024])
                bwts.append(bwt)
            S2 = consts.tile([128, 2], F32)
            nc.sync.dma_start(out=S2[:], in_=cc2o[:])
            # fold: partitions 64:128 hold the odd-tile half of each
            # feature's stats; shift down and add for the global sums
            S2s = consts.tile([64, 2], F32)
            nc.sync.dma_start(out=S2s[:], in_=S2[64:128, :])
            S2t = consts.tile([64, 2], F32)
            nc.vector.tensor_add(out=S2t[:], in0=S2[0:64, :], in1=S2s[:])
            mean2 = consts.tile([64, 1], F32)
            nc.scalar.mul(out=mean2[:], in_=S2t[:, 0:1], mul=1.0 / (2 * NCORES))
            mm2 = consts.tile([64, 1], F32)
            nc.scalar.square(out=mm2[:], in_=mean2[:])
            var2 = consts.tile([64, 1], F32)
            nc.vector.scalar_tensor_tensor(
                out=var2[:], in0=S2t[:, 1:2], scalar=1.0 / (2 * NCORES),
                in1=mm2[:], op0=ALU.mult, op1=ALU.subtract)
            sd2 = consts.tile([64, 1], F32)
            nc.scalar.activation(out=sd2[:], in_=var2[:], func=AF.Sqrt,
                                 bias=eps_sb[0:64, :], scale=1.0)
            rs2 = consts.tile([64, 1], F32)
            nc.vector.reciprocal(out=rs2[:], in_=sd2[:])
            scale2 = consts.tile([64, 1], F32)
            nc.vector.tensor_mul(out=scale2[:], in0=rs2[:], in1=g2_sb[:])
            negm2 = consts.tile([64, 1], F32)
            nc.scalar.mul(out=negm2[:], in_=mean2[:], mul=-1.0)
            bias2 = consts.tile([64, 1], F32)
            nc.vector.scalar_tensor_tensor(
                out=bias2[:], in0=scale2[:], scalar=negm2[:], in1=be2_sb[:],
                op0=ALU.mult, op1=ALU.add)
            scale2r = consts.tile([128, 1], F32)
            nc.sync.dma_start(out=scale2r[0:64, :], in_=scale2[:])
            nc.sync.dma_start(out=scale2r[64:128, :], in_=scale2[:])
            bias2r = consts.tile([128, 1], F32)
            nc.sync.dma_start(out=bias2r[0:64, :], in_=bias2[:])
            nc.sync.dma_start(out=bias2r[64:128, :], in_=bias2[:])

            # ---------------- phase 3: softplus2 * bw -> out -----------
            for g in range(NSLAB):
                zf = z1_sb[:, 4 * g:4 * g + 2, :].rearrange("p a b -> p (a b)")
                nc.scalar.activation(out=zf, in_=zf, func=AF.Exp,
                                     bias=bias2r[:], scale=scale2r[:])
                sp = sp_pl.tile([128, 1024], BF16, tag="sp")
                nc.scalar.activation(out=sp[:], in_=zf, func=AF.Ln, bias=1.0)
                nc.vector.tensor_mul(
                    out=sp[:], in0=sp[:],
                    in1=bwts[g][:].rearrange("p a b -> p (a b)"))
                nc.sync.dma_start(
                    out=out_p[:, g * 1024:(g + 1) * 1024], in_=sp[:])

    nc.compile()
    _CACHE["nc"] = nc
    return nc


def _prep_core(c, atom_fea, nbr_fea, nbr_fea_idx, angle_fea, bond_weights,
               shared):
    lo = c * NLOC
    hi = lo + NLOC
    atom_locT = np.zeros((64, NPAD), BF16_NP)
    atom_locT[:, :NLOC] = atom_fea[lo:hi].T.astype(BF16_NP)

    # angle chunk-major: [128, NCHUNK, KA, 512], one contiguous DMA run
    # per partition per chunk
    ang = np.zeros((KA * 128, NPAD), np.float32)
    ang[:A * ANG_F, :NLOC] = angle_fea[lo:hi].reshape(NLOC, A * ANG_F).T
    angle_t = np.ascontiguousarray(
        ang.reshape(KA, 128, NCHUNK, CW).transpose(1, 2, 0, 3)
    ).reshape(128, NCHUNK * KA * CW).astype(BF16_NP)

    # nbr stacked [nbr_j; nbr_{j+1}] chunk-major: [128, NCHUNK, M, 512]
    nb = np.zeros((M, 64, NPAD), np.float32)
    nb[:, :, :NLOC] = nbr_fea[lo:hi].transpose(1, 2, 0)
    nb = nb.reshape(M, 64, NCHUNK, CW)
    top = nb.transpose(1, 2, 0, 3)                    # [64, NCHUNK, M, CW]
    bot = np.roll(nb, -1, axis=0).transpose(1, 2, 0, 3)
    nbr_t = np.ascontiguousarray(
        np.concatenate([top, bot], axis=0)
    ).reshape(128, NCHUNK * M * CW).astype(BF16_NP)

    # bond weights pre-broadcast in phase-3 pair layout [128, NSLAB, 1024]
    bwf = np.zeros((NPAD, M), np.float32)
    bwf[:NLOC] = bond_weights[lo:hi]
    bw_p = np.zeros((128, NSLAB, 2, CW), np.float32)
    for g in range(NSLAB):
        for m in range(2):
            cc, j0 = divmod(2 * (2 * g + m), M)
            bw_p[0:64, g, m, :] = bwf[cc * CW:(cc + 1) * CW, j0]
            bw_p[64:128, g, m, :] = bwf[cc * CW:(cc + 1) * CW, j0 + 1]
    bw_p = bw_p.reshape(128, NSLAB * 1024).astype(BF16_NP)

    idxp = np.full((NPAD, M), N, np.int32)   # pad bonds -> zero table row
    idxp[:NLOC] = nbr_fea_idx[lo:hi].astype(np.int32)
    # remap to the partition-major table layout: atom a lives at DRAM row
    # (a % 128) * NRANK + a // 128
    idxp = ((idxp % 128) * NRANK + idxp // 128).astype(np.int16)
    idx_w = np.zeros((128, NCHUNK * (NIDX // 16)), np.int16)
    for cc in range(NCHUNK):
        flat = idxp[cc * CW:(cc + 1) * CW, :].T.reshape(-1)   # slot-major
        wr = flat.reshape(NIDX // 16, 16).T                   # (16, 384)
        col = cc * (NIDX // 16)
        idx_w[:, col:col + NIDX // 16] = np.tile(wr, (8, 1))

    d = dict(shared)
    d.update(atom_locT=atom_locT, angle_t=angle_t, nbr_t=nbr_t, idx_w=idx_w,
             bw=bw_p)
    return d


def _make_in_maps(inputs):
    """Build per-core input dicts from the full (unsharded) input dict."""
    atom_fea = np.asarray(inputs["atom_fea"], dtype=np.float32)
    nbr_fea = np.asarray(inputs["nbr_fea"], dtype=np.float32)
    nbr_fea_idx = np.asarray(inputs["nbr_fea_idx"])
    angle_fea = np.asarray(inputs["angle_fea"], dtype=np.float32)
    bond_weights = np.asarray(inputs["bond_weights"], dtype=np.float32)
    W1 = np.asarray(inputs["W1"]); W2 = np.asarray(inputs["W2"])
    g1 = np.asarray(inputs["g1"]); be1 = np.asarray(inputs["be1"])
    g2 = np.asarray(inputs["g2"]); be2 = np.asarray(inputs["be2"])

    atom_fullT = np.zeros((64, NFULL), BF16_NP)
    atom_fullT[:, :N] = atom_fea.T.astype(BF16_NP)
    w1t = W1.T.astype(np.float32)
    w1t_a = np.zeros((KA * 128, 128), BF16_NP)
    w1t_a[:A * ANG_F] = w1t[256:1312].astype(BF16_NP)
    shared = dict(
        atom_fullT=atom_fullT,
        w1t_c=np.ascontiguousarray(w1t[0:64]).astype(BF16_NP),
        w1t_n=np.ascontiguousarray(w1t[64:128]).astype(BF16_NP),
        w1t_fr=np.ascontiguousarray(w1t[128:256]).astype(BF16_NP),
        w1t_a=w1t_a,
        w2t=np.ascontiguousarray(W2.T).astype(BF16_NP),
        ident=np.eye(128, dtype=np.float32).astype(BF16_NP),
        g1=g1.reshape(128, 1).astype(np.float32),
        be1=be1.reshape(128, 1).astype(np.float32),
        g2=g2.reshape(64, 1).astype(np.float32),
        be2=be2.reshape(64, 1).astype(np.float32),
    )
    return [_prep_core(c, atom_fea, nbr_fea, nbr_fea_idx, angle_fea,
                       bond_weights, shared)
            for c in range(NCORES)]


def _assemble(results):
    """Per-core out_p buffers -> full (N, M, NBR_F) output."""
    out = np.empty((N, M, NBR_F), np.float32)
    for c in range(NCORES):
        op = np.asarray(results[c]["out_p"]).astype(np.float32)
        lo = c * NLOC
        for t in range(NPAIR):
            cc, j0 = divmod(2 * t, M)
            blk = op[:, t * 512:(t + 1) * 512]
            a0 = cc * CW
            nA = min(CW, NLOC - a0)
            out[lo + a0:lo + a0 + nA, j0, :] = blk[0:64, :nA].T
            out[lo + a0:lo + a0 + nA, j0 + 1, :] = blk[64:128, :nA].T
    return out


def kernel(atom_fea, nbr_fea, nbr_fea_idx, angle_fea, bond_weights,
           W1, b1, g1, be1, W2, b2, g2, be2):
    global LAST_EXEC_NS, LAST_RESULTS
    nc = _build()
    in_maps = _make_in_maps(dict(
        atom_fea=atom_fea, nbr_fea=nbr_fea, nbr_fea_idx=nbr_fea_idx,
        angle_fea=angle_fea, bond_weights=bond_weights, W1=W1, W2=W2,
        g1=g1, be1=be1, g2=g2, be2=be2))

    if TRACE:
        _install_ntff_hook()
    br = run_bass_kernel_spmd(nc, in_maps, list(range(NCORES)), trace=TRACE)
    LAST_EXEC_NS = br.exec_time_ns
    LAST_RESULTS = br
    return _assemble(br.results)


def _install_ntff_hook():
    """Inject antenv.axon_hooks (missing in this image) so trace=True works."""
    import types
    if "antenv.axon_hooks" in sys.modules:
        return
    sys.path.insert(0, "/root/.axon_site")
    mod = types.ModuleType("antenv.axon_hooks")
    mod._hook = None
    mod.set_axon_ntff_profile_hook = lambda h: setattr(mod, "_hook", h)
    mod.get_axon_ntff_profile_hook = lambda: mod._hook
    sys.modules["antenv.axon_hooks"] = mod
    try:
        from trn_agent_boot.trn_boot import _ntff_profile_via_ctypes
        h = _ntff_profile_via_ctypes("/opt/axon/libaxon_pjrt.so")
        if h is not None:
            mod.set_axon_ntff_profile_hook(h)
    except Exception as e:
        print("ntff hook install failed:", e)


# revision 24
# speedup vs baseline: 1.9976x; 1.0280x over previous
"""Trainium2 Bass kernel for nn_BondConvLayer (gnn_message_passing).

8-core data-parallel: 2500 atoms (30000 bonds) per core.

out = softplus(bn2(softplus(bn1(cat @ W1.T)) @ W2.T)) * bw  where
cat = [center, gathered_nbr_atom, nbr_fea, rolled_nbr_fea, angle] per bond;
b1/b2 cancel inside training-mode BatchNorm and are dropped.

v2 layout: everything bf16 on the wire and in the PE. Each core projects
the full atom table atom-major ([atom, 128h] rows, one matmul per
128-atom rank) and stores it to DRAM; per-bond rows are then pulled with
a DRAM-source non-transpose dma_gather (contiguous 256B descriptors -
fast path) arriving bond-major, and re-transposed into the feature-major
z1 PSUM accumulation with identity matmuls on the PE (stat=G block,
mov=I, start=False). nbr f/r projections run as one 128-deep stacked
matmul per slot ([W1f;W1r] weights, one 128-partition DMA spanning
adjacent nbr slots via a wraparound row). The per-atom center+angle base
is added on DVE during PSUM evacuation (scalar_tensor_tensor), which
also casts z1 to bf16 slabs kept in SBUF (no DRAM spill). BN batch
stats: bn_stats/bn_aggr per core + tiny AllReduce; phase 2 runs W2 as
two half-partition matmuls per PSUM bank (tile_position) so softplus /
stats / output work on full 128-partition tiles. Softplus = Exp then
Ln(x+1) on ACT with the BN affine fused in.
"""
import sys, os

sys.path.insert(0, "/opt/trn_rl_repo")

import numpy as np

import concourse.bass as bass
import concourse.bacc as bacc
import concourse.tile as tile
from concourse import mybir
from concourse.bass_utils import run_bass_kernel_spmd

F32 = mybir.dt.float32
BF16 = mybir.dt.bfloat16
I16 = mybir.dt.int16
AF = mybir.ActivationFunctionType
ALU = mybir.AluOpType
BF16_NP = mybir.dt.np(BF16)

NCORES = 8
N, M = 20000, 12
ATOM_F, NBR_F, ANG_F, A = 64, 64, 16, 66
H, O = 128, 64
BN_EPS = 1e-5
NLOC = N // NCORES          # 2500
NCHUNK = 5                  # chunks of 512 atoms (last ragged 452)
CW = 512
NPAD = NCHUNK * CW          # 2560
TAIL = NLOC - (NCHUNK - 1) * CW   # 452
NTILE = NCHUNK * M          # 60 bond tiles per core
NPAIR = NTILE // 2          # 30 paired tiles in phase 2/3
NSLAB = NTILE // 4          # 15 slabs of 4 bond-tiles
KA = 9                      # angle K-chunks of 128 (1056 padded to 1152)
NRANK = 160                 # table ranks (20000 atoms padded to 20480)
NFULL = NRANK * 128         # 20480
NIDX = M * CW               # 6144 gather indices per chunk (one call)
TGRP = 20                   # table build groups of 8 ranks

_CACHE = {}
TRACE = bool(int(os.environ.get("BASS_KERNEL_TRACE", "0")))
LAST_EXEC_NS = None
LAST_RESULTS = None


def _pin_act_tables():
    """Restrict the activation-table sets bacc may choose so Exp/Ln/Copy/
    Square all land in natural_log_exp_and_others (one load, no per-op
    table swaps). Set names/order (= act_func_set_id) are preserved."""
    if getattr(bacc, "_act_tables_pinned", False):
        return
    orig = bacc.get_activation_tables

    def pinned(arch):
        tabs = orig(arch)
        keep_all = "natural_log_exp_and_others"
        sqrt_home = "sqrt_and_others"
        strip = {AF.Exp, AF.Ln, AF.Copy, AF.Identity, AF.Square, AF.Sqrt}
        out = {}
        for name, funcs in tabs.items():
            if name == keep_all:
                out[name] = funcs
            elif name == sqrt_home:
                out[name] = {f for f in funcs
                             if f not in (strip - {AF.Sqrt})}
            else:
                out[name] = {f for f in funcs if f not in strip}
        return out

    bacc.get_activation_tables = pinned
    bacc._act_tables_pinned = True


def _build():
    if "nc" in _CACHE:
        return _CACHE["nc"]
    _pin_act_tables()
    import concourse.tile_utils as tile_utils
    tile_utils.max_sbuf_usage = 206 * 1024

    nc = bacc.Bacc("TRN2", target_bir_lowering=False, debug=False,
                   num_devices=NCORES, num_swdge_queues=4)

    atom_fullT = nc.dram_tensor("atom_fullT", [64, NFULL], BF16, kind="ExternalInput").ap()
    atom_locT = nc.dram_tensor("atom_locT", [64, NPAD], BF16, kind="ExternalInput").ap()
    angle_t = nc.dram_tensor("angle_t", [128, NCHUNK * KA * CW], BF16, kind="ExternalInput").ap()
    nbr_t = nc.dram_tensor("nbr_t", [128, NCHUNK * M * CW], BF16, kind="ExternalInput").ap()
    idx_w = nc.dram_tensor("idx_w", [128, NCHUNK * (NIDX // 16)], I16, kind="ExternalInput").ap()
    bw = nc.dram_tensor("bw", [128, NSLAB * 1024], BF16, kind="ExternalInput").ap()
    w1t_c = nc.dram_tensor("w1t_c", [64, 128], BF16, kind="ExternalInput").ap()
    w1t_n = nc.dram_tensor("w1t_n", [64, 128], BF16, kind="ExternalInput").ap()
    w1t_fr = nc.dram_tensor("w1t_fr", [128, 128], BF16, kind="ExternalInput").ap()
    w1t_a = nc.dram_tensor("w1t_a", [KA * 128, 128], BF16, kind="ExternalInput").ap()
    w2t = nc.dram_tensor("w2t", [128, 64], BF16, kind="ExternalInput").ap()
    ident = nc.dram_tensor("ident", [128, 128], BF16, kind="ExternalInput").ap()
    g1 = nc.dram_tensor("g1", [128, 1], F32, kind="ExternalInput").ap()
    be1 = nc.dram_tensor("be1", [128, 1], F32, kind="ExternalInput").ap()
    g2 = nc.dram_tensor("g2", [64, 1], F32, kind="ExternalInput").ap()
    be2 = nc.dram_tensor("be2", [64, 1], F32, kind="ExternalInput").ap()
    out_p = nc.dram_tensor("out_p", [128, NPAIR * 512], BF16, kind="ExternalOutput").ap()

    with tile.TileContext(nc) as tc:
        with (tc.tile_pool(name="consts", bufs=1) as consts,
              tc.tile_pool(name="astr", bufs=4) as astr_pl,
              tc.tile_pool(name="tb", bufs=6) as tb_pl,
              tc.tile_pool(name="atom", bufs=2) as atom_pl,
              tc.tile_pool(name="angle", bufs=2) as angle_pl,
              tc.tile_pool(name="nbr", bufs=2) as nbr_pl,
              tc.tile_pool(name="gath", bufs=2) as gath_pl,
              tc.tile_pool(name="absb", bufs=NCHUNK) as absb_pl,
              tc.tile_pool(name="h1", bufs=2) as h1_pl,
              tc.tile_pool(name="sp", bufs=4) as sp_pl,
              tc.tile_pool(name="sq", bufs=2) as sq_pl,
              tc.tile_pool(name="bwbc", bufs=6) as bwbc_pl,
              tc.tile_pool(name="psA", bufs=5, space="PSUM") as psA,
              tc.tile_pool(name="psB", bufs=2, space="PSUM") as psB,
              tc.tile_pool(name="dram", bufs=1, space="DRAM") as dram):

            # ---------------- constants -------------------------------
            # w1n first: the table build (critical path to the gathers)
            # needs only it plus the first atom piece
            w1n_sb = consts.tile([64, 128], BF16)
            nc.sync.dma_start(out=w1n_sb, in_=w1t_n)
            ident_sb = consts.tile([128, 128], BF16)
            nc.sync.dma_start(out=ident_sb, in_=ident)
            w1c_sb = consts.tile([64, 128], BF16)
            nc.sync.dma_start(out=w1c_sb, in_=w1t_c)
            w1fr_sb = consts.tile([128, 128], BF16)
            nc.sync.dma_start(out=w1fr_sb, in_=w1t_fr)
            w1a_sb = consts.tile([128, KA, 128], BF16)
            nc.sync.dma_start(
                out=w1a_sb,
                in_=bass.AP(tensor=w1t_a.tensor, offset=0,
                            ap=[[128, 128], [128 * 128, KA], [1, 128]]))
            w2t_sb = consts.tile([128, 64], BF16)
            nc.sync.dma_start(out=w2t_sb, in_=w2t)
            idx_sb = consts.tile([128, NCHUNK * (NIDX // 16)], I16)
            nc.sync.dma_start(out=idx_sb, in_=idx_w)
            g1_sb = consts.tile([128, 1], F32)
            nc.sync.dma_start(out=g1_sb, in_=g1)
            be1_sb = consts.tile([128, 1], F32)
            nc.sync.dma_start(out=be1_sb, in_=be1)
            g2_sb = consts.tile([64, 1], F32)
            nc.sync.dma_start(out=g2_sb, in_=g2)
            be2_sb = consts.tile([64, 1], F32)
            nc.sync.dma_start(out=be2_sb, in_=be2)
            eps_sb = consts.tile([128, 1], F32)
            nc.vector.memset(eps_sb, BN_EPS)

            s1 = consts.tile([128, NTILE], F32)
            q1 = consts.tile([128, NTILE], F32)
            stats2 = consts.tile([128, NPAIR, 6], F32)
            # z2 pair tiles are overlaid into z1 slots 4g / 4g+1, which are
            # dead once slab g's h1 is computed
            z1_sb = consts.tile([128, NTILE, 512], BF16)

            table_d = dram.tile([NFULL, 128], BF16)
            table_ap = bass.AP(tensor=table_d.tensor, offset=0,
                               ap=[[128, NFULL], [1, 128]])

            # ------- phase 0: bf16 projection table in DRAM -----------
            # table row a = atom_fea[a] @ W1n.T, built atom-major: one
            # matmul per 128-atom rank (atoms land on partitions), so the
            # store to DRAM is a plain contiguous-row DMA. Atom features
            # stream in 5 big pieces to amortize DMA latency.
            a_sb = None
            for grp in range(TGRP):
                if grp % 2 == 0:
                    a_sb = astr_pl.tile([64, 2048], BF16, tag="astr")
                    nc.sync.dma_start(
                        out=a_sb,
                        in_=atom_fullT[:, grp * 1024:(grp + 2) * 1024])
                base = (grp % 2) * 1024
                tb = tb_pl.tile([128, 8, 128], BF16, tag="tb")
                for half in range(2):
                    ps = psA.tile([128, 512], F32, tag="slot")
                    for k in range(4):
                        col = base + (half * 4 + k) * 128
                        nc.tensor.matmul(
                            ps[:, k * 128:(k + 1) * 128],
                            a_sb[:, col:col + 128],
                            w1n_sb[:], start=True, stop=True,
                            skip_group_check=True)
                    nc.vector.tensor_copy(
                        out=tb[:, half * 4:half * 4 + 4, :].rearrange(
                            "p a b -> p (a b)"),
                        in_=ps[:])
                nc.scalar.dma_start(
                    out=bass.AP(tensor=table_d.tensor,
                                offset=grp * 8 * 128,
                                ap=[[NRANK * 128, 128], [1, 8 * 128]]),
                    in_=tb[:].rearrange("p a b -> p (a b)"))

            # ---------------- phase 1: z1 assembly + stats -------------
            # per-atom bases (center + angle) for ALL chunks first: this
            # is the only gather-independent PE work, and emitting it ahead
            # fills the PE idle window while the table DMAs land and the
            # first gather transfers.
            ab_sbs = []
            for c in range(NCHUNK):
                at_sb = atom_pl.tile([64, CW], BF16, tag="atom")
                nc.sync.dma_start(out=at_sb,
                                  in_=atom_locT[:, c * CW:(c + 1) * CW])
                ab = psB.tile([128, 512], F32, tag="psB")
                nc.tensor.matmul(ab[:], w1c_sb[:], at_sb[:],
                                 start=True, stop=False)
                an_sb = angle_pl.tile([128, KA, CW], BF16, tag="angle")
                nc.sync.dma_start(
                    out=an_sb[:].rearrange("p a b -> p (a b)"),
                    in_=angle_t[:, c * KA * CW:(c + 1) * KA * CW])
                for k in range(KA):
                    nc.tensor.matmul(ab[:], w1a_sb[:, k, :], an_sb[:, k, :],
                                     start=False, stop=(k == KA - 1))
                ab_sb = absb_pl.tile([128, 512], BF16, tag="absb")
                nc.scalar.copy(out=ab_sb[:], in_=ab[:])
                ab_sbs.append(ab_sb)
            for c in range(NCHUNK):
                valid = CW if c < NCHUNK - 1 else TAIL
                ab_sb = ab_sbs[c]
                # bond-major gathers, split across the 4 SWDGE queues so
                # 4 DMA rings pull table rows concurrently (3 slots each)
                gts = []
                for q in range(4):
                    gt = gath_pl.tile([128, NIDX // 512, 128], BF16,
                                      tag=f"gath{q}")
                    col = c * (NIDX // 16) + q * (NIDX // 64)
                    nc.gpsimd.dma_gather(
                        out_ap=gt[:], in_ap=table_ap,
                        idxs_ap=idx_sb[:, col:col + NIDX // 64],
                        num_idxs=NIDX // 4, num_idxs_reg=NIDX // 4,
                        elem_size=128, transpose=False, single_packet=False,
                        queue_num=q)
                    gts.append(gt)
                # all 12 stacked [nbr_j; nbr_{j+1}] tiles in one DMA
                # (row M is a host-side copy of row 0 for wraparound)
                nbA = nbr_pl.tile([128, M, CW], BF16, tag="nbr")
                nc.sync.dma_start(
                    out=nbA[:].rearrange("p a b -> p (a b)"),
                    in_=nbr_t[:, c * M * CW:(c + 1) * M * CW])
                for j in range(M):
                    ps = psA.tile([128, 512], F32, tag="slot")
                    nc.tensor.matmul(ps[:], w1fr_sb[:], nbA[:, j, :],
                                     start=True, stop=False)
                    # transpose-inject gathered nbr_atom rows: G_block.T
                    for k in range(4):
                        nc.tensor.matmul(ps[:, k * 128:(k + 1) * 128],
                                         gts[j // 3][:, 4 * (j % 3) + k, :],
                                         ident_sb[:],
                                         start=False, stop=(k == 3))
                    t = c * M + j
                    z1t = z1_sb[:, t, :]
                    nc.vector.scalar_tensor_tensor(
                        out=z1t, in0=ps[:], scalar=1.0, in1=ab_sb[:],
                        op0=ALU.mult, op1=ALU.add,
                        accum_out=s1[:, t:t + 1])
                    sq = sq_pl.tile([128, 512], BF16, tag="sq")
                    nc.scalar.activation(out=sq[:], in_=z1t, func=AF.Square,
                                         accum_out=q1[:, t:t + 1])

            # ---------------- BN1 stats allreduce ----------------------
            # payload is plain [sum(x), sum(x^2)] per feature; pad bonds
            # gather the zero table row so full-width accums are exact
            pay1 = consts.tile([128, 2], F32)
            nc.vector.reduce_sum(out=pay1[:, 0:1], in_=s1[:],
                                 axis=mybir.AxisListType.X)
            nc.vector.reduce_sum(out=pay1[:, 1:2], in_=q1[:],
                                 axis=mybir.AxisListType.X)
            cc1i = dram.tile([128, 2], F32)
            cc1o = dram.tile([128, 2], F32)
            nc.sync.dma_start(out=cc1i[:], in_=pay1[:])
            nc.gpsimd.collective_compute(
                "AllReduce", ALU.add, replica_groups=[list(range(NCORES))],
                ins=[cc1i[:].opt()], outs=[cc1o[:].opt()])
            S1 = consts.tile([128, 2], F32)
            nc.sync.dma_start(out=S1[:], in_=cc1o[:])
            mean1 = consts.tile([128, 1], F32)
            nc.scalar.mul(out=mean1[:], in_=S1[:, 0:1], mul=1.0 / (N * M))
            mm1 = consts.tile([128, 1], F32)
            nc.scalar.square(out=mm1[:], in_=mean1[:])
            var1 = consts.tile([128, 1], F32)
            nc.vector.scalar_tensor_tensor(
                out=var1[:], in0=S1[:, 1:2], scalar=1.0 / (N * M), in1=mm1[:],
                op0=ALU.mult, op1=ALU.subtract)
            sd1 = consts.tile([128, 1], F32)
            nc.scalar.activation(out=sd1[:], in_=var1[:], func=AF.Sqrt,
                                 bias=eps_sb[:], scale=1.0)
            rs1 = consts.tile([128, 1], F32)
            nc.vector.reciprocal(out=rs1[:], in_=sd1[:])
            scale1 = consts.tile([128, 1], F32)
            nc.vector.tensor_mul(out=scale1[:], in0=rs1[:], in1=g1_sb[:])
            negm1 = consts.tile([128, 1], F32)
            nc.scalar.mul(out=negm1[:], in_=mean1[:], mul=-1.0)
            bias1 = consts.tile([128, 1], F32)
            nc.vector.scalar_tensor_tensor(
                out=bias1[:], in0=scale1[:], scalar=negm1[:], in1=be1_sb[:],
                op0=ALU.mult, op1=ALU.add)

            # ---------------- phase 2: h1, z2, stats2 ------------------
            # softplus per 4-tile slab; W2 as two half-partition matmuls
            # per PSUM bank so downstream tiles are full 128 partitions
            # (partitions 0:64 <- even tile features, 64:128 <- odd).
            for g in range(NSLAB):
                zsl = z1_sb[:, 4 * g:4 * g + 4, :].rearrange("p a b -> p (a b)")
                nc.scalar.activation(out=zsl, in_=zsl, func=AF.Exp,
                                     bias=bias1[:], scale=scale1[:])
                h1s = h1_pl.tile([128, 2048], BF16, tag="h1")
                nc.scalar.activation(out=h1s[:], in_=zsl, func=AF.Ln,
                                     bias=1.0)
                for m in range(2):
                    t = 2 * g + m
                    c = (4 * g + 2 * m) // M
                    valid = CW if c < NCHUNK - 1 else TAIL
                    ps = psB.tile([128, 512], F32, tag="psB")
                    nc.tensor.matmul(ps[0:64, :], w2t_sb[:],
                                     h1s[:, (2 * m) * 512:(2 * m + 1) * 512],
                                     start=True, stop=True,
                                     skip_group_check=True)
                    nc.tensor.matmul(ps[64:128, :], w2t_sb[:],
                                     h1s[:, (2 * m + 1) * 512:(2 * m + 2) * 512],
                                     start=True, stop=True,
                                     skip_group_check=True)
                    nc.vector.bn_stats(out=stats2[:, t, :],
                                       in_=ps[:, 0:valid])
                    nc.vector.tensor_copy(out=z1_sb[:, 4 * g + m, :],
                                          in_=ps[:])

            # ---------------- BN2 stats allreduce ----------------------
            mv2 = consts.tile([128, 2], F32)
            nc.vector.bn_aggr(out=mv2[:], in_=stats2[:])
            pay2 = consts.tile([128, 2], F32)
            msq2 = consts.tile([128, 1], F32)
            nc.scalar.square(out=msq2[:], in_=mv2[:, 0:1])
            nc.vector.tensor_copy(out=pay2[:, 0:1], in_=mv2[:, 0:1])
            nc.vector.tensor_add(out=pay2[:, 1:2], in0=mv2[:, 1:2], in1=msq2[:])
            cc2i = dram.tile([128, 2], F32)
            cc2o = dram.tile([128, 2], F32)
            nc.sync.dma_start(out=cc2i[:], in_=pay2[:])
            nc.gpsimd.collective_compute(
                "AllReduce", ALU.add, replica_groups=[list(range(NCORES))],
                ins=[cc2i[:].opt()], outs=[cc2o[:].opt()])
            # prefetch phase-3 bond weights during the collective (Pool
            # queue: idle here and dispatch is cheap)
            bwts = []
            for g in range(NSLAB):
                bwt = bwbc_pl.tile([128, 2, 512], BF16, tag="bwbc")
                nc.sync.dma_start(
                    out=bwt[:].rearrange("p a b -> p (a b)"),
                    in_=bw[:, g * 1024:(g + 1) * 1024])
                bwts.append(bwt)
            S2 = consts.tile([128, 2], F32)
            nc.sync.dma_start(out=S2[:], in_=cc2o[:])
            # fold: partitions 64:128 hold the odd-tile half of each
            # feature's stats; shift down and add for the global sums
            S2s = consts.tile([64, 2], F32)
            nc.sync.dma_start(out=S2s[:], in_=S2[64:128, :])
            S2t = consts.tile([64, 2], F32)
            nc.vector.tensor_add(out=S2t[:], in0=S2[0:64, :], in1=S2s[:])
            mean2 = consts.tile([64, 1], F32)
            nc.scalar.mul(out=mean2[:], in_=S2t[:, 0:1], mul=1.0 / (2 * NCORES))
            mm2 = consts.tile([64, 1], F32)
            nc.scalar.square(out=mm2[:], in_=mean2[:])
            var2 = consts.tile([64, 1], F32)
            nc.vector.scalar_tensor_tensor(
                out=var2[:], in0=S2t[:, 1:2], scalar=1.0 / (2 * NCORES),
                in1=mm2[:], op0=ALU.mult, op1=ALU.subtract)
            sd2 = consts.tile([64, 1], F32)
            nc.scalar.activation(out=sd2[:], in_=var2[:], func=AF.Sqrt,
                                 bias=eps_sb[0:64, :], scale=1.0)
            rs2 = consts.tile([64, 1], F32)
            nc.vector.reciprocal(out=rs2[:], in_=sd2[:])
            scale2 = consts.tile([64, 1], F32)
            nc.vector.tensor_mul(out=scale2[:], in0=rs2[:], in1=g2_sb[:])
            negm2 = consts.tile([64, 1], F32)
            nc.scalar.mul(out=negm2[:], in_=mean2[:], mul=-1.0)
            bias2 = consts.tile([64, 1], F32)
            nc.vector.scalar_tensor_tensor(
                out=bias2[:], in0=scale2[:], scalar=negm2[:], in1=be2_sb[:],
                op0=ALU.mult, op1=ALU.add)
            scale2r = consts.tile([128, 1], F32)
            nc.sync.dma_start(out=scale2r[0:64, :], in_=scale2[:])
            nc.sync.dma_start(out=scale2r[64:128, :], in_=scale2[:])
            bias2r = consts.tile([128, 1], F32)
            nc.sync.dma_start(out=bias2r[0:64, :], in_=bias2[:])
            nc.sync.dma_start(out=bias2r[64:128, :], in_=bias2[:])

            # ---------------- phase 3: softplus2 * bw -> out -----------
            for g in range(NSLAB):
                zf = z1_sb[:, 4 * g:4 * g + 2, :].rearrange("p a b -> p (a b)")
                nc.scalar.activation(out=zf, in_=zf, func=AF.Exp,
                                     bias=bias2r[:], scale=scale2r[:])
                sp = sp_pl.tile([128, 1024], BF16, tag="sp")
                nc.scalar.activation(out=sp[:], in_=zf, func=AF.Ln, bias=1.0)
                nc.vector.tensor_mul(
                    out=sp[:], in0=sp[:],
                    in1=bwts[g][:].rearrange("p a b -> p (a b)"))
                nc.sync.dma_start(
                    out=out_p[:, g * 1024:(g + 1) * 1024], in_=sp[:])

    nc.compile()
    _CACHE["nc"] = nc
    return nc


def _prep_core(c, atom_fea, nbr_fea, nbr_fea_idx, angle_fea, bond_weights,
               shared):
    lo = c * NLOC
    hi = lo + NLOC
    atom_locT = np.zeros((64, NPAD), BF16_NP)
    atom_locT[:, :NLOC] = atom_fea[lo:hi].T.astype(BF16_NP)

    # angle chunk-major: [128, NCHUNK, KA, 512], one contiguous DMA run
    # per partition per chunk
    ang = np.zeros((KA * 128, NPAD), np.float32)
    ang[:A * ANG_F, :NLOC] = angle_fea[lo:hi].reshape(NLOC, A * ANG_F).T
    angle_t = np.ascontiguousarray(
        ang.reshape(KA, 128, NCHUNK, CW).transpose(1, 2, 0, 3)
    ).reshape(128, NCHUNK * KA * CW).astype(BF16_NP)

    # nbr stacked [nbr_j; nbr_{j+1}] chunk-major: [128, NCHUNK, M, 512]
    nb = np.zeros((M, 64, NPAD), np.float32)
    nb[:, :, :NLOC] = nbr_fea[lo:hi].transpose(1, 2, 0)
    nb = nb.reshape(M, 64, NCHUNK, CW)
    top = nb.transpose(1, 2, 0, 3)                    # [64, NCHUNK, M, CW]
    bot = np.roll(nb, -1, axis=0).transpose(1, 2, 0, 3)
    nbr_t = np.ascontiguousarray(
        np.concatenate([top, bot], axis=0)
    ).reshape(128, NCHUNK * M * CW).astype(BF16_NP)

    # bond weights pre-broadcast in phase-3 pair layout [128, NSLAB, 1024]
    bwf = np.zeros((NPAD, M), np.float32)
    bwf[:NLOC] = bond_weights[lo:hi]
    bw_p = np.zeros((128, NSLAB, 2, CW), np.float32)
    for g in range(NSLAB):
        for m in range(2):
            cc, j0 = divmod(2 * (2 * g + m), M)
            bw_p[0:64, g, m, :] = bwf[cc * CW:(cc + 1) * CW, j0]
            bw_p[64:128, g, m, :] = bwf[cc * CW:(cc + 1) * CW, j0 + 1]
    bw_p = bw_p.reshape(128, NSLAB * 1024).astype(BF16_NP)

    idxp = np.full((NPAD, M), N, np.int32)   # pad bonds -> zero table row
    idxp[:NLOC] = nbr_fea_idx[lo:hi].astype(np.int32)
    # remap to the partition-major table layout: atom a lives at DRAM row
    # (a % 128) * NRANK + a // 128
    idxp = ((idxp % 128) * NRANK + idxp // 128).astype(np.int16)
    idx_w = np.zeros((128, NCHUNK * (NIDX // 16)), np.int16)
    for cc in range(NCHUNK):
        flat = idxp[cc * CW:(cc + 1) * CW, :].T.reshape(-1)   # slot-major
        wr = flat.reshape(NIDX // 16, 16).T                   # (16, 384)
        col = cc * (NIDX // 16)
        idx_w[:, col:col + NIDX // 16] = np.tile(wr, (8, 1))

    d = dict(shared)
    d.update(atom_locT=atom_locT, angle_t=angle_t, nbr_t=nbr_t, idx_w=idx_w,
             bw=bw_p)
    return d


def _make_in_maps(inputs):
    """Build per-core input dicts from the full (unsharded) input dict."""
    atom_fea = np.asarray(inputs["atom_fea"], dtype=np.float32)
    nbr_fea = np.asarray(inputs["nbr_fea"], dtype=np.float32)
    nbr_fea_idx = np.asarray(inputs["nbr_fea_idx"])
    angle_fea = np.asarray(inputs["angle_fea"], dtype=np.float32)
    bond_weights = np.asarray(inputs["bond_weights"], dtype=np.float32)
    W1 = np.asarray(inputs["W1"]); W2 = np.asarray(inputs["W2"])
    g1 = np.asarray(inputs["g1"]); be1 = np.asarray(inputs["be1"])
    g2 = np.asarray(inputs["g2"]); be2 = np.asarray(inputs["be2"])

    atom_fullT = np.zeros((64, NFULL), BF16_NP)
    atom_fullT[:, :N] = atom_fea.T.astype(BF16_NP)
    w1t = W1.T.astype(np.float32)
    w1t_a = np.zeros((KA * 128, 128), BF16_NP)
    w1t_a[:A * ANG_F] = w1t[256:1312].astype(BF16_NP)
    shared = dict(
        atom_fullT=atom_fullT,
        w1t_c=np.ascontiguousarray(w1t[0:64]).astype(BF16_NP),
        w1t_n=np.ascontiguousarray(w1t[64:128]).astype(BF16_NP),
        w1t_fr=np.ascontiguousarray(w1t[128:256]).astype(BF16_NP),
        w1t_a=w1t_a,
        w2t=np.ascontiguousarray(W2.T).astype(BF16_NP),
        ident=np.eye(128, dtype=np.float32).astype(BF16_NP),
        g1=g1.reshape(128, 1).astype(np.float32),
        be1=be1.reshape(128, 1).astype(np.float32),
        g2=g2.reshape(64, 1).astype(np.float32),
        be2=be2.reshape(64, 1).astype(np.float32),
    )
    return [_prep_core(c, atom_fea, nbr_fea, nbr_fea_idx, angle_fea,
                       bond_weights, shared)
            for c in range(NCORES)]


def _assemble(results):
    """Per-core out_p buffers -> full (N, M, NBR_F) output."""
    out = np.empty((N, M, NBR_F), np.float32)
    for c in range(NCORES):
        op = np.asarray(results[c]["out_p"]).astype(np.float32)
        lo = c * NLOC
        for t in range(NPAIR):
            cc, j0 = divmod(2 * t, M)
            blk = op[:, t * 512:(t + 1) * 512]
            a0 = cc * CW
            nA = min(CW, NLOC - a0)
            out[lo + a0:lo + a0 + nA, j0, :] = blk[0:64, :nA].T
            out[lo + a0:lo + a0 + nA, j0 + 1, :] = blk[64:128, :nA].T
    return out


def kernel(atom_fea, nbr_fea, nbr_fea_idx, angle_fea, bond_weights,
           W1, b1, g1, be1, W2, b2, g2, be2):
    global LAST_EXEC_NS, LAST_RESULTS
    nc = _build()
    in_maps = _make_in_maps(dict(
        atom_fea=atom_fea, nbr_fea=nbr_fea, nbr_fea_idx=nbr_fea_idx,
        angle_fea=angle_fea, bond_weights=bond_weights, W1=W1, W2=W2,
        g1=g1, be1=be1, g2=g2, be2=be2))

    if TRACE:
        _install_ntff_hook()
    br = run_bass_kernel_spmd(nc, in_maps, list(range(NCORES)), trace=TRACE)
    LAST_EXEC_NS = br.exec_time_ns
    LAST_RESULTS = br
    return _assemble(br.results)


def _install_ntff_hook():
    """Inject antenv.axon_hooks (missing in this image) so trace=True works."""
    import types
    if "antenv.axon_hooks" in sys.modules:
        return
    sys.path.insert(0, "/root/.axon_site")
    mod = types.ModuleType("antenv.axon_hooks")
    mod._hook = None
    mod.set_axon_ntff_profile_hook = lambda h: setattr(mod, "_hook", h)
    mod.get_axon_ntff_profile_hook = lambda: mod._hook
    sys.modules["antenv.axon_hooks"] = mod
    try:
        from trn_agent_boot.trn_boot import _ntff_profile_via_ctypes
        h = _ntff_profile_via_ctypes("/opt/axon/libaxon_pjrt.so")
        if h is not None:
            mod.set_axon_ntff_profile_hook(h)
    except Exception as e:
        print("ntff hook install failed:", e)


# revision 25
# speedup vs baseline: 2.0735x; 1.0380x over previous
"""Trainium2 Bass kernel for nn_BondConvLayer (gnn_message_passing).

8-core data-parallel: 2500 atoms (30000 bonds) per core.

out = softplus(bn2(softplus(bn1(cat @ W1.T)) @ W2.T)) * bw  where
cat = [center, gathered_nbr_atom, nbr_fea, rolled_nbr_fea, angle] per bond;
b1/b2 cancel inside training-mode BatchNorm and are dropped.

v2 layout: everything bf16 on the wire and in the PE. Each core projects
the full atom table atom-major ([atom, 128h] rows, one matmul per
128-atom rank) and stores it to DRAM; per-bond rows are then pulled with
a DRAM-source non-transpose dma_gather (contiguous 256B descriptors -
fast path) arriving bond-major, and re-transposed into the feature-major
z1 PSUM accumulation with identity matmuls on the PE (stat=G block,
mov=I, start=False). nbr f/r projections run as one 128-deep stacked
matmul per slot ([W1f;W1r] weights, one 128-partition DMA spanning
adjacent nbr slots via a wraparound row). The per-atom center+angle base
is added on DVE during PSUM evacuation (scalar_tensor_tensor), which
also casts z1 to bf16 slabs kept in SBUF (no DRAM spill). BN batch
stats: bn_stats/bn_aggr per core + tiny AllReduce; phase 2 runs W2 as
two half-partition matmuls per PSUM bank (tile_position) so softplus /
stats / output work on full 128-partition tiles. Softplus = Exp then
Ln(x+1) on ACT with the BN affine fused in.
"""
import sys, os

sys.path.insert(0, "/opt/trn_rl_repo")

import numpy as np

import concourse.bass as bass
import concourse.bacc as bacc
import concourse.tile as tile
from concourse import mybir
from concourse.bass_utils import run_bass_kernel_spmd

F32 = mybir.dt.float32
BF16 = mybir.dt.bfloat16
I16 = mybir.dt.int16
AF = mybir.ActivationFunctionType
ALU = mybir.AluOpType
BF16_NP = mybir.dt.np(BF16)

NCORES = 8
N, M = 20000, 12
ATOM_F, NBR_F, ANG_F, A = 64, 64, 16, 66
H, O = 128, 64
BN_EPS = 1e-5
NLOC = N // NCORES          # 2500
NCHUNK = 5                  # chunks of 512 atoms (last ragged 452)
CW = 512
NPAD = NCHUNK * CW          # 2560
TAIL = NLOC - (NCHUNK - 1) * CW   # 452
NTILE = NCHUNK * M          # 60 bond tiles per core
NPAIR = NTILE // 2          # 30 paired tiles in phase 2/3
NSLAB = NTILE // 4          # 15 slabs of 4 bond-tiles
KA = 9                      # angle K-chunks of 128 (1056 padded to 1152)
NRANK = 160                 # table ranks (20000 atoms padded to 20480)
NFULL = NRANK * 128         # 20480
NIDX = M * CW               # 6144 gather indices per chunk (one call)
TGRP = 20                   # table build groups of 8 ranks

_CACHE = {}
TRACE = bool(int(os.environ.get("BASS_KERNEL_TRACE", "0")))
LAST_EXEC_NS = None
LAST_RESULTS = None


def _pin_act_tables():
    """Restrict the activation-table sets bacc may choose so Exp/Ln/Copy/
    Square all land in natural_log_exp_and_others (one load, no per-op
    table swaps). Set names/order (= act_func_set_id) are preserved."""
    if getattr(bacc, "_act_tables_pinned", False):
        return
    orig = bacc.get_activation_tables

    def pinned(arch):
        tabs = orig(arch)
        keep_all = "natural_log_exp_and_others"
        sqrt_home = "sqrt_and_others"
        strip = {AF.Exp, AF.Ln, AF.Copy, AF.Identity, AF.Square, AF.Sqrt}
        out = {}
        for name, funcs in tabs.items():
            if name == keep_all:
                out[name] = funcs
            elif name == sqrt_home:
                out[name] = {f for f in funcs
                             if f not in (strip - {AF.Sqrt})}
            else:
                out[name] = {f for f in funcs if f not in strip}
        return out

    bacc.get_activation_tables = pinned
    bacc._act_tables_pinned = True


def _build():
    if "nc" in _CACHE:
        return _CACHE["nc"]
    _pin_act_tables()
    import concourse.tile_utils as tile_utils
    tile_utils.max_sbuf_usage = 206 * 1024

    nc = bacc.Bacc("TRN2", target_bir_lowering=False, debug=False,
                   num_devices=NCORES, num_swdge_queues=4)

    atom_fullT = nc.dram_tensor("atom_fullT", [64, NFULL], BF16, kind="ExternalInput").ap()
    atom_locT = nc.dram_tensor("atom_locT", [64, NPAD], BF16, kind="ExternalInput").ap()
    angle_t = nc.dram_tensor("angle_t", [128, NCHUNK * KA * CW], BF16, kind="ExternalInput").ap()
    nbr_t = nc.dram_tensor("nbr_t", [128, NCHUNK * M * CW], BF16, kind="ExternalInput").ap()
    idx_w = nc.dram_tensor("idx_w", [128, NCHUNK * (NIDX // 16)], I16, kind="ExternalInput").ap()
    bw = nc.dram_tensor("bw", [128, NSLAB * 1024], BF16, kind="ExternalInput").ap()
    w1t_c = nc.dram_tensor("w1t_c", [64, 128], BF16, kind="ExternalInput").ap()
    w1t_n = nc.dram_tensor("w1t_n", [64, 128], BF16, kind="ExternalInput").ap()
    w1t_fr = nc.dram_tensor("w1t_fr", [128, 128], BF16, kind="ExternalInput").ap()
    w1t_a = nc.dram_tensor("w1t_a", [KA * 128, 128], BF16, kind="ExternalInput").ap()
    w2t = nc.dram_tensor("w2t", [128, 64], BF16, kind="ExternalInput").ap()
    ident = nc.dram_tensor("ident", [128, 128], BF16, kind="ExternalInput").ap()
    g1 = nc.dram_tensor("g1", [128, 1], F32, kind="ExternalInput").ap()
    be1 = nc.dram_tensor("be1", [128, 1], F32, kind="ExternalInput").ap()
    g2 = nc.dram_tensor("g2", [64, 1], F32, kind="ExternalInput").ap()
    be2 = nc.dram_tensor("be2", [64, 1], F32, kind="ExternalInput").ap()
    out_p = nc.dram_tensor("out_p", [128, NPAIR * 512], BF16, kind="ExternalOutput").ap()

    with tile.TileContext(nc) as tc:
        with (tc.tile_pool(name="consts", bufs=1) as consts,
              tc.tile_pool(name="astr", bufs=4) as astr_pl,
              tc.tile_pool(name="tb", bufs=8) as tb_pl,
              tc.tile_pool(name="atom", bufs=2) as atom_pl,
              tc.tile_pool(name="angle", bufs=2) as angle_pl,
              tc.tile_pool(name="nbr", bufs=2) as nbr_pl,
              tc.tile_pool(name="gath", bufs=3) as gath_pl,
              tc.tile_pool(name="absb", bufs=NCHUNK) as absb_pl,
              tc.tile_pool(name="h1", bufs=2) as h1_pl,
              tc.tile_pool(name="sp", bufs=4) as sp_pl,
              tc.tile_pool(name="sq", bufs=2) as sq_pl,
              tc.tile_pool(name="bwbc", bufs=6) as bwbc_pl,
              tc.tile_pool(name="psA", bufs=6, space="PSUM") as psA,
              tc.tile_pool(name="psB", bufs=2, space="PSUM") as psB,
              tc.tile_pool(name="dram", bufs=1, space="DRAM") as dram):

            # ---------------- constants -------------------------------
            # w1n first: the table build (critical path to the gathers)
            # needs only it plus the first atom piece
            w1n_sb = consts.tile([64, 128], BF16)
            nc.sync.dma_start(out=w1n_sb, in_=w1t_n)
            ident_sb = consts.tile([128, 128], BF16)
            nc.sync.dma_start(out=ident_sb, in_=ident)
            w1c_sb = consts.tile([64, 128], BF16)
            nc.sync.dma_start(out=w1c_sb, in_=w1t_c)
            w1fr_sb = consts.tile([128, 128], BF16)
            nc.sync.dma_start(out=w1fr_sb, in_=w1t_fr)
            w1a_sb = consts.tile([128, KA, 128], BF16)
            nc.sync.dma_start(
                out=w1a_sb,
                in_=bass.AP(tensor=w1t_a.tensor, offset=0,
                            ap=[[128, 128], [128 * 128, KA], [1, 128]]))
            w2t_sb = consts.tile([128, 64], BF16)
            nc.sync.dma_start(out=w2t_sb, in_=w2t)
            idx_sb = consts.tile([128, NCHUNK * (NIDX // 16)], I16)
            nc.sync.dma_start(out=idx_sb, in_=idx_w)
            g1_sb = consts.tile([128, 1], F32)
            nc.sync.dma_start(out=g1_sb, in_=g1)
            be1_sb = consts.tile([128, 1], F32)
            nc.sync.dma_start(out=be1_sb, in_=be1)
            g2_sb = consts.tile([64, 1], F32)
            nc.sync.dma_start(out=g2_sb, in_=g2)
            be2_sb = consts.tile([64, 1], F32)
            nc.sync.dma_start(out=be2_sb, in_=be2)
            eps_sb = consts.tile([128, 1], F32)
            nc.vector.memset(eps_sb, BN_EPS)

            s1 = consts.tile([128, NTILE], F32)
            q1 = consts.tile([128, NTILE], F32)
            stats2 = consts.tile([128, NPAIR, 6], F32)
            # z2 pair tiles are overlaid into z1 slots 4g / 4g+1, which are
            # dead once slab g's h1 is computed
            z1_sb = consts.tile([128, NTILE, 512], BF16)

            table_d = dram.tile([NFULL, 128], BF16)
            table_ap = bass.AP(tensor=table_d.tensor, offset=0,
                               ap=[[128, NFULL], [1, 128]])

            # ------- phase 0: bf16 projection table in DRAM -----------
            # table row a = atom_fea[a] @ W1n.T, built atom-major: one
            # matmul per 128-atom rank (atoms land on partitions), so the
            # store to DRAM is a plain contiguous-row DMA. Atom features
            # stream in 5 big pieces to amortize DMA latency.
            a_sb = None
            for grp in range(TGRP):
                if grp % 2 == 0:
                    a_sb = astr_pl.tile([64, 2048], BF16, tag="astr")
                    nc.sync.dma_start(
                        out=a_sb,
                        in_=atom_fullT[:, grp * 1024:(grp + 2) * 1024])
                base = (grp % 2) * 1024
                tb = tb_pl.tile([128, 8, 128], BF16, tag="tb")
                for half in range(2):
                    ps = psA.tile([128, 512], F32, tag="slot")
                    for k in range(4):
                        col = base + (half * 4 + k) * 128
                        nc.tensor.matmul(
                            ps[:, k * 128:(k + 1) * 128],
                            a_sb[:, col:col + 128],
                            w1n_sb[:], start=True, stop=True,
                            skip_group_check=True)
                    nc.vector.tensor_copy(
                        out=tb[:, half * 4:half * 4 + 4, :].rearrange(
                            "p a b -> p (a b)"),
                        in_=ps[:])
                nc.sync.dma_start(
                    out=bass.AP(tensor=table_d.tensor,
                                offset=grp * 8 * 128,
                                ap=[[NRANK * 128, 128], [1, 8 * 128]]),
                    in_=tb[:].rearrange("p a b -> p (a b)"))

            # ---------------- phase 1: z1 assembly + stats -------------
            # per-atom bases (center + angle) for ALL chunks first: this
            # is the only gather-independent PE work, and emitting it ahead
            # fills the PE idle window while the table DMAs land and the
            # first gather transfers.
            ab_sbs = []
            for c in range(NCHUNK):
                at_sb = atom_pl.tile([64, CW], BF16, tag="atom")
                nc.sync.dma_start(out=at_sb,
                                  in_=atom_locT[:, c * CW:(c + 1) * CW])
                ab = psB.tile([128, 512], F32, tag="psB")
                nc.tensor.matmul(ab[:], w1c_sb[:], at_sb[:],
                                 start=True, stop=False)
                an_sb = angle_pl.tile([128, KA, CW], BF16, tag="angle")
                nc.sync.dma_start(
                    out=an_sb[:].rearrange("p a b -> p (a b)"),
                    in_=angle_t[:, c * KA * CW:(c + 1) * KA * CW])
                for k in range(KA):
                    nc.tensor.matmul(ab[:], w1a_sb[:, k, :], an_sb[:, k, :],
                                     start=False, stop=(k == KA - 1))
                ab_sb = absb_pl.tile([128, 512], BF16, tag="absb")
                nc.scalar.copy(out=ab_sb[:], in_=ab[:])
                ab_sbs.append(ab_sb)
            for c in range(NCHUNK):
                valid = CW if c < NCHUNK - 1 else TAIL
                ab_sb = ab_sbs[c]
                # bond-major gathers, split across the 4 SWDGE queues so
                # 4 DMA rings pull table rows concurrently (3 slots each)
                gts = []
                for i in range(8):
                    gt = gath_pl.tile([128, 6, 128], BF16,
                                      tag=f"gath{i % 4}")
                    col = c * (NIDX // 16) + i * (NIDX // 128)
                    nc.gpsimd.dma_gather(
                        out_ap=gt[:], in_ap=table_ap,
                        idxs_ap=idx_sb[:, col:col + NIDX // 128],
                        num_idxs=NIDX // 8, num_idxs_reg=NIDX // 8,
                        elem_size=128, transpose=False, single_packet=False,
                        queue_num=i % 4)
                    gts.append(gt)
                # all 12 stacked [nbr_j; nbr_{j+1}] tiles in one DMA
                # (row M is a host-side copy of row 0 for wraparound)
                nbA = nbr_pl.tile([128, M, CW], BF16, tag="nbr")
                nc.sync.dma_start(
                    out=nbA[:].rearrange("p a b -> p (a b)"),
                    in_=nbr_t[:, c * M * CW:(c + 1) * M * CW])
                for j in range(M):
                    ps = psA.tile([128, 512], F32, tag="slot")
                    nc.tensor.matmul(ps[:], w1fr_sb[:], nbA[:, j, :],
                                     start=True, stop=False)
                    # transpose-inject gathered nbr_atom rows: G_block.T
                    for k in range(4):
                        B = 4 * j + k
                        nc.tensor.matmul(ps[:, k * 128:(k + 1) * 128],
                                         gts[B // 6][:, B % 6, :],
                                         ident_sb[:],
                                         start=False, stop=(k == 3))
                    t = c * M + j
                    z1t = z1_sb[:, t, :]
                    nc.vector.scalar_tensor_tensor(
                        out=z1t, in0=ps[:], scalar=1.0, in1=ab_sb[:],
                        op0=ALU.mult, op1=ALU.add,
                        accum_out=s1[:, t:t + 1])
                    sq = sq_pl.tile([128, 512], BF16, tag="sq")
                    nc.scalar.activation(out=sq[:], in_=z1t, func=AF.Square,
                                         accum_out=q1[:, t:t + 1])

            # ---------------- BN1 stats allreduce ----------------------
            # payload is plain [sum(x), sum(x^2)] per feature; pad bonds
            # gather the zero table row so full-width accums are exact
            pay1 = consts.tile([128, 2], F32)
            nc.vector.reduce_sum(out=pay1[:, 0:1], in_=s1[:],
                                 axis=mybir.AxisListType.X)
            nc.vector.reduce_sum(out=pay1[:, 1:2], in_=q1[:],
                                 axis=mybir.AxisListType.X)
            cc1i = dram.tile([128, 2], F32)
            cc1o = dram.tile([128, 2], F32)
            nc.sync.dma_start(out=cc1i[:], in_=pay1[:])
            nc.gpsimd.collective_compute(
                "AllReduce", ALU.add, replica_groups=[list(range(NCORES))],
                ins=[cc1i[:].opt()], outs=[cc1o[:].opt()])
            S1 = consts.tile([128, 2], F32)
            nc.sync.dma_start(out=S1[:], in_=cc1o[:])
            mean1 = consts.tile([128, 1], F32)
            nc.scalar.mul(out=mean1[:], in_=S1[:, 0:1], mul=1.0 / (N * M))
            mm1 = consts.tile([128, 1], F32)
            nc.scalar.square(out=mm1[:], in_=mean1[:])
            var1 = consts.tile([128, 1], F32)
            nc.vector.scalar_tensor_tensor(
                out=var1[:], in0=S1[:, 1:2], scalar=1.0 / (N * M), in1=mm1[:],
                op0=ALU.mult, op1=ALU.subtract)
            sd1 = consts.tile([128, 1], F32)
            nc.scalar.activation(out=sd1[:], in_=var1[:], func=AF.Sqrt,
                                 bias=eps_sb[:], scale=1.0)
            rs1 = consts.tile([128, 1], F32)
            nc.vector.reciprocal(out=rs1[:], in_=sd1[:])
            scale1 = consts.tile([128, 1], F32)
            nc.vector.tensor_mul(out=scale1[:], in0=rs1[:], in1=g1_sb[:])
            negm1 = consts.tile([128, 1], F32)
            nc.scalar.mul(out=negm1[:], in_=mean1[:], mul=-1.0)
            bias1 = consts.tile([128, 1], F32)
            nc.vector.scalar_tensor_tensor(
                out=bias1[:], in0=scale1[:], scalar=negm1[:], in1=be1_sb[:],
                op0=ALU.mult, op1=ALU.add)

            # ---------------- phase 2: h1, z2, stats2 ------------------
            # softplus per 4-tile slab; W2 as two half-partition matmuls
            # per PSUM bank so downstream tiles are full 128 partitions
            # (partitions 0:64 <- even tile features, 64:128 <- odd).
            for g in range(NSLAB):
                zsl = z1_sb[:, 4 * g:4 * g + 4, :].rearrange("p a b -> p (a b)")
                nc.scalar.activation(out=zsl, in_=zsl, func=AF.Exp,
                                     bias=bias1[:], scale=scale1[:])
                h1s = h1_pl.tile([128, 2048], BF16, tag="h1")
                nc.scalar.activation(out=h1s[:], in_=zsl, func=AF.Ln,
                                     bias=1.0)
                for m in range(2):
                    t = 2 * g + m
                    c = (4 * g + 2 * m) // M
                    valid = CW if c < NCHUNK - 1 else TAIL
                    ps = psB.tile([128, 512], F32, tag="psB")
                    nc.tensor.matmul(ps[0:64, :], w2t_sb[:],
                                     h1s[:, (2 * m) * 512:(2 * m + 1) * 512],
                                     start=True, stop=True,
                                     skip_group_check=True)
                    nc.tensor.matmul(ps[64:128, :], w2t_sb[:],
                                     h1s[:, (2 * m + 1) * 512:(2 * m + 2) * 512],
                                     start=True, stop=True,
                                     skip_group_check=True)
                    nc.vector.bn_stats(out=stats2[:, t, :],
                                       in_=ps[:, 0:valid])
                    nc.vector.tensor_copy(out=z1_sb[:, 4 * g + m, :],
                                          in_=ps[:])

            # ---------------- BN2 stats allreduce ----------------------
            mv2 = consts.tile([128, 2], F32)
            nc.vector.bn_aggr(out=mv2[:], in_=stats2[:])
            pay2 = consts.tile([128, 2], F32)
            msq2 = consts.tile([128, 1], F32)
            nc.scalar.square(out=msq2[:], in_=mv2[:, 0:1])
            nc.vector.tensor_copy(out=pay2[:, 0:1], in_=mv2[:, 0:1])
            nc.vector.tensor_add(out=pay2[:, 1:2], in0=mv2[:, 1:2], in1=msq2[:])
            cc2i = dram.tile([128, 2], F32)
            cc2o = dram.tile([128, 2], F32)
            nc.sync.dma_start(out=cc2i[:], in_=pay2[:])
            nc.gpsimd.collective_compute(
                "AllReduce", ALU.add, replica_groups=[list(range(NCORES))],
                ins=[cc2i[:].opt()], outs=[cc2o[:].opt()])
            # prefetch phase-3 bond weights during the collective (Pool
            # queue: idle here and dispatch is cheap)
            bwts = []
            for g in range(NSLAB):
                bwt = bwbc_pl.tile([128, 2, 512], BF16, tag="bwbc")
                nc.sync.dma_start(
                    out=bwt[:].rearrange("p a b -> p (a b)"),
                    in_=bw[:, g * 1024:(g + 1) * 1024])
                bwts.append(bwt)
            S2 = consts.tile([128, 2], F32)
            nc.sync.dma_start(out=S2[:], in_=cc2o[:])
            # fold: partitions 64:128 hold the odd-tile half of each
            # feature's stats; shift down and add for the global sums
            S2s = consts.tile([64, 2], F32)
            nc.sync.dma_start(out=S2s[:], in_=S2[64:128, :])
            S2t = consts.tile([64, 2], F32)
            nc.vector.tensor_add(out=S2t[:], in0=S2[0:64, :], in1=S2s[:])
            mean2 = consts.tile([64, 1], F32)
            nc.scalar.mul(out=mean2[:], in_=S2t[:, 0:1], mul=1.0 / (2 * NCORES))
            mm2 = consts.tile([64, 1], F32)
            nc.scalar.square(out=mm2[:], in_=mean2[:])
            var2 = consts.tile([64, 1], F32)
            nc.vector.scalar_tensor_tensor(
                out=var2[:], in0=S2t[:, 1:2], scalar=1.0 / (2 * NCORES),
                in1=mm2[:], op0=ALU.mult, op1=ALU.subtract)
            sd2 = consts.tile([64, 1], F32)
            nc.scalar.activation(out=sd2[:], in_=var2[:], func=AF.Sqrt,
                                 bias=eps_sb[0:64, :], scale=1.0)
            rs2 = consts.tile([64, 1], F32)
            nc.vector.reciprocal(out=rs2[:], in_=sd2[:])
            scale2 = consts.tile([64, 1], F32)
            nc.vector.tensor_mul(out=scale2[:], in0=rs2[:], in1=g2_sb[:])
            negm2 = consts.tile([64, 1], F32)
            nc.scalar.mul(out=negm2[:], in_=mean2[:], mul=-1.0)
            bias2 = consts.tile([64, 1], F32)
            nc.vector.scalar_tensor_tensor(
                out=bias2[:], in0=scale2[:], scalar=negm2[:], in1=be2_sb[:],
                op0=ALU.mult, op1=ALU.add)
            scale2r = consts.tile([128, 1], F32)
            nc.sync.dma_start(out=scale2r[0:64, :], in_=scale2[:])
            nc.sync.dma_start(out=scale2r[64:128, :], in_=scale2[:])
            bias2r = consts.tile([128, 1], F32)
            nc.sync.dma_start(out=bias2r[0:64, :], in_=bias2[:])
            nc.sync.dma_start(out=bias2r[64:128, :], in_=bias2[:])

            # ---------------- phase 3: softplus2 * bw -> out -----------
            for g in range(NSLAB):
                zf = z1_sb[:, 4 * g:4 * g + 2, :].rearrange("p a b -> p (a b)")
                nc.scalar.activation(out=zf, in_=zf, func=AF.Exp,
                                     bias=bias2r[:], scale=scale2r[:])
                sp = sp_pl.tile([128, 1024], BF16, tag="sp")
                nc.scalar.activation(out=sp[:], in_=zf, func=AF.Ln, bias=1.0)
                nc.vector.tensor_mul(
                    out=sp[:], in0=sp[:],
                    in1=bwts[g][:].rearrange("p a b -> p (a b)"))
                nc.sync.dma_start(
                    out=out_p[:, g * 1024:(g + 1) * 1024], in_=sp[:])

    nc.compile()
    _CACHE["nc"] = nc
    return nc


def _prep_core(c, atom_fea, nbr_fea, nbr_fea_idx, angle_fea, bond_weights,
               shared):
    lo = c * NLOC
    hi = lo + NLOC
    atom_locT = np.zeros((64, NPAD), BF16_NP)
    atom_locT[:, :NLOC] = atom_fea[lo:hi].T.astype(BF16_NP)

    # angle chunk-major: [128, NCHUNK, KA, 512], one contiguous DMA run
    # per partition per chunk
    ang = np.zeros((KA * 128, NPAD), np.float32)
    ang[:A * ANG_F, :NLOC] = angle_fea[lo:hi].reshape(NLOC, A * ANG_F).T
    angle_t = np.ascontiguousarray(
        ang.reshape(KA, 128, NCHUNK, CW).transpose(1, 2, 0, 3)
    ).reshape(128, NCHUNK * KA * CW).astype(BF16_NP)

    # nbr stacked [nbr_j; nbr_{j+1}] chunk-major: [128, NCHUNK, M, 512]
    nb = np.zeros((M, 64, NPAD), np.float32)
    nb[:, :, :NLOC] = nbr_fea[lo:hi].transpose(1, 2, 0)
    nb = nb.reshape(M, 64, NCHUNK, CW)
    top = nb.transpose(1, 2, 0, 3)                    # [64, NCHUNK, M, CW]
    bot = np.roll(nb, -1, axis=0).transpose(1, 2, 0, 3)
    nbr_t = np.ascontiguousarray(
        np.concatenate([top, bot], axis=0)
    ).reshape(128, NCHUNK * M * CW).astype(BF16_NP)

    # bond weights pre-broadcast in phase-3 pair layout [128, NSLAB, 1024]
    bwf = np.zeros((NPAD, M), np.float32)
    bwf[:NLOC] = bond_weights[lo:hi]
    bw_p = np.zeros((128, NSLAB, 2, CW), np.float32)
    for g in range(NSLAB):
        for m in range(2):
            cc, j0 = divmod(2 * (2 * g + m), M)
            bw_p[0:64, g, m, :] = bwf[cc * CW:(cc + 1) * CW, j0]
            bw_p[64:128, g, m, :] = bwf[cc * CW:(cc + 1) * CW, j0 + 1]
    bw_p = bw_p.reshape(128, NSLAB * 1024).astype(BF16_NP)

    idxp = np.full((NPAD, M), N, np.int32)   # pad bonds -> zero table row
    idxp[:NLOC] = nbr_fea_idx[lo:hi].astype(np.int32)
    # remap to the partition-major table layout: atom a lives at DRAM row
    # (a % 128) * NRANK + a // 128
    idxp = ((idxp % 128) * NRANK + idxp // 128).astype(np.int16)
    idx_w = np.zeros((128, NCHUNK * (NIDX // 16)), np.int16)
    for cc in range(NCHUNK):
        flat = idxp[cc * CW:(cc + 1) * CW, :].T.reshape(-1)   # slot-major
        wr = flat.reshape(NIDX // 16, 16).T                   # (16, 384)
        col = cc * (NIDX // 16)
        idx_w[:, col:col + NIDX // 16] = np.tile(wr, (8, 1))

    d = dict(shared)
    d.update(atom_locT=atom_locT, angle_t=angle_t, nbr_t=nbr_t, idx_w=idx_w,
             bw=bw_p)
    return d


def _make_in_maps(inputs):
    """Build per-core input dicts from the full (unsharded) input dict."""
    atom_fea = np.asarray(inputs["atom_fea"], dtype=np.float32)
    nbr_fea = np.asarray(inputs["nbr_fea"], dtype=np.float32)
    nbr_fea_idx = np.asarray(inputs["nbr_fea_idx"])
    angle_fea = np.asarray(inputs["angle_fea"], dtype=np.float32)
    bond_weights = np.asarray(inputs["bond_weights"], dtype=np.float32)
    W1 = np.asarray(inputs["W1"]); W2 = np.asarray(inputs["W2"])
    g1 = np.asarray(inputs["g1"]); be1 = np.asarray(inputs["be1"])
    g2 = np.asarray(inputs["g2"]); be2 = np.asarray(inputs["be2"])

    atom_fullT = np.zeros((64, NFULL), BF16_NP)
    atom_fullT[:, :N] = atom_fea.T.astype(BF16_NP)
    w1t = W1.T.astype(np.float32)
    w1t_a = np.zeros((KA * 128, 128), BF16_NP)
    w1t_a[:A * ANG_F] = w1t[256:1312].astype(BF16_NP)
    shared = dict(
        atom_fullT=atom_fullT,
        w1t_c=np.ascontiguousarray(w1t[0:64]).astype(BF16_NP),
        w1t_n=np.ascontiguousarray(w1t[64:128]).astype(BF16_NP),
        w1t_fr=np.ascontiguousarray(w1t[128:256]).astype(BF16_NP),
        w1t_a=w1t_a,
        w2t=np.ascontiguousarray(W2.T).astype(BF16_NP),
        ident=np.eye(128, dtype=np.float32).astype(BF16_NP),
        g1=g1.reshape(128, 1).astype(np.float32),
        be1=be1.reshape(128, 1).astype(np.float32),
        g2=g2.reshape(64, 1).astype(np.float32),
        be2=be2.reshape(64, 1).astype(np.float32),
    )
    return [_prep_core(c, atom_fea, nbr_fea, nbr_fea_idx, angle_fea,
                       bond_weights, shared)
            for c in range(NCORES)]


def _assemble(results):
    """Per-core out_p buffers -> full (N, M, NBR_F) output."""
    out = np.empty((N, M, NBR_F), np.float32)
    for c in range(NCORES):
        op = np.asarray(results[c]["out_p"]).astype(np.float32)
        lo = c * NLOC
        for t in range(NPAIR):
            cc, j0 = divmod(2 * t, M)
            blk = op[:, t * 512:(t + 1) * 512]
            a0 = cc * CW
            nA = min(CW, NLOC - a0)
            out[lo + a0:lo + a0 + nA, j0, :] = blk[0:64, :nA].T
            out[lo + a0:lo + a0 + nA, j0 + 1, :] = blk[64:128, :nA].T
    return out


def kernel(atom_fea, nbr_fea, nbr_fea_idx, angle_fea, bond_weights,
           W1, b1, g1, be1, W2, b2, g2, be2):
    global LAST_EXEC_NS, LAST_RESULTS
    nc = _build()
    in_maps = _make_in_maps(dict(
        atom_fea=atom_fea, nbr_fea=nbr_fea, nbr_fea_idx=nbr_fea_idx,
        angle_fea=angle_fea, bond_weights=bond_weights, W1=W1, W2=W2,
        g1=g1, be1=be1, g2=g2, be2=be2))

    if TRACE:
        _install_ntff_hook()
    br = run_bass_kernel_spmd(nc, in_maps, list(range(NCORES)), trace=TRACE)
    LAST_EXEC_NS = br.exec_time_ns
    LAST_RESULTS = br
    return _assemble(br.results)


def _install_ntff_hook():
    """Inject antenv.axon_hooks (missing in this image) so trace=True works."""
    import types
    if "antenv.axon_hooks" in sys.modules:
        return
    sys.path.insert(0, "/root/.axon_site")
    mod = types.ModuleType("antenv.axon_hooks")
    mod._hook = None
    mod.set_axon_ntff_profile_hook = lambda h: setattr(mod, "_hook", h)
    mod.get_axon_ntff_profile_hook = lambda: mod._hook
    sys.modules["antenv.axon_hooks"] = mod
    try:
        from trn_agent_boot.trn_boot import _ntff_profile_via_ctypes
        h = _ntff_profile_via_ctypes("/opt/axon/libaxon_pjrt.so")
        if h is not None:
            mod.set_axon_ntff_profile_hook(h)
    except Exception as e:
        print("ntff hook install failed:", e)
